# revision 1
# baseline (speedup 1.0000x reference)
"""AttentionBlock (1x1-conv QKV + 4-head softmax attention + 1x1-conv proj)
on 8 Trainium2 NeuronCores.

Sharding: data-parallel over (batch b, query-half h) -> 8 shards. Each core
gets x rotated so its 2048 query columns are always columns 0:2048 (key order
is a permutation, which softmax-attention is invariant to), computes
qkv projections, 4-head attention for its half of the queries, and the output
projection for its [256, 2048] output slice. No collectives.

Core kernel tricks:
  - all matmuls in float32r (full-rate PE, ~1.5e-4 rel rounding)
  - S^T = K^T Q with two heads row-tiled in the PE array (K=64 each)
  - exp of scores: half the heads on the Scalar engine (exact), half via a
    fused custom DVE op ((x+c0)((x+c1)x+c2))^8 ~ C*e^x (scale cancels in
    softmax; assignment is per-(head, query-tile) so rows stay consistent)
  - attn @ V with two heads col-tiled, plus 4-way col-tiled ones-matmul
    rowsums accumulated in PSUM
  - softmax normalization via reciprocal + a tiny broadcast matmul
"""
import sys

sys.path.insert(0, '/opt/trn_rl_repo')

import numpy as np
from contextlib import ExitStack

from concourse import bass, bacc, mybir
import concourse.tile as tile
from concourse import dve_ops
from concourse.dve_ops import DveOp, OPS, CUSTOM_DVE_SPECS, _SUB_OPCODE_FOR_NAME
from concourse.dve_spec import Spec, Src0, Src1, C0, C1, C2, C3, lower, sq, _spill_c3_to_src1
from concourse.dve_uop import DveOpSpec
from concourse.bass_utils import run_bass_kernel_spmd

F32 = mybir.dt.float32
F32R = mybir.dt.float32r
BF16 = mybir.dt.bfloat16
ActFn = mybir.ActivationFunctionType

B, C, H, W = 4, 256, 64, 64
HEADS, DH = 4, 64
N = H * W            # 4096 keys
NQ = N // 2          # 2048 queries per core
NT = 512             # query tile (one PSUM bank of fp32)
N_NT = NQ // NT      # 4 query tiles
N_MC = N // 128      # 32 key chunks

# exp(x) ~ C * [q3(x) * (x^2 + b0 x + b1)]^16 over x in [-8.8, 8.4]
# (max rel err 3.3e-4; the constant C cancels in softmax normalization).
# Two DVE instructions: EXP5A computes the cubic q3, EXP5B multiplies by the
# monic quadratic and raises to the 16th power.
EXP_A = (0.00039684202121525346, 2.589769573122113e-05,
         6.891462469732395e-07, 7.771052073346383e-09)   # a0..a3
EXP_B = (-6.95331830849084, 2519.7822812996437)          # b0, b1


def _ref_exp5a(in0, in1, c0, c1, c2):
    x = in0.astype(np.float32)
    a3 = in1.astype(np.float32) if isinstance(in1, np.ndarray) else np.float32(in1)
    return (((a3 * x + np.float32(c2)) * x + np.float32(c1)) * x
            + np.float32(c0)).astype(np.float32)


def _ref_exp5b(in0, in1, c0, c1, c2):
    x = in0.astype(np.float32)
    q3 = in1.astype(np.float32)
    p = (q3 * ((x + np.float32(c0)) * x + np.float32(c1))).astype(np.float32)
    for _ in range(4):
        p = (p * p).astype(np.float32)
    return p


def _register(name, spec, rd1_en):
    row = dve_ops._CUSTOM_DVE_ROW_BASE + len(OPS)
    assert row < 0x20
    _SUB_OPCODE_FOR_NAME[name] = row
    shas = {}
    for ver in ("v3", "v4"):
        uops = lower(spec, ver=ver)
        shas[ver] = DveOpSpec(name=name, opcode=row, uops=uops, rd1_en=rd1_en).sha(ver)
    op = DveOp(name, spec, subdim=False, uops_sha=shas)
    OPS.append(op)
    CUSTOM_DVE_SPECS[name] = spec
    return op


def register_exp_op():
    if "EXP5A_ANT" in _SUB_OPCODE_FOR_NAME:
        a = next(op for op in OPS if op.name == "EXP5A_ANT")
        b = next(op for op in OPS if op.name == "EXP5B_ANT")
        return a, b
    x = Src0
    body_a = _spill_c3_to_src1(((C3 * x + C2) * x + C1) * x + C0)
    op_a = _register("EXP5A_ANT", Spec(body=body_a, reference=_ref_exp5a), True)
    body_b = sq(sq(sq(sq(Src1 * ((x + C0) * x + C1)))))
    op_b = _register("EXP5B_ANT", Spec(body=body_b, reference=_ref_exp5b), True)
    return op_a, op_b


def emit_exp_dve(nc, ops, out, in_, y1, a3_t):
    op_a, op_b = ops
    nc.vector._custom_dve(op_a, out=y1, in0=in_, in1=a3_t,
                          s0=float(EXP_A[0]), s1=float(EXP_A[1]), imm2=float(EXP_A[2]))
    return nc.vector._custom_dve(op_b, out=out, in0=in_, in1=y1,
                                 s0=float(EXP_B[0]), s1=float(EXP_B[1]))


# exp-engine split: ACT computes pair-0 tiles fully plus the first EXP_N0
# query-columns of each pair-1 head; the DVE two-op pipeline takes the rest.
# Constant per (pair, nt, n-range) so every softmax row uses one implementation.
import os as _os
EXP_N0 = int(_os.environ.get("EXP_N0", "192"))


def build_program(exp_op):
    nc = bacc.Bacc(target_bir_lowering=False)

    x_d = nc.declare_dram_parameter("x", [C, N], F32, isOutput=False)
    wq_d = nc.declare_dram_parameter("wq", [C, C], F32, isOutput=False)
    wk_d = nc.declare_dram_parameter("wk", [C, C], F32, isOutput=False)
    wv_d = nc.declare_dram_parameter("wv", [C, C], F32, isOutput=False)
    wp_d = nc.declare_dram_parameter("wp", [C, C], F32, isOutput=False)
    bias_d = nc.declare_dram_parameter("bias", [128, 2], F32, isOutput=False)
    y_d = nc.declare_dram_parameter("y", [C, NQ], F32, isOutput=True)
    import os as _os
    _DBG = bool(int(_os.environ.get("KERNEL_DEBUG", "0")))
    dbg = {}
    if _DBG:
        BF16_ = mybir.dt.bfloat16
        dbg["q0"] = nc.declare_dram_parameter("dbg_q0", [128, 512], F32, isOutput=True)
        dbg["st0"] = nc.declare_dram_parameter("dbg_st0", [128, 1024], F32, isOutput=True)
        dbg["e0"] = nc.declare_dram_parameter("dbg_e0", [128, 1024], BF16_, isOutput=True)
        dbg["e1"] = nc.declare_dram_parameter("dbg_e1", [128, 1024], BF16_, isOutput=True)
        dbg["rs"] = nc.declare_dram_parameter("dbg_rs", [128, 512], F32, isOutput=True)
        dbg["rsinv"] = nc.declare_dram_parameter("dbg_rsinv", [128, 512], F32, isOutput=True)
        dbg["rb0"] = nc.declare_dram_parameter("dbg_rb0", [128, 512], F32, isOutput=True)
        dbg["rb1"] = nc.declare_dram_parameter("dbg_rb1", [128, 512], F32, isOutput=True)
        dbg["po0"] = nc.declare_dram_parameter("dbg_po0", [128, 512], F32, isOutput=True)
        dbg["vt0"] = nc.declare_dram_parameter("dbg_vt0", [128, 256], BF16_, isOutput=True)

    with tile.TileContext(nc) as tc, ExitStack() as ctx:
        sb = ctx.enter_context(tc.tile_pool(name="sb", bufs=1))
        pex = ctx.enter_context(tc.tile_pool(name="pex", bufs=3))
        pout = ctx.enter_context(tc.tile_pool(name="pout", bufs=2))
        ps = ctx.enter_context(tc.tile_pool(name="ps", bufs=1, space="PSUM"))

        # ---------------- load + round inputs to f32r ----------------
        x_f = [sb.tile([128, N], F32, tag=f"xf{i}", name=f"xf{i}") for i in range(2)]
        x_r = [sb.tile([128, N], F32R, tag=f"xr{i}", name=f"xr{i}") for i in range(2)]
        for kc in range(2):
            nc.sync.dma_start(out=x_f[kc], in_=x_d[kc * 128:(kc + 1) * 128, :])
        nc.scalar.copy(x_r[0][:, :], x_f[0][:, :])
        nc.vector.tensor_copy(x_r[1][:, :], x_f[1][:, :])

        w_sb = {}
        for name, dram in (("wq", wq_d), ("wk", wk_d), ("wv", wv_d), ("wp", wp_d)):
            tiles = []
            for kc in range(2):
                f = sb.tile([128, C], F32, tag="wf", name=f"{name}f{kc}")
                nc.sync.dma_start(out=f, in_=dram[kc * 128:(kc + 1) * 128, :])
                r = sb.tile([128, C], F32R, tag=f"{name}{kc}", name=f"{name}r{kc}")
                (nc.vector.tensor_copy if kc else nc.scalar.copy)(r[:, :], f[:, :])
                tiles.append(r)
            w_sb[name] = tiles
        bias_sb = sb.tile([128, 2], F32, tag="bias")
        nc.sync.dma_start(out=bias_sb, in_=bias_d[:, :])

        # constants: ones column + broadcast matrices
        ones = sb.tile([128, 1], BF16, tag="ones")
        nc.vector.memset(ones, 1.0)
        a3_t = sb.tile([128, 1], F32, tag="a3")
        nc.vector.memset(a3_t, float(EXP_A[3]))
        zero_f = sb.tile([128, 512], F32, tag="zerof")
        nc.vector.memset(zero_f, 0.0)
        # broadcast matrix: out[m, n] = rhs[32*head(m) + 64*oc, n]
        bc_f = sb.tile([128, 256], F32, tag="bc_f")
        nc.vector.memset(bc_f, 0.0)
        nc.vector.memset(bc_f[0:1, 0:64], 1.0)
        nc.vector.memset(bc_f[32:33, 64:128], 1.0)
        nc.vector.memset(bc_f[64:65, 128:192], 1.0)
        nc.vector.memset(bc_f[96:97, 192:256], 1.0)
        bc = sb.tile([128, 256], F32R, tag="bc")
        nc.vector.tensor_copy(bc, bc_f[:, :])


        # ---------------- phase 1: qkv projections ----------------
        q_sb = [sb.tile([128, NQ], F32R, tag=f"q{oc}", name=f"q_sb{oc}") for oc in range(2)]
        k_sb = [sb.tile([128, N], F32R, tag=f"k{oc}", name=f"k_sb{oc}") for oc in range(2)]
        vT_sb = sb.tile([128, N_MC * 256], BF16, tag="vT")

        for oc in range(2):
            for nt in range(N_NT):
                pq = ps.tile([128, 512], F32, tag="s", bufs=2, name=f"pq{oc}_{nt}")
                sl = slice(nt * 512, (nt + 1) * 512)
                nc.tensor.matmul(out=pq[:, :], lhsT=w_sb["wq"][0][:, oc * 128:(oc + 1) * 128],
                                 rhs=x_r[0][:, sl], start=True, stop=False)
                nc.tensor.matmul(out=pq[:, :], lhsT=w_sb["wq"][1][:, oc * 128:(oc + 1) * 128],
                                 rhs=x_r[1][:, sl], start=False, stop=True)
                (nc.scalar.copy if (oc + nt) % 2 else nc.vector.tensor_copy)(q_sb[oc][:, sl], pq[:, :])
                if _DBG and oc == 0 and nt == 0:
                    nc.sync.dma_start(out=dbg["q0"][:, :], in_=q_sb[0][:, 0:512].bitcast(F32))
        for oc in range(2):
            for nt in range(2 * N_NT):
                pk = ps.tile([128, 512], F32, tag="s", bufs=2, name=f"pk{oc}_{nt}")
                sl = slice(nt * 512, (nt + 1) * 512)
                nc.tensor.matmul(out=pk[:, :], lhsT=w_sb["wk"][0][:, oc * 128:(oc + 1) * 128],
                                 rhs=x_r[0][:, sl], start=True, stop=False)
                nc.tensor.matmul(out=pk[:, :], lhsT=w_sb["wk"][1][:, oc * 128:(oc + 1) * 128],
                                 rhs=x_r[1][:, sl], start=False, stop=True)
                (nc.vector.tensor_copy if nt % 2 else nc.scalar.copy)(k_sb[oc][:, sl], pk[:, :])
        for mc in range(N_MC):
            pv = ps.tile([128, 256], F32, tag="rs", name=f"pv{mc}")
            msl = slice(mc * 128, (mc + 1) * 128)
            nc.tensor.matmul(out=pv[:, :], lhsT=x_r[0][:, msl], rhs=w_sb["wv"][0][:, :],
                             start=True, stop=False)
            nc.tensor.matmul(out=pv[:, :], lhsT=x_r[1][:, msl], rhs=w_sb["wv"][1][:, :],
                             start=False, stop=True)
            (nc.vector.tensor_copy if mc % 2 else nc.scalar.copy)(
                vT_sb[:, mc * 256:(mc + 1) * 256], pv[:, :])
            if _DBG and mc == 0:
                nc.sync.dma_start(out=dbg["vt0"][:, :], in_=vT_sb[:, 0:256])

        import os
        _PH = int(os.environ.get("KERNEL_PHASES", "3"))
        # ---------------- phase 2: attention ----------------
        out_sp = [sb.tile([128, NQ], F32R, tag=f"osp{oc}", name=f"out_sp{oc}") for oc in range(2)]

        if _PH < 2:
            zero_f = sb.tile([128, 512], F32, tag="zero_f")
            nc.vector.memset(zero_f, 0.0)
            for oc in range(2):
                for z in range(4):
                    nc.vector.tensor_copy(out_sp[oc][:, z * 512:(z + 1) * 512], zero_f[:, :])
        for nt in range(N_NT if _PH >= 2 else 0):
            qsl = slice(nt * 512, (nt + 1) * 512)
            po = [ps.tile([128, 512], F32, tag="o", bufs=3, name=f"po{pair}_{nt}") for pair in (0, 1)]
            prs = ps.tile([128, 512], F32, tag="rs", name=f"prs{nt}")
            for mc in range(N_MC):
                msl = slice(mc * 128, (mc + 1) * 128)
                exps = []
                for pair in (0, 1):
                    pst = ps.tile([128, 1024], F32, tag="s", bufs=2, name=f"pst{pair}_{nt}_{mc}")
                    # S^T: two heads row-tiled (dh=64 each)
                    nc.tensor.matmul(out=pst[:, 0:512],
                                     lhsT=k_sb[pair][0:64, msl], rhs=q_sb[pair][0:64, qsl],
                                     start=True, stop=True, tile_position=(0, 0))
                    nc.tensor.matmul(out=pst[:, 512:1024],
                                     lhsT=k_sb[pair][64:128, msl], rhs=q_sb[pair][64:128, qsl],
                                     start=True, stop=True, tile_position=(64, 0))
                    et = pex.tile([128, 1024], BF16, tag=f"e{pair}", name=f"et{pair}_{nt}_{mc}")
                    if pair == 0 or EXP_N0 >= 512:
                        nc.scalar.activation(et[:, :], pst[:, :], ActFn.Exp)
                    elif EXP_N0 == 0:
                        y1 = pex.tile([128, 1024], F32, tag="y1", name=f"y1_{pair}_{nt}_{mc}")
                        emit_exp_dve(nc, exp_op, et[:, :], pst[:, :], y1[:, :], a3_t[:, :])
                    else:
                        # strided APs covering (h2 cols [a:b]) u (h3 cols [512+a:512+b])
                        def _two(ap_t, a, b):
                            base = ap_t[:, a:b]
                            return bass.AP(tensor=base.tensor, offset=base.offset,
                                           ap=[list(base.ap[0]), [512, 2], [1, b - a]])
                        nc.scalar.activation(_two(et, 0, EXP_N0), _two(pst, 0, EXP_N0),
                                             ActFn.Exp)
                        y1 = pex.tile([128, 1024], F32, tag="y1", name=f"y1_{pair}_{nt}_{mc}")
                        emit_exp_dve(nc, exp_op, _two(et, EXP_N0, 512),
                                     _two(pst, EXP_N0, 512), _two(y1, EXP_N0, 512),
                                     a3_t[:, :])
                    if _DBG and nt == 0 and mc == 0:
                        nc.sync.dma_start(out=dbg[f"e{pair}"][:, :], in_=et[:, :])
                        if pair == 0:
                            st_f = sb.tile([128, 1024], F32, tag="dbg_st", name="dbg_st_t")
                            nc.vector.tensor_copy(st_f, pst[:, :])
                            nc.sync.dma_start(out=dbg["st0"][:, :], in_=st_f)
                    exps.append(et)
                first, last = mc == 0, mc == N_MC - 1
                for pair in (0, 1):
                    vb = mc * 256 + pair * 128
                    nc.tensor.matmul(out=po[pair][0:64, :],
                                     lhsT=vT_sb[:, vb:vb + 64], rhs=exps[pair][:, 0:512],
                                     start=first, stop=last, tile_position=(0, 0))
                    nc.tensor.matmul(out=po[pair][64:128, :],
                                     lhsT=vT_sb[:, vb + 64:vb + 128], rhs=exps[pair][:, 512:1024],
                                     start=first, stop=last, tile_position=(0, 64))
                for hh in range(4):
                    nc.tensor.matmul(out=prs[32 * hh:32 * hh + 1, :],
                                     lhsT=ones[:, :], rhs=exps[hh // 2][:, (hh % 2) * 512:(hh % 2 + 1) * 512],
                                     start=first, stop=last, tile_position=(0, 32 * hh))
            if _DBG and nt == 0:
                rs_f = sb.tile([128, 512], F32, tag="dbg_rs", name="dbg_rs_t")
                nc.vector.tensor_copy(rs_f[0:1, :], prs[0:1, :])
                nc.vector.tensor_copy(rs_f[32:33, :], prs[32:33, :])
                nc.vector.tensor_copy(rs_f[64:65, :], prs[64:65, :])
                nc.vector.tensor_copy(rs_f[96:97, :], prs[96:97, :])
                nc.sync.dma_start(out=dbg["rs"][:, :], in_=rs_f)
                po_f = sb.tile([128, 512], F32, tag="dbg_po", name="dbg_po_t")
                nc.vector.tensor_copy(po_f, po[1][:, :])
                nc.sync.dma_start(out=dbg["po0"][:, :], in_=po_f)
            # normalization: copy the 4 rowsum rows to SBUF (ACT, f32r), matmul
            # against the 0/1 broadcast matrix to replicate each head's rowsum
            # to its 64 output partitions, evacuate to SBUF, reciprocal, multiply.
            # (custom DVE ops and partition_broadcast only work at base 0.)
            rs_sb = sb.tile([128, 512], F32R, tag="rs_sb", name=f"rs_sb{nt}")
            # zero-fill: the broadcast matmul reads all 128 partitions and
            # uninitialized SBUF can contain NaNs (0 * NaN = NaN)
            nc.vector.tensor_copy(rs_sb[:, :], zero_f[:, :])
            for hh in range(4):
                nc.scalar.copy(rs_sb[32 * hh:32 * hh + 1, :], prs[32 * hh:32 * hh + 1, :])
            for oc in range(2):
                pb = ps.tile([128, 512], F32, tag="s", bufs=2, name=f"pb{oc}_{nt}")
                nc.tensor.matmul(out=pb[:, :], lhsT=bc[:, oc * 128:(oc + 1) * 128],
                                 rhs=rs_sb[:, :], start=True, stop=True)
                rbr = sb.tile([128, 512], F32, tag="rbr", name=f"rbr{oc}_{nt}")
                nc.scalar.copy(rbr[:, :], pb[:, :])
                rb = sb.tile([128, 512], F32, tag="rb", name=f"rb{oc}_{nt}")
                nc.vector.reciprocal_approx_fast(out=rb[:, :], in_=rbr[:, :])
                nc.vector.tensor_tensor(
                    out=out_sp[oc][:, qsl], in0=po[oc][:, :], in1=rb[:, :],
                    op=mybir.AluOpType.mult)
                if _DBG and nt == 0:
                    nc.sync.dma_start(out=dbg[f"rb{oc}"][:, :], in_=rb[:, :])
            if _DBG and nt == 0:
                nc.sync.dma_start(out=dbg["rsinv"][:, :], in_=rs_sb[:, :])

        # ---------------- phase 3: output projection + bias ----------------
        for oc in range(2):
            for nt in range(N_NT):
                sl = slice(nt * 512, (nt + 1) * 512)
                py = ps.tile([128, 512], F32, tag="o", bufs=3, name=f"py{oc}_{nt}")
                nc.tensor.matmul(out=py[:, :], lhsT=w_sb["wp"][0][:, oc * 128:(oc + 1) * 128],
                                 rhs=out_sp[0][:, sl], start=True, stop=False)
                nc.tensor.matmul(out=py[:, :], lhsT=w_sb["wp"][1][:, oc * 128:(oc + 1) * 128],
                                 rhs=out_sp[1][:, sl], start=False, stop=True)
                y_sb = pout.tile([128, 512], F32, tag="y", name=f"y_sb{oc}_{nt}")
                nc.vector.tensor_scalar_add(y_sb[:, :], py[:, :], bias_sb[:, oc:oc + 1])
                nc.sync.dma_start(out=y_d[oc * 128:(oc + 1) * 128, sl], in_=y_sb[:, :])

    nc.compile()
    return nc


_CACHE = {}


def _get_program():
    if "nc" not in _CACHE:
        op = register_exp_op()
        _CACHE["nc"] = build_program(op)
    return _CACHE["nc"]


def kernel(x, w_qkv, w_proj, b_proj):
    x = np.asarray(x, np.float32)
    w_qkv = np.asarray(w_qkv, np.float32)
    w_proj = np.asarray(w_proj, np.float32)
    b_proj = np.asarray(b_proj, np.float32)

    nc = _get_program()

    x2 = x.reshape(B, C, N)
    wq_t = np.ascontiguousarray((w_qkv[0:C] / 8.0).T)
    wk_t = np.ascontiguousarray(w_qkv[C:2 * C].T)
    wv_t = np.ascontiguousarray(w_qkv[2 * C:3 * C].T)
    wp_t = np.ascontiguousarray(w_proj.T)
    bias2 = np.ascontiguousarray(b_proj.reshape(2, 128).T)

    in_maps = []
    for core in range(8):
        b, half = divmod(core, 2)
        n0 = half * NQ
        x_rot = np.concatenate([x2[b][:, n0:], x2[b][:, :n0]], axis=1)
        in_maps.append({
            "x": np.ascontiguousarray(x_rot),
            "wq": wq_t, "wk": wk_t, "wv": wv_t, "wp": wp_t,
            "bias": bias2,
        })

    res = run_bass_kernel_spmd(nc, in_maps, list(range(8)))

    y = np.empty((B, C, N), np.float32)
    for core in range(8):
        b, half = divmod(core, 2)
        n0 = half * NQ
        y[b][:, n0:n0 + NQ] = res.results[core]["y"]
    return y.reshape(B, C, H, W)



# revision 15
# speedup vs baseline: 1.3619x; 1.3619x over previous
"""AttentionBlock (1x1-conv QKV + 4-head softmax attention + 1x1-conv proj)
on 8 Trainium2 NeuronCores.

Sharding: data-parallel over (batch b, query-half h) -> 8 shards. Each core
gets x rotated so its 2048 query columns are always columns 0:2048 (key order
is a permutation, which softmax-attention is invariant to), computes
qkv projections, 4-head attention for its half of the queries, and the output
projection for its [256, 2048] output slice. No collectives.

v2 structure (cost-model aware: matmul cost = streamed rhs columns):
  - scores S^T = K^T Q in f32r, 256-col tiles (full-rate), keys-major PSUM
  - exp split between Act (native Exp, scale=16) and DVE (custom single-instr
    quartic: (q1(x)*q2(x))^16 ~ 24^16 * e^(16x); the 24^16 scale cancels in
    softmax since rowsums are computed from the same values). The engine
    assignment is per-(nt, column) so every softmax row is consistent.
  - attn@V in O-form: out[query, dh] with rhs=[V_h | ones] so rowsums ride
    along as a 65th column; 65-col bf16 matmuls (128-partition output).
  - normalization per 128-query chunk on DVE (reciprocal + stride-0-broadcast
    tensor_tensor), then PE transposes O back to channel-major for the
    output projection.
  - f32 -> f32r via bitcast (no conversion copies).
"""
import os
import sys

sys.path.insert(0, '/opt/trn_rl_repo')

import numpy as np
from contextlib import ExitStack

from concourse import bass, bacc, mybir
import concourse.tile as tile
from concourse import dve_ops
from concourse.dve_ops import DveOp, OPS, CUSTOM_DVE_SPECS, _SUB_OPCODE_FOR_NAME
from concourse.dve_spec import Spec, Src0, C0, C1, C2, C3, lower, sq, _spill_c3_to_src1
from concourse.dve_uop import DveOpSpec
from concourse.bass_utils import run_bass_kernel_spmd

F32 = mybir.dt.float32
F32R = mybir.dt.float32r
BF16 = mybir.dt.bfloat16
ActFn = mybir.ActivationFunctionType

B, C, H, W = 4, 256, 64, 64
HEADS, DH = 4, 64
N = H * W            # 4096 keys
NQ = N // 2          # 2048 queries per core
NT = 256             # phase-2 query tile
N_NT = NQ // NT      # 8
N_MC = N // 128      # 32 key chunks
VSTR = HEADS * (DH + 1)  # 260: per-mc vT stride ([V_h | ones] x 4 heads)

# exp(16t) * 24^16 ~ [(t^2 + c0 t + c1)(t^2 + c2 t + c3)]^16 for t in
# [-0.625, 0.625] (score x = 16t in [-10, 10]); max rel err ~9e-4. The
# 24^16 factor cancels in softmax normalization. Split into two DVE
# instructions: EXPQ2A computes P^2 (quartic + one square, 8 ALU ops),
# EXPQ2B cubes the squaring three more times ((P^2)^8 = P^16).
EQ = (0.5504330780327099, 6.148042182109957,
      3.5525352677618507, 3.903596315668177)

# Act exp column count (0..1024) per (nt, pair) slot; rest go to the DVE pipeline.
EXP_ACOLS = [int(v) for v in os.environ.get(
    "EXP_ACOLS", "1024,1024,1024,1024,1024,676,0,0").split(",")]
assert len(EXP_ACOLS) == 8


def _ref_expq2a(in0, in1, c0, c1, c2):
    x = in0.astype(np.float32)
    c3 = in1.astype(np.float32) if isinstance(in1, np.ndarray) else np.float32(in1)
    p = (((x + np.float32(c0)) * x + np.float32(c1))
         * ((x + np.float32(c2)) * x + c3)).astype(np.float32)
    return (p * p).astype(np.float32)


def _ref_expq2b(in0, in1, c0, c1, c2):
    p = in0.astype(np.float32)
    for _ in range(3):
        p = (p * p).astype(np.float32)
    return p


def _register(name, spec, rd1_en):
    row = dve_ops._CUSTOM_DVE_ROW_BASE + len(OPS)
    assert row < 0x20
    _SUB_OPCODE_FOR_NAME[name] = row
    shas = {}
    for ver in ("v3", "v4"):
        uops = lower(spec, ver=ver)
        shas[ver] = DveOpSpec(name=name, opcode=row, uops=uops, rd1_en=rd1_en).sha(ver)
    op = DveOp(name, spec, subdim=False, uops_sha=shas)
    OPS.append(op)
    CUSTOM_DVE_SPECS[name] = spec
    return op


def register_expq_op():
    if "EXPQ2A_ANT" in _SUB_OPCODE_FOR_NAME:
        a = next(op for op in OPS if op.name == "EXPQ2A_ANT")
        b = next(op for op in OPS if op.name == "EXPQ2B_ANT")
        return a, b
    x = Src0
    body_a = _spill_c3_to_src1(
        sq(((x + C0) * x + C1) * ((x + C2) * x + C3)))
    op_a = _register("EXPQ2A_ANT", Spec(body=body_a, reference=_ref_expq2a), True)
    body_b = sq(sq(sq(x)))
    op_b = _register("EXPQ2B_ANT", Spec(body=body_b, reference=_ref_expq2b), False)
    return op_a, op_b


def _ap3(base_ap, dims):
    """Manual AP with the partition dim of base_ap plus custom free dims."""
    return bass.AP(tensor=base_ap.tensor, offset=base_ap.offset,
                   ap=[list(base_ap.ap[0])] + [list(d) for d in dims])


def build_program(expq_op):
    nc = bacc.Bacc(target_bir_lowering=False)

    x_d = nc.declare_dram_parameter("x", [C, N], F32R, isOutput=False)
    wq_d = nc.declare_dram_parameter("wq", [C, C], F32R, isOutput=False)
    wk_d = nc.declare_dram_parameter("wk", [C, C], F32R, isOutput=False)
    wv_d = nc.declare_dram_parameter("wv", [C, C], F32R, isOutput=False)
    wp_d = nc.declare_dram_parameter("wp", [C, C], F32R, isOutput=False)
    bias_d = nc.declare_dram_parameter("bias", [128, 2], F32, isOutput=False)
    id_d = nc.declare_dram_parameter("ident", [128, 128], F32R, isOutput=False)
    y_d = nc.declare_dram_parameter("y", [C, NQ], F32, isOutput=True)

    with tile.TileContext(nc) as tc, ExitStack() as ctx:
        sb = ctx.enter_context(tc.tile_pool(name="sb", bufs=1))
        pex = ctx.enter_context(tc.tile_pool(name="pex", bufs=3))
        pout = ctx.enter_context(tc.tile_pool(name="pout", bufs=2))
        ps = ctx.enter_context(tc.tile_pool(name="ps", bufs=1, space="PSUM"))

        # ---------------- loads (f32 tiles, bitcast to f32r at use) --------
        XC = 512  # x DMA chunk width so phase 1 can start early
        x_f = [sb.tile([128, N], F32R, tag=f"xf{i}", name=f"xf{i}") for i in range(2)]
        for kc in range(2):
            for ch in range(N // XC):
                nc.sync.dma_start(out=x_f[kc][:, ch * XC:(ch + 1) * XC],
                                  in_=x_d[kc * 128:(kc + 1) * 128, ch * XC:(ch + 1) * XC])
        w_sb = {}
        for name, dram in (("wq", wq_d), ("wk", wk_d), ("wv", wv_d), ("wp", wp_d)):
            tiles = []
            for kc in range(2):
                f = sb.tile([128, C], F32R, tag=f"{name}{kc}", name=f"{name}f{kc}")
                nc.sync.dma_start(out=f, in_=dram[kc * 128:(kc + 1) * 128, :])
                tiles.append(f)
            w_sb[name] = tiles
        bias_sb = sb.tile([128, 2], F32, tag="bias")
        nc.sync.dma_start(out=bias_sb, in_=bias_d[:, :])
        id_sb = sb.tile([128, 128], F32R, tag="id")
        nc.sync.dma_start(out=id_sb, in_=id_d[:, :])

        c3_t = sb.tile([128, 1], F32, tag="c3")
        nc.vector.memset(c3_t, float(EQ[3]))

        def xr(kc, sl):
            return x_f[kc][:, sl]

        def wr(name, kc, oc):
            return w_sb[name][kc][:, oc * 128:(oc + 1) * 128]

        # ---------------- phase 1: qkv projections ----------------
        q_sb = [sb.tile([128, NQ], F32R, tag=f"q{oc}", name=f"q_sb{oc}") for oc in range(2)]
        k_sb = [sb.tile([128, N], F32R, tag=f"k{oc}", name=f"k_sb{oc}") for oc in range(2)]
        vT_sb = sb.tile([128, N_MC * VSTR], BF16, tag="vT")

        # ones columns of vT (col 64 + 65*h + 260*mc), written once on Pool
        ones_ap = _ap3(vT_sb[:, DH:DH + 1], [[VSTR, N_MC], [DH + 1, HEADS]])
        nc.gpsimd.memset(ones_ap, 1.0)

        evac_i = [0]

        def evac_copy(out_ap, in_ap):
            # alternate PSUM evacuations between Act and DVE
            eng = nc.scalar.copy if evac_i[0] % 2 == 0 else nc.vector.tensor_copy
            evac_i[0] += 1
            return eng(out_ap, in_ap)

        for oc in range(2):
            for t4 in range(4):
                pq = ps.tile([128, 512], F32, tag="st", bufs=2, name=f"pq{oc}_{t4}")
                sl = slice(t4 * 512, (t4 + 1) * 512)
                nc.tensor.matmul(out=pq[:, :], lhsT=wr("wq", 0, oc), rhs=xr(0, sl),
                                 start=True, stop=False)
                nc.tensor.matmul(out=pq[:, :], lhsT=wr("wq", 1, oc), rhs=xr(1, sl),
                                 start=False, stop=True)
                evac_copy(q_sb[oc][:, sl], pq[:, :])
        for oc in range(2):
            for t8 in range(8):
                pk = ps.tile([128, 512], F32, tag="st", bufs=2, name=f"pk{oc}_{t8}")
                sl = slice(t8 * 512, (t8 + 1) * 512)
                nc.tensor.matmul(out=pk[:, :], lhsT=wr("wk", 0, oc), rhs=xr(0, sl),
                                 start=True, stop=False)
                nc.tensor.matmul(out=pk[:, :], lhsT=wr("wk", 1, oc), rhs=xr(1, sl),
                                 start=False, stop=True)
                evac_copy(k_sb[oc][:, sl], pk[:, :])
        for mc in range(N_MC):
            pv = ps.tile([128, 256], F32, tag="st", bufs=2, name=f"pv{mc}")
            msl = slice(mc * 128, (mc + 1) * 128)
            nc.tensor.matmul(out=pv[:, :], lhsT=xr(0, msl), rhs=w_sb["wv"][0][:, :],
                             start=True, stop=False)
            nc.tensor.matmul(out=pv[:, :], lhsT=xr(1, msl), rhs=w_sb["wv"][1][:, :],
                             start=False, stop=True)
            # strided copy into the [V_h | ones] layout: col 65*h + d
            vout = _ap3(vT_sb[:, mc * VSTR:mc * VSTR + 1], [[DH + 1, HEADS], [1, DH]])
            vin = _ap3(pv[:, 0:1], [[DH, HEADS], [1, DH]])
            evac_copy(vout, vin)

        # ---------------- phase 2: attention ----------------
        o_n = sb.tile([128, 16 * 256], F32R, tag="on")   # normalized O, [q, c]

        for nt in range(4):                  # 512-query tiles
            qsl = slice(nt * 512, (nt + 1) * 512)
            O_ps = [ps.tile([128, 512], F32, tag="o", bufs=4, name=f"O{nt}_{qs}")
                    for qs in range(4)]
            for mc in range(N_MC):
                msl = slice(mc * 128, (mc + 1) * 128)
                for pair in range(2):
                    pst = ps.tile([128, 1024], F32, tag="st", bufs=2,
                                  name=f"pst{nt}_{mc}_{pair}")
                    # S^T for the pair's two heads, row-tiled in the PE array
                    # (baseline-proven tile_position pattern; each matmul owns
                    # a full 2KB bank).
                    nc.tensor.matmul(out=pst[:, 0:512],
                                     lhsT=k_sb[pair][0:64, msl],
                                     rhs=q_sb[pair][0:64, qsl],
                                     start=True, stop=True, tile_position=(0, 0))
                    nc.tensor.matmul(out=pst[:, 512:1024],
                                     lhsT=k_sb[pair][64:128, msl],
                                     rhs=q_sb[pair][64:128, qsl],
                                     start=True, stop=True, tile_position=(64, 0))
                    et = pex.tile([128, 1024], BF16, tag="et",
                                  name=f"et{nt}_{mc}_{pair}")
                    acols = EXP_ACOLS[nt * 2 + pair]
                    if acols > 0:
                        nc.scalar.activation(et[:, 0:acols], pst[:, 0:acols],
                                             ActFn.Exp, scale=16.0)
                    if acols < 1024:
                        y1 = pex.tile([128, 1024], F32, tag="y1",
                                      name=f"y1{nt}_{mc}_{pair}")
                        op_a, op_b = expq_op
                        nc.vector._custom_dve(op_a, out=y1[:, acols:1024],
                                              in0=pst[:, acols:1024],
                                              in1=c3_t[:, :], s0=float(EQ[0]),
                                              s1=float(EQ[1]), imm2=float(EQ[2]))
                        nc.vector._custom_dve(op_b, out=et[:, acols:1024],
                                              in0=y1[:, acols:1024])
                    for hh in range(2):
                        h = pair * 2 + hh
                        for qs in range(4):
                            # one accumulation group per O bank: start only on
                            # the very first write (the zero-region covers all
                            # 4 heads' columns), stop only on the very last.
                            nc.tensor.matmul(
                                out=O_ps[qs][:, h * 128:h * 128 + DH + 1],
                                lhsT=et[:, hh * 512 + qs * 128:hh * 512 + qs * 128 + 128],
                                rhs=vT_sb[:, mc * VSTR + h * (DH + 1):mc * VSTR + (h + 1) * (DH + 1)],
                                start=(mc == 0 and h == 0),
                                stop=(mc == N_MC - 1 and h == 3))
            for qs in range(4):
                rcp = sb.tile([128, 4], F32, tag="rcp", bufs=2, name=f"rcp{nt}_{qs}")
                rs_ap = _ap3(O_ps[qs][:, DH:DH + 1], [[128, 4], [1, 1]])
                nc.vector.reciprocal_approx_fast(out=rcp[:, :], in_=rs_ap)
                qc = nt * 4 + qs
                o_out = _ap3(o_n[:, qc * 256:qc * 256 + 1], [[64, 4], [1, 64]])
                o_in = _ap3(O_ps[qs][:, 0:1], [[128, 4], [1, 64]])
                r_in = _ap3(rcp[:, 0:1], [[1, 4], [0, 64]])
                nc.vector.tensor_tensor(out=o_out, in0=o_in, in1=r_in,
                                        op=mybir.AluOpType.mult)

        # ---------------- phase 2.5: transpose O to channel-major ----------
        out_sp = [sb.tile([128, NQ], F32R, tag=f"osp{oc}", name=f"osp{oc}") for oc in range(2)]
        for g in range(4):
            for cc in range(2):
                psT = ps.tile([128, 512], F32R, tag="o", bufs=4, name=f"psT{g}_{cc}")
                for j in range(4):
                    qc = g * 4 + j
                    nc.tensor.matmul(
                        out=psT[:, j * 128:(j + 1) * 128],
                        lhsT=o_n[:, qc * 256 + cc * 128:qc * 256 + cc * 128 + 128],
                        rhs=id_sb[:, :],
                        is_transpose=True, start=(j == 0), stop=(j == 3))
                evac_copy(out_sp[cc][:, g * 512:(g + 1) * 512], psT[:, :])

        # ---------------- phase 3: output projection + bias ----------------
        for oc in range(2):
            for t4 in range(4):
                sl = slice(t4 * 512, (t4 + 1) * 512)
                py = ps.tile([128, 512], F32, tag="o", bufs=4, name=f"py{oc}_{t4}")
                nc.tensor.matmul(out=py[:, :], lhsT=wr("wp", 0, oc),
                                 rhs=out_sp[0][:, sl],
                                 start=True, stop=False)
                nc.tensor.matmul(out=py[:, :], lhsT=wr("wp", 1, oc),
                                 rhs=out_sp[1][:, sl],
                                 start=False, stop=True)
                y_sb = pout.tile([128, 512], F32, tag="y", name=f"y_sb{oc}_{t4}")
                nc.vector.tensor_scalar_add(y_sb[:, :], py[:, :], bias_sb[:, oc:oc + 1])
                nc.sync.dma_start(out=y_d[oc * 128:(oc + 1) * 128, sl], in_=y_sb[:, :])

    nc.compile()
    return nc


_CACHE = {}


def _get_program():
    if "nc" not in _CACHE:
        op = register_expq_op()
        _CACHE["nc"] = build_program(op)
    return _CACHE["nc"]


_IDENT = np.eye(128, dtype=np.float32)


def make_in_maps(x, w_qkv, w_proj, b_proj):
    x2 = x.reshape(B, C, N)
    wq_t = np.ascontiguousarray((w_qkv[0:C] / 128.0).T)
    wk_t = np.ascontiguousarray(w_qkv[C:2 * C].T)
    wv_t = np.ascontiguousarray(w_qkv[2 * C:3 * C].T)
    wp_t = np.ascontiguousarray(w_proj.T)
    bias2 = np.ascontiguousarray(b_proj.reshape(2, 128).T)
    in_maps = []
    for core in range(8):
        b, half = divmod(core, 2)
        n0 = half * NQ
        x_rot = np.concatenate([x2[b][:, n0:], x2[b][:, :n0]], axis=1)
        in_maps.append({
            "x": np.ascontiguousarray(x_rot),
            "wq": wq_t, "wk": wk_t, "wv": wv_t, "wp": wp_t,
            "bias": bias2, "ident": _IDENT,
        })
    return in_maps


def kernel(x, w_qkv, w_proj, b_proj):
    x = np.asarray(x, np.float32)
    w_qkv = np.asarray(w_qkv, np.float32)
    w_proj = np.asarray(w_proj, np.float32)
    b_proj = np.asarray(b_proj, np.float32)

    nc = _get_program()
    in_maps = make_in_maps(x, w_qkv, w_proj, b_proj)
    res = run_bass_kernel_spmd(nc, in_maps, list(range(8)))

    y = np.empty((B, C, N), np.float32)
    for core in range(8):
        b, half = divmod(core, 2)
        n0 = half * NQ
        y[b][:, n0:n0 + NQ] = res.results[core]["y"]
    return y.reshape(B, C, H, W)


# revision 16
# speedup vs baseline: 1.6912x; 1.2418x over previous
"""AttentionBlock (1x1-conv QKV + 4-head softmax attention + 1x1-conv proj)
on 8 Trainium2 NeuronCores.

Sharding: data-parallel over (batch b, query-half h) -> 8 shards. Each core
gets x rotated so its 2048 query columns are always columns 0:2048 (key order
is a permutation, which softmax-attention is invariant to), computes
qkv projections, 4-head attention for its half of the queries, and the output
projection for its [256, 2048] output slice. No collectives.

v2 structure (cost-model aware: matmul cost = streamed rhs columns):
  - scores S^T = K^T Q in f32r, 256-col tiles (full-rate), keys-major PSUM
  - exp split between Act (native Exp, scale=16) and DVE (custom single-instr
    quartic: (q1(x)*q2(x))^16 ~ 24^16 * e^(16x); the 24^16 scale cancels in
    softmax since rowsums are computed from the same values). The engine
    assignment is per-(nt, column) so every softmax row is consistent.
  - attn@V in O-form: out[query, dh] with rhs=[V_h | ones] so rowsums ride
    along as a 65th column; 65-col bf16 matmuls (128-partition output).
  - normalization per 128-query chunk on DVE (reciprocal + stride-0-broadcast
    tensor_tensor), then PE transposes O back to channel-major for the
    output projection.
  - f32 -> f32r via bitcast (no conversion copies).
"""
import os
import sys

sys.path.insert(0, '/opt/trn_rl_repo')

import numpy as np
from contextlib import ExitStack

from concourse import bass, bacc, mybir
import concourse.tile as tile
from concourse import dve_ops
from concourse.dve_ops import DveOp, OPS, CUSTOM_DVE_SPECS, _SUB_OPCODE_FOR_NAME
from concourse.dve_spec import Spec, Src0, C0, C1, C2, C3, lower, sq, _spill_c3_to_src1
from concourse.dve_uop import DveOpSpec
from concourse.bass_utils import run_bass_kernel_spmd

F32 = mybir.dt.float32
F32R = mybir.dt.float32r
BF16 = mybir.dt.bfloat16
ActFn = mybir.ActivationFunctionType

B, C, H, W = 4, 256, 64, 64
HEADS, DH = 4, 64
N = H * W            # 4096 keys
NQ = N // 2          # 2048 queries per core
NT = 256             # phase-2 query tile
N_NT = NQ // NT      # 8
N_MC = N // 128      # 32 key chunks
VSTR = HEADS * (DH + 1)  # 260: per-mc vT stride ([V_h | ones] x 4 heads)

# exp(16t) * 24^16 ~ [(t^2 + c0 t + c1)(t^2 + c2 t + c3)]^16 for t in
# [-0.625, 0.625] (score x = 16t in [-10, 10]); max rel err ~9e-4. The
# 24^16 factor cancels in softmax normalization. Split into two DVE
# instructions: EXPQ2A computes P^2 (quartic + one square, 8 ALU ops),
# EXPQ2B cubes the squaring three more times ((P^2)^8 = P^16).
EQ = (0.5504330780327099, 6.148042182109957,
      3.5525352677618507, 3.903596315668177)

# Act exp column count (0..1024) per (nt, pair) slot; rest go to the DVE
# pipeline. Balanced per-mc: pair0 pure Act, pair1 split so both engines
# carry equal exp load concurrently (Act ~1.54us/mc == DVE ~1.54us/mc).
EXP_ACOLS = [int(v) for v in os.environ.get(
    "EXP_ACOLS", "1024,375,1024,375,1024,375,1024,375").split(",")]
assert len(EXP_ACOLS) == 8


def _ref_expq2a(in0, in1, c0, c1, c2):
    x = in0.astype(np.float32)
    c3 = in1.astype(np.float32) if isinstance(in1, np.ndarray) else np.float32(in1)
    p = (((x + np.float32(c0)) * x + np.float32(c1))
         * ((x + np.float32(c2)) * x + c3)).astype(np.float32)
    return (p * p).astype(np.float32)


def _ref_expq2b(in0, in1, c0, c1, c2):
    p = in0.astype(np.float32)
    for _ in range(3):
        p = (p * p).astype(np.float32)
    return p


def _register(name, spec, rd1_en):
    row = dve_ops._CUSTOM_DVE_ROW_BASE + len(OPS)
    assert row < 0x20
    _SUB_OPCODE_FOR_NAME[name] = row
    shas = {}
    for ver in ("v3", "v4"):
        uops = lower(spec, ver=ver)
        shas[ver] = DveOpSpec(name=name, opcode=row, uops=uops, rd1_en=rd1_en).sha(ver)
    op = DveOp(name, spec, subdim=False, uops_sha=shas)
    OPS.append(op)
    CUSTOM_DVE_SPECS[name] = spec
    return op


def register_expq_op():
    if "EXPQ2A_ANT" in _SUB_OPCODE_FOR_NAME:
        a = next(op for op in OPS if op.name == "EXPQ2A_ANT")
        b = next(op for op in OPS if op.name == "EXPQ2B_ANT")
        return a, b
    x = Src0
    body_a = _spill_c3_to_src1(
        sq(((x + C0) * x + C1) * ((x + C2) * x + C3)))
    op_a = _register("EXPQ2A_ANT", Spec(body=body_a, reference=_ref_expq2a), True)
    body_b = sq(sq(sq(x)))
    op_b = _register("EXPQ2B_ANT", Spec(body=body_b, reference=_ref_expq2b), False)
    return op_a, op_b


def _ap3(base_ap, dims):
    """Manual AP with the partition dim of base_ap plus custom free dims."""
    return bass.AP(tensor=base_ap.tensor, offset=base_ap.offset,
                   ap=[list(base_ap.ap[0])] + [list(d) for d in dims])


def build_program(expq_op):
    nc = bacc.Bacc(target_bir_lowering=False)

    x_d = nc.declare_dram_parameter("x", [C, N], F32R, isOutput=False)
    wq_d = nc.declare_dram_parameter("wq", [C, C], F32R, isOutput=False)
    wk_d = nc.declare_dram_parameter("wk", [C, C], F32R, isOutput=False)
    wv_d = nc.declare_dram_parameter("wv", [C, C], F32R, isOutput=False)
    wp_d = nc.declare_dram_parameter("wp", [C, C], F32R, isOutput=False)
    bias_d = nc.declare_dram_parameter("bias", [128, 2], F32, isOutput=False)
    id_d = nc.declare_dram_parameter("ident", [128, 128], F32R, isOutput=False)
    y_d = nc.declare_dram_parameter("y", [C, NQ], F32, isOutput=True)

    with tile.TileContext(nc) as tc, ExitStack() as ctx:
        sb = ctx.enter_context(tc.tile_pool(name="sb", bufs=1))
        pex = ctx.enter_context(tc.tile_pool(name="pex", bufs=3))
        pout = ctx.enter_context(tc.tile_pool(name="pout", bufs=2))
        ps = ctx.enter_context(tc.tile_pool(name="ps", bufs=1, space="PSUM"))

        # ---------------- loads (f32 tiles, bitcast to f32r at use) --------
        XC = 512  # x DMA chunk width so phase 1 can start early
        x_f = [sb.tile([128, N], F32R, tag=f"xf{i}", name=f"xf{i}") for i in range(2)]
        for kc in range(2):
            for ch in range(N // XC):
                nc.sync.dma_start(out=x_f[kc][:, ch * XC:(ch + 1) * XC],
                                  in_=x_d[kc * 128:(kc + 1) * 128, ch * XC:(ch + 1) * XC])
        w_sb = {}
        for name, dram in (("wq", wq_d), ("wk", wk_d), ("wv", wv_d), ("wp", wp_d)):
            tiles = []
            for kc in range(2):
                f = sb.tile([128, C], F32R, tag=f"{name}{kc}", name=f"{name}f{kc}")
                nc.sync.dma_start(out=f, in_=dram[kc * 128:(kc + 1) * 128, :])
                tiles.append(f)
            w_sb[name] = tiles
        bias_sb = sb.tile([128, 2], F32, tag="bias")
        nc.sync.dma_start(out=bias_sb, in_=bias_d[:, :])
        id_sb = sb.tile([128, 128], F32R, tag="id")
        nc.sync.dma_start(out=id_sb, in_=id_d[:, :])

        c3_t = sb.tile([128, 1], F32, tag="c3")
        nc.vector.memset(c3_t, float(EQ[3]))

        def xr(kc, sl):
            return x_f[kc][:, sl]

        def wr(name, kc, oc):
            return w_sb[name][kc][:, oc * 128:(oc + 1) * 128]

        # ---------------- phase 1: qkv projections ----------------
        q_sb = [sb.tile([128, NQ], F32R, tag=f"q{oc}", name=f"q_sb{oc}") for oc in range(2)]
        k_sb = [sb.tile([128, N], F32R, tag=f"k{oc}", name=f"k_sb{oc}") for oc in range(2)]
        vT_sb = sb.tile([128, N_MC * VSTR], BF16, tag="vT")

        # ones columns of vT (col 64 + 65*h + 260*mc), written once on Pool
        ones_ap = _ap3(vT_sb[:, DH:DH + 1], [[VSTR, N_MC], [DH + 1, HEADS]])
        nc.gpsimd.memset(ones_ap, 1.0)

        evac_i = [0]

        def evac_copy(out_ap, in_ap):
            # alternate PSUM evacuations between Act and DVE
            eng = nc.scalar.copy if evac_i[0] % 2 == 0 else nc.vector.tensor_copy
            evac_i[0] += 1
            return eng(out_ap, in_ap)

        for oc in range(2):
            for t4 in range(4):
                pq = ps.tile([128, 512], F32, tag="st", bufs=2, name=f"pq{oc}_{t4}")
                sl = slice(t4 * 512, (t4 + 1) * 512)
                nc.tensor.matmul(out=pq[:, :], lhsT=wr("wq", 0, oc), rhs=xr(0, sl),
                                 start=True, stop=False)
                nc.tensor.matmul(out=pq[:, :], lhsT=wr("wq", 1, oc), rhs=xr(1, sl),
                                 start=False, stop=True)
                evac_copy(q_sb[oc][:, sl], pq[:, :])
        for oc in range(2):
            for t8 in range(8):
                pk = ps.tile([128, 512], F32, tag="st", bufs=2, name=f"pk{oc}_{t8}")
                sl = slice(t8 * 512, (t8 + 1) * 512)
                nc.tensor.matmul(out=pk[:, :], lhsT=wr("wk", 0, oc), rhs=xr(0, sl),
                                 start=True, stop=False)
                nc.tensor.matmul(out=pk[:, :], lhsT=wr("wk", 1, oc), rhs=xr(1, sl),
                                 start=False, stop=True)
                evac_copy(k_sb[oc][:, sl], pk[:, :])
        for mc in range(N_MC):
            pv = ps.tile([128, 256], F32, tag="st", bufs=2, name=f"pv{mc}")
            msl = slice(mc * 128, (mc + 1) * 128)
            nc.tensor.matmul(out=pv[:, :], lhsT=xr(0, msl), rhs=w_sb["wv"][0][:, :],
                             start=True, stop=False)
            nc.tensor.matmul(out=pv[:, :], lhsT=xr(1, msl), rhs=w_sb["wv"][1][:, :],
                             start=False, stop=True)
            # strided copy into the [V_h | ones] layout: col 65*h + d
            vout = _ap3(vT_sb[:, mc * VSTR:mc * VSTR + 1], [[DH + 1, HEADS], [1, DH]])
            vin = _ap3(pv[:, 0:1], [[DH, HEADS], [1, DH]])
            evac_copy(vout, vin)

        # ---------------- phase 2: attention ----------------
        o_n = sb.tile([128, 16 * 256], F32R, tag="on")   # normalized O, [q, c]

        for nt in range(4):                  # 512-query tiles
            qsl = slice(nt * 512, (nt + 1) * 512)
            O_ps = [ps.tile([128, 512], F32, tag="o", bufs=4, name=f"O{nt}_{qs}")
                    for qs in range(4)]
            for mc in range(N_MC):
                msl = slice(mc * 128, (mc + 1) * 128)
                for pair in range(2):
                    pst = ps.tile([128, 1024], F32, tag="st", bufs=2,
                                  name=f"pst{nt}_{mc}_{pair}")
                    # S^T for the pair's two heads, row-tiled in the PE array
                    # (baseline-proven tile_position pattern; each matmul owns
                    # a full 2KB bank).
                    nc.tensor.matmul(out=pst[:, 0:512],
                                     lhsT=k_sb[pair][0:64, msl],
                                     rhs=q_sb[pair][0:64, qsl],
                                     start=True, stop=True, tile_position=(0, 0))
                    nc.tensor.matmul(out=pst[:, 512:1024],
                                     lhsT=k_sb[pair][64:128, msl],
                                     rhs=q_sb[pair][64:128, qsl],
                                     start=True, stop=True, tile_position=(64, 0))
                    et = pex.tile([128, 1024], BF16, tag="et",
                                  name=f"et{nt}_{mc}_{pair}")
                    acols = EXP_ACOLS[nt * 2 + pair]
                    if acols > 0:
                        nc.scalar.activation(et[:, 0:acols], pst[:, 0:acols],
                                             ActFn.Exp, scale=16.0)
                    if acols < 1024:
                        y1 = pex.tile([128, 1024], F32, tag="y1",
                                      name=f"y1{nt}_{mc}_{pair}")
                        op_a, op_b = expq_op
                        nc.vector._custom_dve(op_a, out=y1[:, acols:1024],
                                              in0=pst[:, acols:1024],
                                              in1=c3_t[:, :], s0=float(EQ[0]),
                                              s1=float(EQ[1]), imm2=float(EQ[2]))
                        nc.vector._custom_dve(op_b, out=et[:, acols:1024],
                                              in0=y1[:, acols:1024])
                    for hh in range(2):
                        h = pair * 2 + hh
                        for qs in range(4):
                            # one accumulation group per O bank: start only on
                            # the very first write (the zero-region covers all
                            # 4 heads' columns), stop only on the very last.
                            nc.tensor.matmul(
                                out=O_ps[qs][:, h * 128:h * 128 + DH + 1],
                                lhsT=et[:, hh * 512 + qs * 128:hh * 512 + qs * 128 + 128],
                                rhs=vT_sb[:, mc * VSTR + h * (DH + 1):mc * VSTR + (h + 1) * (DH + 1)],
                                start=(mc == 0 and h == 0),
                                stop=(mc == N_MC - 1 and h == 3))
            for qs in range(4):
                rcp = sb.tile([128, 4], F32, tag="rcp", bufs=2, name=f"rcp{nt}_{qs}")
                rs_ap = _ap3(O_ps[qs][:, DH:DH + 1], [[128, 4], [1, 1]])
                nc.vector.reciprocal_approx_fast(out=rcp[:, :], in_=rs_ap)
                qc = nt * 4 + qs
                o_out = _ap3(o_n[:, qc * 256:qc * 256 + 1], [[64, 4], [1, 64]])
                o_in = _ap3(O_ps[qs][:, 0:1], [[128, 4], [1, 64]])
                r_in = _ap3(rcp[:, 0:1], [[1, 4], [0, 64]])
                nc.vector.tensor_tensor(out=o_out, in0=o_in, in1=r_in,
                                        op=mybir.AluOpType.mult)

        # ---------------- phase 2.5: transpose O to channel-major ----------
        out_sp = [sb.tile([128, NQ], F32R, tag=f"osp{oc}", name=f"osp{oc}") for oc in range(2)]
        for g in range(4):
            for cc in range(2):
                psT = ps.tile([128, 512], F32R, tag="o", bufs=4, name=f"psT{g}_{cc}")
                for j in range(4):
                    qc = g * 4 + j
                    nc.tensor.matmul(
                        out=psT[:, j * 128:(j + 1) * 128],
                        lhsT=o_n[:, qc * 256 + cc * 128:qc * 256 + cc * 128 + 128],
                        rhs=id_sb[:, :],
                        is_transpose=True, start=(j == 0), stop=(j == 3))
                evac_copy(out_sp[cc][:, g * 512:(g + 1) * 512], psT[:, :])

        # ---------------- phase 3: output projection + bias ----------------
        for oc in range(2):
            for t4 in range(4):
                sl = slice(t4 * 512, (t4 + 1) * 512)
                py = ps.tile([128, 512], F32, tag="o", bufs=4, name=f"py{oc}_{t4}")
                nc.tensor.matmul(out=py[:, :], lhsT=wr("wp", 0, oc),
                                 rhs=out_sp[0][:, sl],
                                 start=True, stop=False)
                nc.tensor.matmul(out=py[:, :], lhsT=wr("wp", 1, oc),
                                 rhs=out_sp[1][:, sl],
                                 start=False, stop=True)
                y_sb = pout.tile([128, 512], F32, tag="y", name=f"y_sb{oc}_{t4}")
                nc.vector.tensor_scalar_add(y_sb[:, :], py[:, :], bias_sb[:, oc:oc + 1])
                nc.sync.dma_start(out=y_d[oc * 128:(oc + 1) * 128, sl], in_=y_sb[:, :])

    nc.compile()
    return nc


_CACHE = {}


def _get_program():
    if "nc" not in _CACHE:
        op = register_expq_op()
        _CACHE["nc"] = build_program(op)
    return _CACHE["nc"]


_IDENT = np.eye(128, dtype=np.float32)


def make_in_maps(x, w_qkv, w_proj, b_proj):
    x2 = x.reshape(B, C, N)
    wq_t = np.ascontiguousarray((w_qkv[0:C] / 128.0).T)
    wk_t = np.ascontiguousarray(w_qkv[C:2 * C].T)
    wv_t = np.ascontiguousarray(w_qkv[2 * C:3 * C].T)
    wp_t = np.ascontiguousarray(w_proj.T)
    bias2 = np.ascontiguousarray(b_proj.reshape(2, 128).T)
    in_maps = []
    for core in range(8):
        b, half = divmod(core, 2)
        n0 = half * NQ
        x_rot = np.concatenate([x2[b][:, n0:], x2[b][:, :n0]], axis=1)
        in_maps.append({
            "x": np.ascontiguousarray(x_rot),
            "wq": wq_t, "wk": wk_t, "wv": wv_t, "wp": wp_t,
            "bias": bias2, "ident": _IDENT,
        })
    return in_maps


def kernel(x, w_qkv, w_proj, b_proj):
    x = np.asarray(x, np.float32)
    w_qkv = np.asarray(w_qkv, np.float32)
    w_proj = np.asarray(w_proj, np.float32)
    b_proj = np.asarray(b_proj, np.float32)

    nc = _get_program()
    in_maps = make_in_maps(x, w_qkv, w_proj, b_proj)
    res = run_bass_kernel_spmd(nc, in_maps, list(range(8)))

    y = np.empty((B, C, N), np.float32)
    for core in range(8):
        b, half = divmod(core, 2)
        n0 = half * NQ
        y[b][:, n0:n0 + NQ] = res.results[core]["y"]
    return y.reshape(B, C, H, W)


# revision 17
# speedup vs baseline: 1.7579x; 1.0394x over previous
"""AttentionBlock (1x1-conv QKV + 4-head softmax attention + 1x1-conv proj)
on 8 Trainium2 NeuronCores.

Sharding: data-parallel over (batch b, query-half h) -> 8 shards. Each core
gets x rotated so its 2048 query columns are always columns 0:2048 (key order
is a permutation, which softmax-attention is invariant to), computes
qkv projections, 4-head attention for its half of the queries, and the output
projection for its [256, 2048] output slice. No collectives.

v2 structure (cost-model aware: matmul cost = streamed rhs columns):
  - scores S^T = K^T Q in f32r, 256-col tiles (full-rate), keys-major PSUM
  - exp split between Act (native Exp, scale=16) and DVE (custom single-instr
    quartic: (q1(x)*q2(x))^16 ~ 24^16 * e^(16x); the 24^16 scale cancels in
    softmax since rowsums are computed from the same values). The engine
    assignment is per-(nt, column) so every softmax row is consistent.
  - attn@V in O-form: out[query, dh] with rhs=[V_h | ones] so rowsums ride
    along as a 65th column; 65-col bf16 matmuls (128-partition output).
  - normalization per 128-query chunk on DVE (reciprocal + stride-0-broadcast
    tensor_tensor), then PE transposes O back to channel-major for the
    output projection.
  - f32 -> f32r via bitcast (no conversion copies).
"""
import os
import sys

sys.path.insert(0, '/opt/trn_rl_repo')

import numpy as np
from contextlib import ExitStack

from concourse import bass, bacc, mybir
import concourse.tile as tile
from concourse import dve_ops
from concourse.dve_ops import DveOp, OPS, CUSTOM_DVE_SPECS, _SUB_OPCODE_FOR_NAME
from concourse.dve_spec import Spec, Src0, C0, C1, C2, C3, lower, sq, _spill_c3_to_src1
from concourse.dve_uop import DveOpSpec
from concourse.bass_utils import run_bass_kernel_spmd

F32 = mybir.dt.float32
F32R = mybir.dt.float32r
BF16 = mybir.dt.bfloat16
ActFn = mybir.ActivationFunctionType

B, C, H, W = 4, 256, 64, 64
HEADS, DH = 4, 64
N = H * W            # 4096 keys
NQ = N // 2          # 2048 queries per core
NT = 256             # phase-2 query tile
N_NT = NQ // NT      # 8
N_MC = N // 128      # 32 key chunks
VSTR = HEADS * (DH + 1)  # 260: per-mc vT stride ([V_h | ones] x 4 heads)

# exp(16t) * 24^16 ~ [(t^2 + c0 t + c1)(t^2 + c2 t + c3)]^16 for t in
# [-0.625, 0.625] (score x = 16t in [-10, 10]); max rel err ~9e-4. The
# 24^16 factor cancels in softmax normalization. Split into two DVE
# instructions: EXPQ2A computes P^2 (quartic + one square, 8 ALU ops),
# EXPQ2B cubes the squaring three more times ((P^2)^8 = P^16).
EQ = (0.5504330780327099, 6.148042182109957,
      3.5525352677618507, 3.903596315668177)

# Act exp column count (0..1024) per (nt, pair) slot; rest go to the DVE
# pipeline. Balanced per-mc: pair0 pure Act, pair1 split so both engines
# carry equal exp load concurrently (Act ~1.54us/mc == DVE ~1.54us/mc).
EXP_ACOLS = [int(v) for v in os.environ.get(
    "EXP_ACOLS", "740,740,740,740,740,740,740,740").split(",")]
assert len(EXP_ACOLS) == 8


def _ref_expq2a(in0, in1, c0, c1, c2):
    x = in0.astype(np.float32)
    c3 = in1.astype(np.float32) if isinstance(in1, np.ndarray) else np.float32(in1)
    p = (((x + np.float32(c0)) * x + np.float32(c1))
         * ((x + np.float32(c2)) * x + c3)).astype(np.float32)
    return (p * p).astype(np.float32)


def _ref_expq2b(in0, in1, c0, c1, c2):
    p = in0.astype(np.float32)
    for _ in range(3):
        p = (p * p).astype(np.float32)
    return p


def _register(name, spec, rd1_en):
    row = dve_ops._CUSTOM_DVE_ROW_BASE + len(OPS)
    assert row < 0x20
    _SUB_OPCODE_FOR_NAME[name] = row
    shas = {}
    for ver in ("v3", "v4"):
        uops = lower(spec, ver=ver)
        shas[ver] = DveOpSpec(name=name, opcode=row, uops=uops, rd1_en=rd1_en).sha(ver)
    op = DveOp(name, spec, subdim=False, uops_sha=shas)
    OPS.append(op)
    CUSTOM_DVE_SPECS[name] = spec
    return op


def register_expq_op():
    if "EXPQ2A_ANT" in _SUB_OPCODE_FOR_NAME:
        a = next(op for op in OPS if op.name == "EXPQ2A_ANT")
        b = next(op for op in OPS if op.name == "EXPQ2B_ANT")
        return a, b
    x = Src0
    body_a = _spill_c3_to_src1(
        sq(((x + C0) * x + C1) * ((x + C2) * x + C3)))
    op_a = _register("EXPQ2A_ANT", Spec(body=body_a, reference=_ref_expq2a), True)
    body_b = sq(sq(sq(x)))
    op_b = _register("EXPQ2B_ANT", Spec(body=body_b, reference=_ref_expq2b), False)
    return op_a, op_b


def _ap3(base_ap, dims):
    """Manual AP with the partition dim of base_ap plus custom free dims."""
    return bass.AP(tensor=base_ap.tensor, offset=base_ap.offset,
                   ap=[list(base_ap.ap[0])] + [list(d) for d in dims])


def build_program(expq_op):
    nc = bacc.Bacc(target_bir_lowering=False)

    x_d = nc.declare_dram_parameter("x", [C, N], F32R, isOutput=False)
    wq_d = nc.declare_dram_parameter("wq", [C, C], F32R, isOutput=False)
    wk_d = nc.declare_dram_parameter("wk", [C, C], F32R, isOutput=False)
    wv_d = nc.declare_dram_parameter("wv", [C, C], F32R, isOutput=False)
    wp_d = nc.declare_dram_parameter("wp", [C, C], F32R, isOutput=False)
    bias_d = nc.declare_dram_parameter("bias", [128, 2], F32, isOutput=False)
    id_d = nc.declare_dram_parameter("ident", [128, 128], F32R, isOutput=False)
    y_d = nc.declare_dram_parameter("y", [C, NQ], F32, isOutput=True)

    with tile.TileContext(nc) as tc, ExitStack() as ctx:
        sb = ctx.enter_context(tc.tile_pool(name="sb", bufs=1))
        pex = ctx.enter_context(tc.tile_pool(name="pex", bufs=3))
        pout = ctx.enter_context(tc.tile_pool(name="pout", bufs=2))
        ps = ctx.enter_context(tc.tile_pool(name="ps", bufs=1, space="PSUM"))

        # ---------------- loads (f32 tiles, bitcast to f32r at use) --------
        XC = 512  # x DMA chunk width so phase 1 can start early
        x_f = [sb.tile([128, N], F32R, tag=f"xf{i}", name=f"xf{i}") for i in range(2)]
        for kc in range(2):
            for ch in range(N // XC):
                nc.sync.dma_start(out=x_f[kc][:, ch * XC:(ch + 1) * XC],
                                  in_=x_d[kc * 128:(kc + 1) * 128, ch * XC:(ch + 1) * XC])
        w_sb = {}
        for name, dram in (("wq", wq_d), ("wk", wk_d), ("wv", wv_d), ("wp", wp_d)):
            tiles = []
            for kc in range(2):
                f = sb.tile([128, C], F32R, tag=f"{name}{kc}", name=f"{name}f{kc}")
                nc.sync.dma_start(out=f, in_=dram[kc * 128:(kc + 1) * 128, :])
                tiles.append(f)
            w_sb[name] = tiles
        bias_sb = sb.tile([128, 2], F32, tag="bias")
        nc.sync.dma_start(out=bias_sb, in_=bias_d[:, :])
        id_sb = sb.tile([128, 128], F32R, tag="id")
        nc.sync.dma_start(out=id_sb, in_=id_d[:, :])

        c3_t = sb.tile([128, 1], F32, tag="c3")
        nc.vector.memset(c3_t, float(EQ[3]))

        def xr(kc, sl):
            return x_f[kc][:, sl]

        def wr(name, kc, oc):
            return w_sb[name][kc][:, oc * 128:(oc + 1) * 128]

        # ---------------- phase 1: qkv projections ----------------
        q_sb = [sb.tile([128, NQ], F32R, tag=f"q{oc}", name=f"q_sb{oc}") for oc in range(2)]
        k_sb = [sb.tile([128, N], F32R, tag=f"k{oc}", name=f"k_sb{oc}") for oc in range(2)]
        vT_sb = sb.tile([128, N_MC * VSTR], BF16, tag="vT")

        # ones columns of vT (col 64 + 65*h + 260*mc), written once on Pool
        ones_ap = _ap3(vT_sb[:, DH:DH + 1], [[VSTR, N_MC], [DH + 1, HEADS]])
        nc.gpsimd.memset(ones_ap, 1.0)

        evac_i = [0]

        def evac_copy(out_ap, in_ap):
            # alternate PSUM evacuations between Act and DVE
            eng = nc.scalar.copy if evac_i[0] % 2 == 0 else nc.vector.tensor_copy
            evac_i[0] += 1
            return eng(out_ap, in_ap)

        for oc in range(2):
            for t4 in range(4):
                pq = ps.tile([128, 512], F32, tag="st", bufs=2, name=f"pq{oc}_{t4}")
                sl = slice(t4 * 512, (t4 + 1) * 512)
                nc.tensor.matmul(out=pq[:, :], lhsT=wr("wq", 0, oc), rhs=xr(0, sl),
                                 start=True, stop=False)
                nc.tensor.matmul(out=pq[:, :], lhsT=wr("wq", 1, oc), rhs=xr(1, sl),
                                 start=False, stop=True)
                evac_copy(q_sb[oc][:, sl], pq[:, :])
        for oc in range(2):
            for t8 in range(8):
                pk = ps.tile([128, 512], F32, tag="st", bufs=2, name=f"pk{oc}_{t8}")
                sl = slice(t8 * 512, (t8 + 1) * 512)
                nc.tensor.matmul(out=pk[:, :], lhsT=wr("wk", 0, oc), rhs=xr(0, sl),
                                 start=True, stop=False)
                nc.tensor.matmul(out=pk[:, :], lhsT=wr("wk", 1, oc), rhs=xr(1, sl),
                                 start=False, stop=True)
                evac_copy(k_sb[oc][:, sl], pk[:, :])
        for mc in range(N_MC):
            pv = ps.tile([128, 256], F32, tag="st", bufs=2, name=f"pv{mc}")
            msl = slice(mc * 128, (mc + 1) * 128)
            nc.tensor.matmul(out=pv[:, :], lhsT=xr(0, msl), rhs=w_sb["wv"][0][:, :],
                             start=True, stop=False)
            nc.tensor.matmul(out=pv[:, :], lhsT=xr(1, msl), rhs=w_sb["wv"][1][:, :],
                             start=False, stop=True)
            # strided copy into the [V_h | ones] layout: col 65*h + d
            vout = _ap3(vT_sb[:, mc * VSTR:mc * VSTR + 1], [[DH + 1, HEADS], [1, DH]])
            vin = _ap3(pv[:, 0:1], [[DH, HEADS], [1, DH]])
            evac_copy(vout, vin)

        # ---------------- phase 2: attention ----------------
        o_n = sb.tile([128, 16 * 256], F32R, tag="on")   # normalized O, [q, c]

        for nt in range(4):                  # 512-query tiles
            qsl = slice(nt * 512, (nt + 1) * 512)
            O_ps = [ps.tile([128, 512], F32, tag="o", bufs=4, name=f"O{nt}_{qs}")
                    for qs in range(4)]
            for mc in range(N_MC):
                msl = slice(mc * 128, (mc + 1) * 128)
                for pair in range(2):
                    pst = ps.tile([128, 1024], F32, tag="st", bufs=2,
                                  name=f"pst{nt}_{mc}_{pair}")
                    # S^T for the pair's two heads, row-tiled in the PE array
                    # (baseline-proven tile_position pattern; each matmul owns
                    # a full 2KB bank).
                    nc.tensor.matmul(out=pst[:, 0:512],
                                     lhsT=k_sb[pair][0:64, msl],
                                     rhs=q_sb[pair][0:64, qsl],
                                     start=True, stop=True, tile_position=(0, 0))
                    nc.tensor.matmul(out=pst[:, 512:1024],
                                     lhsT=k_sb[pair][64:128, msl],
                                     rhs=q_sb[pair][64:128, qsl],
                                     start=True, stop=True, tile_position=(64, 0))
                    et = pex.tile([128, 1024], BF16, tag="et",
                                  name=f"et{nt}_{mc}_{pair}")
                    acols = EXP_ACOLS[nt * 2 + pair]
                    if acols > 0:
                        nc.scalar.activation(et[:, 0:acols], pst[:, 0:acols],
                                             ActFn.Exp, scale=16.0)
                    if acols < 1024:
                        y1 = pex.tile([128, 1024], F32, tag="y1",
                                      name=f"y1{nt}_{mc}_{pair}")
                        op_a, op_b = expq_op
                        nc.vector._custom_dve(op_a, out=y1[:, acols:1024],
                                              in0=pst[:, acols:1024],
                                              in1=c3_t[:, :], s0=float(EQ[0]),
                                              s1=float(EQ[1]), imm2=float(EQ[2]))
                        nc.vector._custom_dve(op_b, out=et[:, acols:1024],
                                              in0=y1[:, acols:1024])
                    for hh in range(2):
                        h = pair * 2 + hh
                        for qs in range(4):
                            # one accumulation group per O bank: start only on
                            # the very first write (the zero-region covers all
                            # 4 heads' columns), stop only on the very last.
                            nc.tensor.matmul(
                                out=O_ps[qs][:, h * 128:h * 128 + DH + 1],
                                lhsT=et[:, hh * 512 + qs * 128:hh * 512 + qs * 128 + 128],
                                rhs=vT_sb[:, mc * VSTR + h * (DH + 1):mc * VSTR + (h + 1) * (DH + 1)],
                                start=(mc == 0 and h == 0),
                                stop=(mc == N_MC - 1 and h == 3))
            for qs in range(4):
                rcp = sb.tile([128, 4], F32, tag="rcp", bufs=2, name=f"rcp{nt}_{qs}")
                rs_ap = _ap3(O_ps[qs][:, DH:DH + 1], [[128, 4], [1, 1]])
                nc.vector.reciprocal_approx_fast(out=rcp[:, :], in_=rs_ap)
                qc = nt * 4 + qs
                o_out = _ap3(o_n[:, qc * 256:qc * 256 + 1], [[64, 4], [1, 64]])
                o_in = _ap3(O_ps[qs][:, 0:1], [[128, 4], [1, 64]])
                r_in = _ap3(rcp[:, 0:1], [[1, 4], [0, 64]])
                nc.vector.tensor_tensor(out=o_out, in0=o_in, in1=r_in,
                                        op=mybir.AluOpType.mult)

        # ---------------- phase 2.5: transpose O to channel-major ----------
        out_sp = [sb.tile([128, NQ], F32R, tag=f"osp{oc}", name=f"osp{oc}") for oc in range(2)]
        for g in range(4):
            for cc in range(2):
                psT = ps.tile([128, 512], F32R, tag="o", bufs=4, name=f"psT{g}_{cc}")
                for j in range(4):
                    qc = g * 4 + j
                    nc.tensor.matmul(
                        out=psT[:, j * 128:(j + 1) * 128],
                        lhsT=o_n[:, qc * 256 + cc * 128:qc * 256 + cc * 128 + 128],
                        rhs=id_sb[:, :],
                        is_transpose=True, start=(j == 0), stop=(j == 3))
                evac_copy(out_sp[cc][:, g * 512:(g + 1) * 512], psT[:, :])

        # ---------------- phase 3: output projection + bias ----------------
        for oc in range(2):
            for t4 in range(4):
                sl = slice(t4 * 512, (t4 + 1) * 512)
                py = ps.tile([128, 512], F32, tag="o", bufs=4, name=f"py{oc}_{t4}")
                nc.tensor.matmul(out=py[:, :], lhsT=wr("wp", 0, oc),
                                 rhs=out_sp[0][:, sl],
                                 start=True, stop=False)
                nc.tensor.matmul(out=py[:, :], lhsT=wr("wp", 1, oc),
                                 rhs=out_sp[1][:, sl],
                                 start=False, stop=True)
                y_sb = pout.tile([128, 512], F32, tag="y", name=f"y_sb{oc}_{t4}")
                nc.vector.tensor_scalar_add(y_sb[:, :], py[:, :], bias_sb[:, oc:oc + 1])
                nc.sync.dma_start(out=y_d[oc * 128:(oc + 1) * 128, sl], in_=y_sb[:, :])

    nc.compile()
    return nc


_CACHE = {}


def _get_program():
    if "nc" not in _CACHE:
        op = register_expq_op()
        _CACHE["nc"] = build_program(op)
    return _CACHE["nc"]


_IDENT = np.eye(128, dtype=np.float32)


def make_in_maps(x, w_qkv, w_proj, b_proj):
    x2 = x.reshape(B, C, N)
    wq_t = np.ascontiguousarray((w_qkv[0:C] / 128.0).T)
    wk_t = np.ascontiguousarray(w_qkv[C:2 * C].T)
    wv_t = np.ascontiguousarray(w_qkv[2 * C:3 * C].T)
    wp_t = np.ascontiguousarray(w_proj.T)
    bias2 = np.ascontiguousarray(b_proj.reshape(2, 128).T)
    in_maps = []
    for core in range(8):
        b, half = divmod(core, 2)
        n0 = half * NQ
        x_rot = np.concatenate([x2[b][:, n0:], x2[b][:, :n0]], axis=1)
        in_maps.append({
            "x": np.ascontiguousarray(x_rot),
            "wq": wq_t, "wk": wk_t, "wv": wv_t, "wp": wp_t,
            "bias": bias2, "ident": _IDENT,
        })
    return in_maps


def kernel(x, w_qkv, w_proj, b_proj):
    x = np.asarray(x, np.float32)
    w_qkv = np.asarray(w_qkv, np.float32)
    w_proj = np.asarray(w_proj, np.float32)
    b_proj = np.asarray(b_proj, np.float32)

    nc = _get_program()
    in_maps = make_in_maps(x, w_qkv, w_proj, b_proj)
    res = run_bass_kernel_spmd(nc, in_maps, list(range(8)))

    y = np.empty((B, C, N), np.float32)
    for core in range(8):
        b, half = divmod(core, 2)
        n0 = half * NQ
        y[b][:, n0:n0 + NQ] = res.results[core]["y"]
    return y.reshape(B, C, H, W)


# revision 18
# speedup vs baseline: 1.7900x; 1.0183x over previous
"""AttentionBlock (1x1-conv QKV + 4-head softmax attention + 1x1-conv proj)
on 8 Trainium2 NeuronCores.

Sharding: data-parallel over (batch b, query-half h) -> 8 shards. Each core
gets x rotated so its 2048 query columns are always columns 0:2048 (key order
is a permutation, which softmax-attention is invariant to), computes
qkv projections, 4-head attention for its half of the queries, and the output
projection for its [256, 2048] output slice. No collectives.

v2 structure (cost-model aware: matmul cost = streamed rhs columns):
  - scores S^T = K^T Q in f32r, 256-col tiles (full-rate), keys-major PSUM
  - exp split between Act (native Exp, scale=16) and DVE (custom single-instr
    quartic: (q1(x)*q2(x))^16 ~ 24^16 * e^(16x); the 24^16 scale cancels in
    softmax since rowsums are computed from the same values). The engine
    assignment is per-(nt, column) so every softmax row is consistent.
  - attn@V in O-form: out[query, dh] with rhs=[V_h | ones] so rowsums ride
    along as a 65th column; 65-col bf16 matmuls (128-partition output).
  - normalization per 128-query chunk on DVE (reciprocal + stride-0-broadcast
    tensor_tensor), then PE transposes O back to channel-major for the
    output projection.
  - f32 -> f32r via bitcast (no conversion copies).
"""
import os
import sys

sys.path.insert(0, '/opt/trn_rl_repo')

import numpy as np
from contextlib import ExitStack

from concourse import bass, bacc, mybir
import concourse.tile as tile
from concourse import dve_ops
from concourse.dve_ops import DveOp, OPS, CUSTOM_DVE_SPECS, _SUB_OPCODE_FOR_NAME
from concourse.dve_spec import Spec, Src0, C0, C1, C2, C3, lower, sq, _spill_c3_to_src1
from concourse.dve_uop import DveOpSpec
from concourse.bass_utils import run_bass_kernel_spmd

F32 = mybir.dt.float32
F32R = mybir.dt.float32r
BF16 = mybir.dt.bfloat16
ActFn = mybir.ActivationFunctionType

B, C, H, W = 4, 256, 64, 64
HEADS, DH = 4, 64
N = H * W            # 4096 keys
NQ = N // 2          # 2048 queries per core
NT = 256             # phase-2 query tile
N_NT = NQ // NT      # 8
N_MC = N // 128      # 32 key chunks
VSTR = HEADS * (DH + 1)  # 260: per-mc vT stride ([V_h | ones] x 4 heads)

# exp(16t) * 24^16 ~ [(t^2 + c0 t + c1)(t^2 + c2 t + c3)]^16 for t in
# [-0.625, 0.625] (score x = 16t in [-10, 10]); max rel err ~9e-4. The
# 24^16 factor cancels in softmax normalization. Split into two DVE
# instructions: EXPQ2A computes P^2 (quartic + one square, 8 ALU ops),
# EXPQ2B cubes the squaring three more times ((P^2)^8 = P^16).
EQ = (0.5504330780327099, 6.148042182109957,
      3.5525352677618507, 3.903596315668177)

# Act exp column count (0..1024) per (nt, pair) slot; rest go to the DVE
# pipeline. Balanced per-mc: pair0 pure Act, pair1 split so both engines
# carry equal exp load concurrently (Act ~1.54us/mc == DVE ~1.54us/mc).
EXP_ACOLS = [int(v) for v in os.environ.get(
    "EXP_ACOLS", "740,740,740,740,740,740,740,740").split(",")]
assert len(EXP_ACOLS) == 8


def _ref_expq2a(in0, in1, c0, c1, c2):
    x = in0.astype(np.float32)
    c3 = in1.astype(np.float32) if isinstance(in1, np.ndarray) else np.float32(in1)
    p = (((x + np.float32(c0)) * x + np.float32(c1))
         * ((x + np.float32(c2)) * x + c3)).astype(np.float32)
    return (p * p).astype(np.float32)


def _ref_expq2b(in0, in1, c0, c1, c2):
    p = in0.astype(np.float32)
    for _ in range(3):
        p = (p * p).astype(np.float32)
    return p


def _register(name, spec, rd1_en):
    row = dve_ops._CUSTOM_DVE_ROW_BASE + len(OPS)
    assert row < 0x20
    _SUB_OPCODE_FOR_NAME[name] = row
    shas = {}
    for ver in ("v3", "v4"):
        uops = lower(spec, ver=ver)
        shas[ver] = DveOpSpec(name=name, opcode=row, uops=uops, rd1_en=rd1_en).sha(ver)
    op = DveOp(name, spec, subdim=False, uops_sha=shas)
    OPS.append(op)
    CUSTOM_DVE_SPECS[name] = spec
    return op


def register_expq_op():
    if "EXPQ2A_ANT" in _SUB_OPCODE_FOR_NAME:
        a = next(op for op in OPS if op.name == "EXPQ2A_ANT")
        b = next(op for op in OPS if op.name == "EXPQ2B_ANT")
        return a, b
    x = Src0
    body_a = _spill_c3_to_src1(
        sq(((x + C0) * x + C1) * ((x + C2) * x + C3)))
    op_a = _register("EXPQ2A_ANT", Spec(body=body_a, reference=_ref_expq2a), True)
    body_b = sq(sq(sq(x)))
    op_b = _register("EXPQ2B_ANT", Spec(body=body_b, reference=_ref_expq2b), False)
    return op_a, op_b


def _ap3(base_ap, dims):
    """Manual AP with the partition dim of base_ap plus custom free dims."""
    return bass.AP(tensor=base_ap.tensor, offset=base_ap.offset,
                   ap=[list(base_ap.ap[0])] + [list(d) for d in dims])


def build_program(expq_op):
    nc = bacc.Bacc(target_bir_lowering=False)

    x_d = nc.declare_dram_parameter("x", [C, N], F32R, isOutput=False)
    wq_d = nc.declare_dram_parameter("wq", [C, C], F32R, isOutput=False)
    wk_d = nc.declare_dram_parameter("wk", [C, C], F32R, isOutput=False)
    wv_d = nc.declare_dram_parameter("wv", [C, C], F32R, isOutput=False)
    wp_d = nc.declare_dram_parameter("wp", [C, C], F32R, isOutput=False)
    bias_d = nc.declare_dram_parameter("bias", [128, 2], F32, isOutput=False)
    id_d = nc.declare_dram_parameter("ident", [128, 128], F32R, isOutput=False)
    y_d = nc.declare_dram_parameter("y", [C, NQ], F32, isOutput=True)

    with tile.TileContext(nc) as tc, ExitStack() as ctx:
        sb = ctx.enter_context(tc.tile_pool(name="sb", bufs=1))
        pex = ctx.enter_context(tc.tile_pool(name="pex", bufs=6))
        pout = ctx.enter_context(tc.tile_pool(name="pout", bufs=2))
        ps = ctx.enter_context(tc.tile_pool(name="ps", bufs=1, space="PSUM"))

        # ---------------- loads (f32 tiles, bitcast to f32r at use) --------
        XC = 512  # x DMA chunk width so phase 1 can start early
        x_f = [sb.tile([128, N], F32R, tag=f"xf{i}", name=f"xf{i}") for i in range(2)]
        for ch in range(N // XC):
            for kc in range(2):
                nc.sync.dma_start(out=x_f[kc][:, ch * XC:(ch + 1) * XC],
                                  in_=x_d[kc * 128:(kc + 1) * 128, ch * XC:(ch + 1) * XC])
        w_sb = {}
        for name, dram in (("wq", wq_d), ("wk", wk_d), ("wv", wv_d), ("wp", wp_d)):
            tiles = []
            for kc in range(2):
                f = sb.tile([128, C], F32R, tag=f"{name}{kc}", name=f"{name}f{kc}")
                nc.sync.dma_start(out=f, in_=dram[kc * 128:(kc + 1) * 128, :])
                tiles.append(f)
            w_sb[name] = tiles
        bias_sb = sb.tile([128, 2], F32, tag="bias")
        nc.sync.dma_start(out=bias_sb, in_=bias_d[:, :])
        id_sb = sb.tile([128, 128], F32R, tag="id")
        nc.sync.dma_start(out=id_sb, in_=id_d[:, :])

        c3_t = sb.tile([128, 1], F32, tag="c3")
        nc.vector.memset(c3_t, float(EQ[3]))

        def xr(kc, sl):
            return x_f[kc][:, sl]

        def wr(name, kc, oc):
            return w_sb[name][kc][:, oc * 128:(oc + 1) * 128]

        # ---------------- phase 1: qkv projections ----------------
        q_sb = [sb.tile([128, NQ], F32R, tag=f"q{oc}", name=f"q_sb{oc}") for oc in range(2)]
        k_sb = [sb.tile([128, N], F32R, tag=f"k{oc}", name=f"k_sb{oc}") for oc in range(2)]
        vT_sb = sb.tile([128, N_MC * VSTR], BF16, tag="vT")

        # ones columns of vT (col 64 + 65*h + 260*mc), written once on Pool
        ones_ap = _ap3(vT_sb[:, DH:DH + 1], [[VSTR, N_MC], [DH + 1, HEADS]])
        nc.gpsimd.memset(ones_ap, 1.0)

        evac_i = [0]

        def evac_copy(out_ap, in_ap):
            # alternate PSUM evacuations between Act and DVE
            eng = nc.scalar.copy if evac_i[0] % 2 == 0 else nc.vector.tensor_copy
            evac_i[0] += 1
            return eng(out_ap, in_ap)

        for oc in range(2):
            for t4 in range(4):
                pq = ps.tile([128, 512], F32, tag="st", bufs=2, name=f"pq{oc}_{t4}")
                sl = slice(t4 * 512, (t4 + 1) * 512)
                nc.tensor.matmul(out=pq[:, :], lhsT=wr("wq", 0, oc), rhs=xr(0, sl),
                                 start=True, stop=False)
                nc.tensor.matmul(out=pq[:, :], lhsT=wr("wq", 1, oc), rhs=xr(1, sl),
                                 start=False, stop=True)
                evac_copy(q_sb[oc][:, sl], pq[:, :])
        for oc in range(2):
            for t8 in range(8):
                pk = ps.tile([128, 512], F32, tag="st", bufs=2, name=f"pk{oc}_{t8}")
                sl = slice(t8 * 512, (t8 + 1) * 512)
                nc.tensor.matmul(out=pk[:, :], lhsT=wr("wk", 0, oc), rhs=xr(0, sl),
                                 start=True, stop=False)
                nc.tensor.matmul(out=pk[:, :], lhsT=wr("wk", 1, oc), rhs=xr(1, sl),
                                 start=False, stop=True)
                evac_copy(k_sb[oc][:, sl], pk[:, :])
        for mc in range(N_MC):
            pv = ps.tile([128, 256], F32, tag="st", bufs=2, name=f"pv{mc}")
            msl = slice(mc * 128, (mc + 1) * 128)
            nc.tensor.matmul(out=pv[:, :], lhsT=xr(0, msl), rhs=w_sb["wv"][0][:, :],
                             start=True, stop=False)
            nc.tensor.matmul(out=pv[:, :], lhsT=xr(1, msl), rhs=w_sb["wv"][1][:, :],
                             start=False, stop=True)
            # strided copy into the [V_h | ones] layout: col 65*h + d
            vout = _ap3(vT_sb[:, mc * VSTR:mc * VSTR + 1], [[DH + 1, HEADS], [1, DH]])
            vin = _ap3(pv[:, 0:1], [[DH, HEADS], [1, DH]])
            evac_copy(vout, vin)

        # ---------------- phase 2: attention ----------------
        o_n = sb.tile([128, 16 * 256], F32R, tag="on")   # normalized O, [q, c]
        out_sp = [sb.tile([128, NQ], F32R, tag=f"osp{oc}", name=f"osp{oc}") for oc in range(2)]

        for nt in range(4):                  # 512-query tiles
            qsl = slice(nt * 512, (nt + 1) * 512)
            O_ps = [ps.tile([128, 512], F32, tag="o", bufs=4, name=f"O{nt}_{qs}")
                    for qs in range(4)]
            for mc in range(N_MC):
                msl = slice(mc * 128, (mc + 1) * 128)
                for pair in range(2):
                    pst = ps.tile([128, 1024], F32, tag="st", bufs=2,
                                  name=f"pst{nt}_{mc}_{pair}")
                    # S^T for the pair's two heads, row-tiled in the PE array
                    # (baseline-proven tile_position pattern; each matmul owns
                    # a full 2KB bank).
                    nc.tensor.matmul(out=pst[:, 0:512],
                                     lhsT=k_sb[pair][0:64, msl],
                                     rhs=q_sb[pair][0:64, qsl],
                                     start=True, stop=True, tile_position=(0, 0))
                    nc.tensor.matmul(out=pst[:, 512:1024],
                                     lhsT=k_sb[pair][64:128, msl],
                                     rhs=q_sb[pair][64:128, qsl],
                                     start=True, stop=True, tile_position=(64, 0))
                    et = pex.tile([128, 1024], BF16, tag="et",
                                  name=f"et{nt}_{mc}_{pair}")
                    acols = EXP_ACOLS[nt * 2 + pair]
                    if acols > 0:
                        nc.scalar.activation(et[:, 0:acols], pst[:, 0:acols],
                                             ActFn.Exp, scale=16.0)
                    if acols < 1024:
                        y1 = pex.tile([128, 1024], F32, tag="y1",
                                      name=f"y1{nt}_{mc}_{pair}")
                        op_a, op_b = expq_op
                        nc.vector._custom_dve(op_a, out=y1[:, acols:1024],
                                              in0=pst[:, acols:1024],
                                              in1=c3_t[:, :], s0=float(EQ[0]),
                                              s1=float(EQ[1]), imm2=float(EQ[2]))
                        nc.vector._custom_dve(op_b, out=et[:, acols:1024],
                                              in0=y1[:, acols:1024])
                    for hh in range(2):
                        h = pair * 2 + hh
                        for qs in range(4):
                            # one accumulation group per O bank: start only on
                            # the very first write (the zero-region covers all
                            # 4 heads' columns), stop only on the very last.
                            nc.tensor.matmul(
                                out=O_ps[qs][:, h * 128:h * 128 + DH + 1],
                                lhsT=et[:, hh * 512 + qs * 128:hh * 512 + qs * 128 + 128],
                                rhs=vT_sb[:, mc * VSTR + h * (DH + 1):mc * VSTR + (h + 1) * (DH + 1)],
                                start=(mc == 0 and h == 0),
                                stop=(mc == N_MC - 1 and h == 3))
            for qs in range(4):
                rcp = sb.tile([128, 4], F32, tag="rcp", bufs=2, name=f"rcp{nt}_{qs}")
                rs_ap = _ap3(O_ps[qs][:, DH:DH + 1], [[128, 4], [1, 1]])
                nc.vector.reciprocal_approx_fast(out=rcp[:, :], in_=rs_ap)
                qc = nt * 4 + qs
                o_out = _ap3(o_n[:, qc * 256:qc * 256 + 1], [[64, 4], [1, 64]])
                o_in = _ap3(O_ps[qs][:, 0:1], [[128, 4], [1, 64]])
                r_in = _ap3(rcp[:, 0:1], [[1, 4], [0, 64]])
                nc.vector.tensor_tensor(out=o_out, in0=o_in, in1=r_in,
                                        op=mybir.AluOpType.mult)
            # transpose this nt's O back to channel-major, then project:
            # fully pipelined with the next nt's attention.
            g = nt
            sl = slice(g * 512, (g + 1) * 512)
            for cc in range(2):
                psT = ps.tile([128, 512], F32R, tag="o", bufs=4, name=f"psT{g}_{cc}")
                for j in range(4):
                    qc = g * 4 + j
                    nc.tensor.matmul(
                        out=psT[:, j * 128:(j + 1) * 128],
                        lhsT=o_n[:, qc * 256 + cc * 128:qc * 256 + cc * 128 + 128],
                        rhs=id_sb[:, :],
                        is_transpose=True, start=(j == 0), stop=(j == 3))
                evac_copy(out_sp[cc][:, sl], psT[:, :])
            for oc in range(2):
                py = ps.tile([128, 512], F32, tag="o", bufs=4, name=f"py{oc}_{g}")
                nc.tensor.matmul(out=py[:, :], lhsT=wr("wp", 0, oc),
                                 rhs=out_sp[0][:, sl],
                                 start=True, stop=False)
                nc.tensor.matmul(out=py[:, :], lhsT=wr("wp", 1, oc),
                                 rhs=out_sp[1][:, sl],
                                 start=False, stop=True)
                y_sb = pout.tile([128, 512], F32, tag="y", name=f"y_sb{oc}_{g}")
                nc.vector.tensor_scalar_add(y_sb[:, :], py[:, :], bias_sb[:, oc:oc + 1])
                nc.sync.dma_start(out=y_d[oc * 128:(oc + 1) * 128, sl], in_=y_sb[:, :])

    nc.compile()
    return nc


_CACHE = {}


def _get_program():
    if "nc" not in _CACHE:
        op = register_expq_op()
        _CACHE["nc"] = build_program(op)
    return _CACHE["nc"]


_IDENT = np.eye(128, dtype=np.float32)


def make_in_maps(x, w_qkv, w_proj, b_proj):
    x2 = x.reshape(B, C, N)
    wq_t = np.ascontiguousarray((w_qkv[0:C] / 128.0).T)
    wk_t = np.ascontiguousarray(w_qkv[C:2 * C].T)
    wv_t = np.ascontiguousarray(w_qkv[2 * C:3 * C].T)
    wp_t = np.ascontiguousarray(w_proj.T)
    bias2 = np.ascontiguousarray(b_proj.reshape(2, 128).T)
    in_maps = []
    for core in range(8):
        b, half = divmod(core, 2)
        n0 = half * NQ
        x_rot = np.concatenate([x2[b][:, n0:], x2[b][:, :n0]], axis=1)
        in_maps.append({
            "x": np.ascontiguousarray(x_rot),
            "wq": wq_t, "wk": wk_t, "wv": wv_t, "wp": wp_t,
            "bias": bias2, "ident": _IDENT,
        })
    return in_maps


def kernel(x, w_qkv, w_proj, b_proj):
    x = np.asarray(x, np.float32)
    w_qkv = np.asarray(w_qkv, np.float32)
    w_proj = np.asarray(w_proj, np.float32)
    b_proj = np.asarray(b_proj, np.float32)

    nc = _get_program()
    in_maps = make_in_maps(x, w_qkv, w_proj, b_proj)
    res = run_bass_kernel_spmd(nc, in_maps, list(range(8)))

    y = np.empty((B, C, N), np.float32)
    for core in range(8):
        b, half = divmod(core, 2)
        n0 = half * NQ
        y[b][:, n0:n0 + NQ] = res.results[core]["y"]
    return y.reshape(B, C, H, W)


# revision 19
# speedup vs baseline: 1.8250x; 1.0196x over previous
"""AttentionBlock (1x1-conv QKV + 4-head softmax attention + 1x1-conv proj)
on 8 Trainium2 NeuronCores.

Sharding: data-parallel over (batch b, query-half h) -> 8 shards. Each core
gets x rotated so its 2048 query columns are always columns 0:2048 (key order
is a permutation, which softmax-attention is invariant to), computes
qkv projections, 4-head attention for its half of the queries, and the output
projection for its [256, 2048] output slice. No collectives.

v2 structure (cost-model aware: matmul cost = streamed rhs columns):
  - scores S^T = K^T Q in f32r, 256-col tiles (full-rate), keys-major PSUM
  - exp split between Act (native Exp, scale=16) and DVE (custom single-instr
    quartic: (q1(x)*q2(x))^16 ~ 24^16 * e^(16x); the 24^16 scale cancels in
    softmax since rowsums are computed from the same values). The engine
    assignment is per-(nt, column) so every softmax row is consistent.
  - attn@V in O-form: out[query, dh] with rhs=[V_h | ones] so rowsums ride
    along as a 65th column; 65-col bf16 matmuls (128-partition output).
  - normalization per 128-query chunk on DVE (reciprocal + stride-0-broadcast
    tensor_tensor), then PE transposes O back to channel-major for the
    output projection.
  - f32 -> f32r via bitcast (no conversion copies).
"""
import os
import sys

sys.path.insert(0, '/opt/trn_rl_repo')

import numpy as np
from contextlib import ExitStack

from concourse import bass, bacc, mybir
import concourse.tile as tile
from concourse import dve_ops
from concourse.dve_ops import DveOp, OPS, CUSTOM_DVE_SPECS, _SUB_OPCODE_FOR_NAME
from concourse.dve_spec import Spec, Src0, C0, C1, C2, C3, lower, sq, _spill_c3_to_src1
from concourse.dve_uop import DveOpSpec
from concourse.bass_utils import run_bass_kernel_spmd

F32 = mybir.dt.float32
F32R = mybir.dt.float32r
BF16 = mybir.dt.bfloat16
ActFn = mybir.ActivationFunctionType

B, C, H, W = 4, 256, 64, 64
HEADS, DH = 4, 64
N = H * W            # 4096 keys
NQ = N // 2          # 2048 queries per core
NT = 256             # phase-2 query tile
N_NT = NQ // NT      # 8
N_MC = N // 128      # 32 key chunks
VSTR = HEADS * (DH + 1)  # 260: per-mc vT stride ([V_h | ones] x 4 heads)

# exp(16t) * 24^16 ~ [(t^2 + c0 t + c1)(t^2 + c2 t + c3)]^16 for t in
# [-0.625, 0.625] (score x = 16t in [-10, 10]); max rel err ~9e-4. The
# 24^16 factor cancels in softmax normalization. Split into two DVE
# instructions: EXPQ2A computes P^2 (quartic + one square, 8 ALU ops),
# EXPQ2B cubes the squaring three more times ((P^2)^8 = P^16).
EQ = (0.5504330780327099, 6.148042182109957,
      3.5525352677618507, 3.903596315668177)

# Act exp column count (0..1024) per (nt, pair) slot; rest go to the DVE
# pipeline. Balanced per-mc: pair0 pure Act, pair1 split so both engines
# carry equal exp load concurrently (Act ~1.54us/mc == DVE ~1.54us/mc).
EXP_ACOLS = [int(v) for v in os.environ.get(
    "EXP_ACOLS", "740,740,740,740,740,740,740,740").split(",")]
assert len(EXP_ACOLS) == 8


def _ref_expq2a(in0, in1, c0, c1, c2):
    x = in0.astype(np.float32)
    c3 = in1.astype(np.float32) if isinstance(in1, np.ndarray) else np.float32(in1)
    p = (((x + np.float32(c0)) * x + np.float32(c1))
         * ((x + np.float32(c2)) * x + c3)).astype(np.float32)
    return (p * p).astype(np.float32)


def _ref_expq2b(in0, in1, c0, c1, c2):
    p = in0.astype(np.float32)
    for _ in range(3):
        p = (p * p).astype(np.float32)
    return p


def _register(name, spec, rd1_en):
    row = dve_ops._CUSTOM_DVE_ROW_BASE + len(OPS)
    assert row < 0x20
    _SUB_OPCODE_FOR_NAME[name] = row
    shas = {}
    for ver in ("v3", "v4"):
        uops = lower(spec, ver=ver)
        shas[ver] = DveOpSpec(name=name, opcode=row, uops=uops, rd1_en=rd1_en).sha(ver)
    op = DveOp(name, spec, subdim=False, uops_sha=shas)
    OPS.append(op)
    CUSTOM_DVE_SPECS[name] = spec
    return op


def register_expq_op():
    if "EXPQ2A_ANT" in _SUB_OPCODE_FOR_NAME:
        a = next(op for op in OPS if op.name == "EXPQ2A_ANT")
        b = next(op for op in OPS if op.name == "EXPQ2B_ANT")
        return a, b
    x = Src0
    body_a = _spill_c3_to_src1(
        sq(((x + C0) * x + C1) * ((x + C2) * x + C3)))
    op_a = _register("EXPQ2A_ANT", Spec(body=body_a, reference=_ref_expq2a), True)
    body_b = sq(sq(sq(x)))
    op_b = _register("EXPQ2B_ANT", Spec(body=body_b, reference=_ref_expq2b), False)
    return op_a, op_b


def _ap3(base_ap, dims):
    """Manual AP with the partition dim of base_ap plus custom free dims."""
    return bass.AP(tensor=base_ap.tensor, offset=base_ap.offset,
                   ap=[list(base_ap.ap[0])] + [list(d) for d in dims])


def build_program(expq_op):
    nc = bacc.Bacc(target_bir_lowering=False)

    x_d = nc.declare_dram_parameter("x", [C, N], F32R, isOutput=False)
    wq_d = nc.declare_dram_parameter("wq", [C, C], F32R, isOutput=False)
    wk_d = nc.declare_dram_parameter("wk", [C, C], F32R, isOutput=False)
    wv_d = nc.declare_dram_parameter("wv", [C, C], F32R, isOutput=False)
    wp_d = nc.declare_dram_parameter("wp", [C, C], F32R, isOutput=False)
    bias_d = nc.declare_dram_parameter("bias", [128, 2], F32, isOutput=False)
    id_d = nc.declare_dram_parameter("ident", [128, 128], F32R, isOutput=False)
    y_d = nc.declare_dram_parameter("y", [C, NQ], F32, isOutput=True)

    with tile.TileContext(nc) as tc, ExitStack() as ctx:
        sb = ctx.enter_context(tc.tile_pool(name="sb", bufs=1))
        pex = ctx.enter_context(tc.tile_pool(name="pex", bufs=6))
        pout = ctx.enter_context(tc.tile_pool(name="pout", bufs=2))
        ps = ctx.enter_context(tc.tile_pool(name="ps", bufs=1, space="PSUM"))

        # ---------------- loads (f32 tiles, bitcast to f32r at use) --------
        XC = 512  # x DMA chunk width so phase 1 can start early
        x_f = [sb.tile([128, N], F32R, tag=f"xf{i}", name=f"xf{i}") for i in range(2)]
        for ch in range(N // XC):
            for kc in range(2):
                nc.sync.dma_start(out=x_f[kc][:, ch * XC:(ch + 1) * XC],
                                  in_=x_d[kc * 128:(kc + 1) * 128, ch * XC:(ch + 1) * XC])
        w_sb = {}
        for name, dram in (("wq", wq_d), ("wk", wk_d), ("wv", wv_d), ("wp", wp_d)):
            tiles = []
            for kc in range(2):
                f = sb.tile([128, C], F32R, tag=f"{name}{kc}", name=f"{name}f{kc}")
                nc.sync.dma_start(out=f, in_=dram[kc * 128:(kc + 1) * 128, :])
                tiles.append(f)
            w_sb[name] = tiles
        bias_sb = sb.tile([128, 2], F32, tag="bias")
        nc.sync.dma_start(out=bias_sb, in_=bias_d[:, :])
        id_sb = sb.tile([128, 128], F32R, tag="id")
        nc.sync.dma_start(out=id_sb, in_=id_d[:, :])

        c3_t = sb.tile([128, 1], F32, tag="c3")
        nc.vector.memset(c3_t, float(EQ[3]))

        def xr(kc, sl):
            return x_f[kc][:, sl]

        def wr(name, kc, oc):
            return w_sb[name][kc][:, oc * 128:(oc + 1) * 128]

        # ---------------- phase 1: qkv projections ----------------
        q_sb = [sb.tile([128, NQ], F32R, tag=f"q{oc}", name=f"q_sb{oc}") for oc in range(2)]
        k_sb = [sb.tile([128, N], F32R, tag=f"k{oc}", name=f"k_sb{oc}") for oc in range(2)]
        vT_sb = sb.tile([128, N_MC * VSTR], BF16, tag="vT")

        # ones columns of vT (col 64 + 65*h + 260*mc), written once on Pool
        ones_ap = _ap3(vT_sb[:, DH:DH + 1], [[VSTR, N_MC], [DH + 1, HEADS]])
        nc.gpsimd.memset(ones_ap, 1.0)

        evac_i = [0]

        def evac_copy(out_ap, in_ap):
            # alternate PSUM evacuations between Act and DVE
            eng = nc.scalar.copy if evac_i[0] % 2 == 0 else nc.vector.tensor_copy
            evac_i[0] += 1
            return eng(out_ap, in_ap)

        for oc in range(2):
            for t4 in range(4):
                pq = ps.tile([128, 512], F32, tag="st", bufs=2, name=f"pq{oc}_{t4}")
                sl = slice(t4 * 512, (t4 + 1) * 512)
                nc.tensor.matmul(out=pq[:, :], lhsT=wr("wq", 0, oc), rhs=xr(0, sl),
                                 start=True, stop=False)
                nc.tensor.matmul(out=pq[:, :], lhsT=wr("wq", 1, oc), rhs=xr(1, sl),
                                 start=False, stop=True)
                evac_copy(q_sb[oc][:, sl], pq[:, :])
        for oc in range(2):
            for t8 in range(8):
                pk = ps.tile([128, 512], F32, tag="st", bufs=2, name=f"pk{oc}_{t8}")
                sl = slice(t8 * 512, (t8 + 1) * 512)
                nc.tensor.matmul(out=pk[:, :], lhsT=wr("wk", 0, oc), rhs=xr(0, sl),
                                 start=True, stop=False)
                nc.tensor.matmul(out=pk[:, :], lhsT=wr("wk", 1, oc), rhs=xr(1, sl),
                                 start=False, stop=True)
                evac_copy(k_sb[oc][:, sl], pk[:, :])
        for mc in range(N_MC):
            pv = ps.tile([128, 256], F32, tag="st", bufs=2, name=f"pv{mc}")
            msl = slice(mc * 128, (mc + 1) * 128)
            nc.tensor.matmul(out=pv[:, :], lhsT=xr(0, msl), rhs=w_sb["wv"][0][:, :],
                             start=True, stop=False)
            nc.tensor.matmul(out=pv[:, :], lhsT=xr(1, msl), rhs=w_sb["wv"][1][:, :],
                             start=False, stop=True)
            # strided copy into the [V_h | ones] layout: col 65*h + d
            vout = _ap3(vT_sb[:, mc * VSTR:mc * VSTR + 1], [[DH + 1, HEADS], [1, DH]])
            vin = _ap3(pv[:, 0:1], [[DH, HEADS], [1, DH]])
            evac_copy(vout, vin)

        # ---------------- phase 2: attention ----------------
        o_n = sb.tile([128, 16 * 256], F32R, tag="on")   # normalized O, [q, c]
        out_sp = [sb.tile([128, NQ], F32R, tag=f"osp{oc}", name=f"osp{oc}") for oc in range(2)]

        for nt in range(4):                  # 512-query tiles
            qsl = slice(nt * 512, (nt + 1) * 512)
            O_ps = [ps.tile([128, 512], F32, tag="o", bufs=4, name=f"O{nt}_{qs}")
                    for qs in range(4)]
            for mc in range(N_MC):
                msl = slice(mc * 128, (mc + 1) * 128)
                psts, ets, y1s = [], [], []
                for pair in range(2):
                    pst = ps.tile([128, 1024], F32, tag="st", bufs=2,
                                  name=f"pst{nt}_{mc}_{pair}")
                    psts.append(pst)
                    # S^T for the pair's two heads, row-tiled in the PE array
                    # (baseline-proven tile_position pattern; each matmul owns
                    # a full 2KB bank).
                    nc.tensor.matmul(out=pst[:, 0:512],
                                     lhsT=k_sb[pair][0:64, msl],
                                     rhs=q_sb[pair][0:64, qsl],
                                     start=True, stop=True, tile_position=(0, 0))
                    nc.tensor.matmul(out=pst[:, 512:1024],
                                     lhsT=k_sb[pair][64:128, msl],
                                     rhs=q_sb[pair][64:128, qsl],
                                     start=True, stop=True, tile_position=(64, 0))
                # exp emission order matters: the pst readers (Act exp and the
                # DVE EXPQ2A first stage) are queued for BOTH pairs before any
                # EXPQ2B, so each pair's pst buffer is released as early as
                # possible for the next mc's S^T (write-after-read).
                op_a, op_b = expq_op
                for pair in range(2):
                    et = pex.tile([128, 1024], BF16, tag="et",
                                  name=f"et{nt}_{mc}_{pair}")
                    ets.append(et)
                    acols = EXP_ACOLS[nt * 2 + pair]
                    if acols > 0:
                        nc.scalar.activation(et[:, 0:acols], psts[pair][:, 0:acols],
                                             ActFn.Exp, scale=16.0)
                for pair in range(2):
                    acols = EXP_ACOLS[nt * 2 + pair]
                    if acols < 1024:
                        y1 = pex.tile([128, 1024], F32, tag="y1",
                                      name=f"y1{nt}_{mc}_{pair}")
                        nc.vector._custom_dve(op_a, out=y1[:, acols:1024],
                                              in0=psts[pair][:, acols:1024],
                                              in1=c3_t[:, :], s0=float(EQ[0]),
                                              s1=float(EQ[1]), imm2=float(EQ[2]))
                    else:
                        y1 = None
                    y1s.append(y1)
                for pair in range(2):
                    acols = EXP_ACOLS[nt * 2 + pair]
                    if acols < 1024:
                        nc.vector._custom_dve(op_b, out=ets[pair][:, acols:1024],
                                              in0=y1s[pair][:, acols:1024])
                for pair in range(2):
                    for hh in range(2):
                        h = pair * 2 + hh
                        for qs in range(4):
                            # one accumulation group per O bank: start only on
                            # the very first write (the zero-region covers all
                            # 4 heads' columns), stop only on the very last.
                            nc.tensor.matmul(
                                out=O_ps[qs][:, h * 128:h * 128 + DH + 1],
                                lhsT=ets[pair][:, hh * 512 + qs * 128:hh * 512 + qs * 128 + 128],
                                rhs=vT_sb[:, mc * VSTR + h * (DH + 1):mc * VSTR + (h + 1) * (DH + 1)],
                                start=(mc == 0 and h == 0),
                                stop=(mc == N_MC - 1 and h == 3))
            for qs in range(4):
                rcp = sb.tile([128, 4], F32, tag="rcp", bufs=2, name=f"rcp{nt}_{qs}")
                rs_ap = _ap3(O_ps[qs][:, DH:DH + 1], [[128, 4], [1, 1]])
                nc.vector.reciprocal_approx_fast(out=rcp[:, :], in_=rs_ap)
                qc = nt * 4 + qs
                o_out = _ap3(o_n[:, qc * 256:qc * 256 + 1], [[64, 4], [1, 64]])
                o_in = _ap3(O_ps[qs][:, 0:1], [[128, 4], [1, 64]])
                r_in = _ap3(rcp[:, 0:1], [[1, 4], [0, 64]])
                nc.vector.tensor_tensor(out=o_out, in0=o_in, in1=r_in,
                                        op=mybir.AluOpType.mult)
            # transpose this nt's O back to channel-major, then project:
            # fully pipelined with the next nt's attention.
            g = nt
            sl = slice(g * 512, (g + 1) * 512)
            for cc in range(2):
                psT = ps.tile([128, 512], F32R, tag="o", bufs=4, name=f"psT{g}_{cc}")
                for j in range(4):
                    qc = g * 4 + j
                    nc.tensor.matmul(
                        out=psT[:, j * 128:(j + 1) * 128],
                        lhsT=o_n[:, qc * 256 + cc * 128:qc * 256 + cc * 128 + 128],
                        rhs=id_sb[:, :],
                        is_transpose=True, start=(j == 0), stop=(j == 3))
                evac_copy(out_sp[cc][:, sl], psT[:, :])
            for oc in range(2):
                py = ps.tile([128, 512], F32, tag="o", bufs=4, name=f"py{oc}_{g}")
                nc.tensor.matmul(out=py[:, :], lhsT=wr("wp", 0, oc),
                                 rhs=out_sp[0][:, sl],
                                 start=True, stop=False)
                nc.tensor.matmul(out=py[:, :], lhsT=wr("wp", 1, oc),
                                 rhs=out_sp[1][:, sl],
                                 start=False, stop=True)
                y_sb = pout.tile([128, 512], F32, tag="y", name=f"y_sb{oc}_{g}")
                nc.vector.tensor_scalar_add(y_sb[:, :], py[:, :], bias_sb[:, oc:oc + 1])
                nc.sync.dma_start(out=y_d[oc * 128:(oc + 1) * 128, sl], in_=y_sb[:, :])

    nc.compile()
    return nc


_CACHE = {}


def _get_program():
    if "nc" not in _CACHE:
        op = register_expq_op()
        _CACHE["nc"] = build_program(op)
    return _CACHE["nc"]


_IDENT = np.eye(128, dtype=np.float32)


def make_in_maps(x, w_qkv, w_proj, b_proj):
    x2 = x.reshape(B, C, N)
    wq_t = np.ascontiguousarray((w_qkv[0:C] / 128.0).T)
    wk_t = np.ascontiguousarray(w_qkv[C:2 * C].T)
    wv_t = np.ascontiguousarray(w_qkv[2 * C:3 * C].T)
    wp_t = np.ascontiguousarray(w_proj.T)
    bias2 = np.ascontiguousarray(b_proj.reshape(2, 128).T)
    in_maps = []
    for core in range(8):
        b, half = divmod(core, 2)
        n0 = half * NQ
        x_rot = np.concatenate([x2[b][:, n0:], x2[b][:, :n0]], axis=1)
        in_maps.append({
            "x": np.ascontiguousarray(x_rot),
            "wq": wq_t, "wk": wk_t, "wv": wv_t, "wp": wp_t,
            "bias": bias2, "ident": _IDENT,
        })
    return in_maps


def kernel(x, w_qkv, w_proj, b_proj):
    x = np.asarray(x, np.float32)
    w_qkv = np.asarray(w_qkv, np.float32)
    w_proj = np.asarray(w_proj, np.float32)
    b_proj = np.asarray(b_proj, np.float32)

    nc = _get_program()
    in_maps = make_in_maps(x, w_qkv, w_proj, b_proj)
    res = run_bass_kernel_spmd(nc, in_maps, list(range(8)))

    y = np.empty((B, C, N), np.float32)
    for core in range(8):
        b, half = divmod(core, 2)
        n0 = half * NQ
        y[b][:, n0:n0 + NQ] = res.results[core]["y"]
    return y.reshape(B, C, H, W)


# revision 22
# speedup vs baseline: 1.9627x; 1.0754x over previous
"""AttentionBlock (1x1-conv QKV + 4-head softmax attention + 1x1-conv proj)
on 8 Trainium2 NeuronCores.

Sharding: data-parallel over (batch b, query-half h) -> 8 shards. Each core
gets x rotated so its 2048 query columns are always columns 0:2048 (key order
is a permutation, which softmax-attention is invariant to), computes
qkv projections, 4-head attention for its half of the queries, and the output
projection for its [256, 2048] output slice. No collectives.

v2 structure (cost-model aware: matmul cost = streamed rhs columns):
  - scores S^T = K^T Q in f32r, 256-col tiles (full-rate), keys-major PSUM
  - exp split between Act (native Exp, scale=16) and DVE (custom single-instr
    quartic: (q1(x)*q2(x))^16 ~ 24^16 * e^(16x); the 24^16 scale cancels in
    softmax since rowsums are computed from the same values). The engine
    assignment is per-(nt, column) so every softmax row is consistent.
  - attn@V in O-form: out[query, dh] with rhs=[V_h | ones] so rowsums ride
    along as a 65th column; 65-col bf16 matmuls (128-partition output).
  - normalization per 128-query chunk on DVE (reciprocal + stride-0-broadcast
    tensor_tensor), then PE transposes O back to channel-major for the
    output projection.
  - f32 -> f32r via bitcast (no conversion copies).
"""
import os
import sys

sys.path.insert(0, '/opt/trn_rl_repo')

import numpy as np
from contextlib import ExitStack

from concourse import bass, bacc, mybir
import concourse.tile as tile
from concourse import dve_ops
from concourse.dve_ops import DveOp, OPS, CUSTOM_DVE_SPECS, _SUB_OPCODE_FOR_NAME
from concourse.dve_spec import Spec, Src0, C0, C1, C2, C3, lower, sq, _spill_c3_to_src1
from concourse.dve_uop import DveOpSpec
from concourse.bass_utils import run_bass_kernel_spmd

F32 = mybir.dt.float32
F32R = mybir.dt.float32r
BF16 = mybir.dt.bfloat16
ActFn = mybir.ActivationFunctionType

B, C, H, W = 4, 256, 64, 64
HEADS, DH = 4, 64
N = H * W            # 4096 keys
NQ = N // 2          # 2048 queries per core
NT = 256             # phase-2 query tile
N_NT = NQ // NT      # 8
N_MC = N // 128      # 32 key chunks
VSTR = HEADS * (DH + 1)  # 260: per-mc vT stride ([V_h | ones] x 4 heads)

# exp(16t) * 24^16 ~ [(t^2 + c0 t + c1)(t^2 + c2 t + c3)]^16 for t in
# [-0.625, 0.625] (score x = 16t in [-10, 10]); max rel err ~9e-4. The
# 24^16 factor cancels in softmax normalization. Split into two DVE
# instructions: EXPQ2A computes P^2 (quartic + one square, 8 ALU ops),
# EXPQ2B cubes the squaring three more times ((P^2)^8 = P^16).
EQ = (0.5504330780327099, 6.148042182109957,
      3.5525352677618507, 3.903596315668177)

# Act exp column count (0..1024) per (nt, pair) slot; rest go to the DVE
# pipeline. Balanced per-mc: pair0 pure Act, pair1 split so both engines
# carry equal exp load concurrently (Act ~1.54us/mc == DVE ~1.54us/mc).
EXP_ACOLS = [int(v) for v in os.environ.get(
    "EXP_ACOLS", "740,740,740,740,740,740,740,740").split(",")]
assert len(EXP_ACOLS) == 8


def _ref_expq2a(in0, in1, c0, c1, c2):
    x = in0.astype(np.float32)
    c3 = in1.astype(np.float32) if isinstance(in1, np.ndarray) else np.float32(in1)
    p = (((x + np.float32(c0)) * x + np.float32(c1))
         * ((x + np.float32(c2)) * x + c3)).astype(np.float32)
    return (p * p).astype(np.float32)


def _ref_expq2b(in0, in1, c0, c1, c2):
    p = in0.astype(np.float32)
    for _ in range(3):
        p = (p * p).astype(np.float32)
    return p


def _register(name, spec, rd1_en):
    row = dve_ops._CUSTOM_DVE_ROW_BASE + len(OPS)
    assert row < 0x20
    _SUB_OPCODE_FOR_NAME[name] = row
    shas = {}
    for ver in ("v3", "v4"):
        uops = lower(spec, ver=ver)
        shas[ver] = DveOpSpec(name=name, opcode=row, uops=uops, rd1_en=rd1_en).sha(ver)
    op = DveOp(name, spec, subdim=False, uops_sha=shas)
    OPS.append(op)
    CUSTOM_DVE_SPECS[name] = spec
    return op


def register_expq_op():
    if "EXPQ2A_ANT" in _SUB_OPCODE_FOR_NAME:
        a = next(op for op in OPS if op.name == "EXPQ2A_ANT")
        b = next(op for op in OPS if op.name == "EXPQ2B_ANT")
        return a, b
    x = Src0
    body_a = _spill_c3_to_src1(
        sq(((x + C0) * x + C1) * ((x + C2) * x + C3)))
    op_a = _register("EXPQ2A_ANT", Spec(body=body_a, reference=_ref_expq2a), True)
    body_b = sq(sq(sq(x)))
    op_b = _register("EXPQ2B_ANT", Spec(body=body_b, reference=_ref_expq2b), False)
    return op_a, op_b


def _ap3(base_ap, dims):
    """Manual AP with the partition dim of base_ap plus custom free dims."""
    return bass.AP(tensor=base_ap.tensor, offset=base_ap.offset,
                   ap=[list(base_ap.ap[0])] + [list(d) for d in dims])


def build_program(expq_op):
    nc = bacc.Bacc(target_bir_lowering=False)

    x_d = nc.declare_dram_parameter("x", [C, N], F32R, isOutput=False)
    wq_d = nc.declare_dram_parameter("wq", [C, C], F32R, isOutput=False)
    wk_d = nc.declare_dram_parameter("wk", [C, C], F32R, isOutput=False)
    wv_d = nc.declare_dram_parameter("wv", [C, C], F32R, isOutput=False)
    wp_d = nc.declare_dram_parameter("wp", [C, C], F32R, isOutput=False)
    bias_d = nc.declare_dram_parameter("bias", [128, 2], F32, isOutput=False)
    id_d = nc.declare_dram_parameter("ident", [128, 128], F32R, isOutput=False)
    y_d = nc.declare_dram_parameter("y", [C, NQ], F32, isOutput=True)

    with tile.TileContext(nc) as tc, ExitStack() as ctx:
        sb = ctx.enter_context(tc.tile_pool(name="sb", bufs=1))
        pex = ctx.enter_context(tc.tile_pool(name="pex", bufs=3))
        pout = ctx.enter_context(tc.tile_pool(name="pout", bufs=2))
        ps = ctx.enter_context(tc.tile_pool(name="ps", bufs=1, space="PSUM"))

        # ---------------- loads (f32 tiles, bitcast to f32r at use) --------
        XC = 512  # x DMA chunk width so phase 1 can start early
        x_f = [sb.tile([128, N], F32R, tag=f"xf{i}", name=f"xf{i}") for i in range(2)]
        for ch in range(N // XC):
            for kc in range(2):
                nc.sync.dma_start(out=x_f[kc][:, ch * XC:(ch + 1) * XC],
                                  in_=x_d[kc * 128:(kc + 1) * 128, ch * XC:(ch + 1) * XC])
        w_sb = {}
        for name, dram in (("wq", wq_d), ("wk", wk_d), ("wv", wv_d), ("wp", wp_d)):
            tiles = []
            for kc in range(2):
                f = sb.tile([128, C], F32R, tag=f"{name}{kc}", name=f"{name}f{kc}")
                nc.sync.dma_start(out=f, in_=dram[kc * 128:(kc + 1) * 128, :])
                tiles.append(f)
            w_sb[name] = tiles
        bias_sb = sb.tile([128, 2], F32, tag="bias")
        nc.sync.dma_start(out=bias_sb, in_=bias_d[:, :])
        id_sb = sb.tile([128, 128], F32R, tag="id")
        nc.sync.dma_start(out=id_sb, in_=id_d[:, :])

        c3_t = sb.tile([128, 1], F32, tag="c3")
        nc.vector.memset(c3_t, float(EQ[3]))

        def xr(kc, sl):
            return x_f[kc][:, sl]

        def wr(name, kc, oc):
            return w_sb[name][kc][:, oc * 128:(oc + 1) * 128]

        # ---------------- phase 1: qkv projections ----------------
        # per-head tiles, always at partition offset 0 (mixed-partition-offset
        # matmul operands crash the walrus/HW path)
        q_sb = [sb.tile([64, NQ], F32R, tag=f"q{h}", name=f"q_sb{h}") for h in range(4)]
        k_sb = [sb.tile([64, N], F32R, tag=f"k{h}", name=f"k_sb{h}") for h in range(4)]
        vT_sb = sb.tile([128, N_MC * VSTR], BF16, tag="vT")

        # ones columns of vT (col 64 + 65*h + 260*mc), written once on Pool
        ones_ap = _ap3(vT_sb[:, DH:DH + 1], [[VSTR, N_MC], [DH + 1, HEADS]])
        nc.gpsimd.memset(ones_ap, 1.0)

        evac_i = [0]

        def evac_copy(out_ap, in_ap):
            # alternate PSUM evacuations between Act and DVE
            eng = nc.scalar.copy if evac_i[0] % 2 == 0 else nc.vector.tensor_copy
            evac_i[0] += 1
            return eng(out_ap, in_ap)

        for oc in range(2):
            for t4 in range(4):
                pq = ps.tile([128, 512], F32, tag="st", bufs=3, name=f"pq{oc}_{t4}")
                sl = slice(t4 * 512, (t4 + 1) * 512)
                nc.tensor.matmul(out=pq[:, :], lhsT=wr("wq", 0, oc), rhs=xr(0, sl),
                                 start=True, stop=False)
                nc.tensor.matmul(out=pq[:, :], lhsT=wr("wq", 1, oc), rhs=xr(1, sl),
                                 start=False, stop=True)
                evac_copy(q_sb[2 * oc][:, sl], pq[0:64, :])
                evac_copy(q_sb[2 * oc + 1][:, sl], pq[64:128, :])
        for oc in range(2):
            for t8 in range(8):
                pk = ps.tile([128, 512], F32, tag="st", bufs=3, name=f"pk{oc}_{t8}")
                sl = slice(t8 * 512, (t8 + 1) * 512)
                nc.tensor.matmul(out=pk[:, :], lhsT=wr("wk", 0, oc), rhs=xr(0, sl),
                                 start=True, stop=False)
                nc.tensor.matmul(out=pk[:, :], lhsT=wr("wk", 1, oc), rhs=xr(1, sl),
                                 start=False, stop=True)
                evac_copy(k_sb[2 * oc][:, sl], pk[0:64, :])
                evac_copy(k_sb[2 * oc + 1][:, sl], pk[64:128, :])
        for mc in range(N_MC):
            pv = ps.tile([128, 256], F32, tag="st", bufs=3, name=f"pv{mc}")
            msl = slice(mc * 128, (mc + 1) * 128)
            nc.tensor.matmul(out=pv[:, :], lhsT=xr(0, msl), rhs=w_sb["wv"][0][:, :],
                             start=True, stop=False)
            nc.tensor.matmul(out=pv[:, :], lhsT=xr(1, msl), rhs=w_sb["wv"][1][:, :],
                             start=False, stop=True)
            # strided copy into the [V_h | ones] layout: col 65*h + d
            vout = _ap3(vT_sb[:, mc * VSTR:mc * VSTR + 1], [[DH + 1, HEADS], [1, DH]])
            vin = _ap3(pv[:, 0:1], [[DH, HEADS], [1, DH]])
            evac_copy(vout, vin)

        # ---------------- phase 2: attention ----------------
        o_n = sb.tile([128, 16 * 256], F32R, tag="on")   # normalized O, [q, c]
        out_sp = [sb.tile([128, NQ], F32R, tag=f"osp{oc}", name=f"osp{oc}") for oc in range(2)]

        op_a, op_b = expq_op
        for nt in range(N_NT):               # 256-query tiles
            qsl = slice(nt * NT, (nt + 1) * NT)
            O_ps = [ps.tile([128, 512], F32, tag="o", bufs=2, name=f"O{nt}_{qs}")
                    for qs in range(2)]
            for mc in range(N_MC):
                msl = slice(mc * 128, (mc + 1) * 128)
                # all 4 heads' scores for this (nt, mc) in one 2-bank tile;
                # triple-buffered so the exp WAR chain stays off the
                # critical path.
                pst = ps.tile([128, 1024], F32, tag="st", bufs=3,
                              name=f"pst{nt}_{mc}")
                for h in range(4):
                    # per-head operands at partition offset 0; two heads per
                    # 2KB PSUM bank: first starts the group (lazy-zeroing the
                    # bank), second stops it.
                    nc.tensor.matmul(out=pst[:, h * 256:(h + 1) * 256],
                                     lhsT=k_sb[h][:, msl],
                                     rhs=q_sb[h][:, qsl],
                                     start=(h % 2 == 0), stop=(h % 2 == 1))
                et = pex.tile([128, 1024], BF16, tag="et", name=f"et{nt}_{mc}")
                acols = EXP_ACOLS[nt]
                if acols > 0:
                    nc.scalar.activation(et[:, 0:acols], pst[:, 0:acols],
                                         ActFn.Exp, scale=16.0)
                if acols < 1024:
                    y1 = pex.tile([128, 1024], F32, tag="y1", name=f"y1{nt}_{mc}")
                    nc.vector._custom_dve(op_a, out=y1[:, acols:1024],
                                          in0=pst[:, acols:1024],
                                          in1=c3_t[:, :], s0=float(EQ[0]),
                                          s1=float(EQ[1]), imm2=float(EQ[2]))
                    nc.vector._custom_dve(op_b, out=et[:, acols:1024],
                                          in0=y1[:, acols:1024])
                first, last = mc == 0, mc == N_MC - 1
                for h in range(4):
                    for qs in range(2):
                        # one accumulation group per O bank: start only on the
                        # very first write (the zero-region covers all 4 heads'
                        # columns), stop only on the very last.
                        nc.tensor.matmul(
                            out=O_ps[qs][:, h * 128:h * 128 + DH + 1],
                            lhsT=et[:, h * 256 + qs * 128:h * 256 + qs * 128 + 128],
                            rhs=vT_sb[:, mc * VSTR + h * (DH + 1):mc * VSTR + (h + 1) * (DH + 1)],
                            start=(first and h == 0), stop=(last and h == 3))
            for qs in range(2):
                rcp = sb.tile([128, 4], F32, tag="rcp", bufs=2, name=f"rcp{nt}_{qs}")
                rs_ap = _ap3(O_ps[qs][:, DH:DH + 1], [[128, 4], [1, 1]])
                nc.vector.reciprocal_approx_fast(out=rcp[:, :], in_=rs_ap)
                qc = nt * 2 + qs
                o_out = _ap3(o_n[:, qc * 256:qc * 256 + 1], [[64, 4], [1, 64]])
                o_in = _ap3(O_ps[qs][:, 0:1], [[128, 4], [1, 64]])
                r_in = _ap3(rcp[:, 0:1], [[1, 4], [0, 64]])
                nc.vector.tensor_tensor(out=o_out, in0=o_in, in1=r_in,
                                        op=mybir.AluOpType.mult)
            if nt % 2 == 1:
                # transpose the last 4 qchunks back to channel-major and
                # project, pipelined with the next nt's attention.
                g = nt // 2
                sl = slice(g * 512, (g + 1) * 512)
                for cc in range(2):
                    psT = ps.tile([128, 512], F32R, tag="o", bufs=2,
                                  name=f"psT{g}_{cc}")
                    for j in range(4):
                        qc = g * 4 + j
                        nc.tensor.matmul(
                            out=psT[:, j * 128:(j + 1) * 128],
                            lhsT=o_n[:, qc * 256 + cc * 128:qc * 256 + cc * 128 + 128],
                            rhs=id_sb[:, :],
                            is_transpose=True, start=(j == 0), stop=(j == 3))
                    evac_copy(out_sp[cc][:, sl], psT[:, :])
                for oc in range(2):
                    py = ps.tile([128, 512], F32, tag="o", bufs=2, name=f"py{oc}_{g}")
                    nc.tensor.matmul(out=py[:, :], lhsT=wr("wp", 0, oc),
                                     rhs=out_sp[0][:, sl],
                                     start=True, stop=False)
                    nc.tensor.matmul(out=py[:, :], lhsT=wr("wp", 1, oc),
                                     rhs=out_sp[1][:, sl],
                                     start=False, stop=True)
                    y_sb = pout.tile([128, 512], F32, tag="y", name=f"y_sb{oc}_{g}")
                    nc.vector.tensor_scalar_add(y_sb[:, :], py[:, :],
                                                bias_sb[:, oc:oc + 1])
                    nc.sync.dma_start(out=y_d[oc * 128:(oc + 1) * 128, sl],
                                      in_=y_sb[:, :])

    nc.compile()
    return nc


_CACHE = {}


def _get_program():
    if "nc" not in _CACHE:
        op = register_expq_op()
        _CACHE["nc"] = build_program(op)
    return _CACHE["nc"]


_IDENT = np.eye(128, dtype=np.float32)


def make_in_maps(x, w_qkv, w_proj, b_proj):
    x2 = x.reshape(B, C, N)
    wq_t = np.ascontiguousarray((w_qkv[0:C] / 128.0).T)
    wk_t = np.ascontiguousarray(w_qkv[C:2 * C].T)
    wv_t = np.ascontiguousarray(w_qkv[2 * C:3 * C].T)
    wp_t = np.ascontiguousarray(w_proj.T)
    bias2 = np.ascontiguousarray(b_proj.reshape(2, 128).T)
    in_maps = []
    for core in range(8):
        b, half = divmod(core, 2)
        n0 = half * NQ
        x_rot = np.concatenate([x2[b][:, n0:], x2[b][:, :n0]], axis=1)
        in_maps.append({
            "x": np.ascontiguousarray(x_rot),
            "wq": wq_t, "wk": wk_t, "wv": wv_t, "wp": wp_t,
            "bias": bias2, "ident": _IDENT,
        })
    return in_maps


def kernel(x, w_qkv, w_proj, b_proj):
    x = np.asarray(x, np.float32)
    w_qkv = np.asarray(w_qkv, np.float32)
    w_proj = np.asarray(w_proj, np.float32)
    b_proj = np.asarray(b_proj, np.float32)

    nc = _get_program()
    in_maps = make_in_maps(x, w_qkv, w_proj, b_proj)
    res = run_bass_kernel_spmd(nc, in_maps, list(range(8)))

    y = np.empty((B, C, N), np.float32)
    for core in range(8):
        b, half = divmod(core, 2)
        n0 = half * NQ
        y[b][:, n0:n0 + NQ] = res.results[core]["y"]
    return y.reshape(B, C, H, W)


# revision 23
# speedup vs baseline: 1.9674x; 1.0024x over previous
"""AttentionBlock (1x1-conv QKV + 4-head softmax attention + 1x1-conv proj)
on 8 Trainium2 NeuronCores.

Sharding: data-parallel over (batch b, query-half h) -> 8 shards. Each core
gets x rotated so its 2048 query columns are always columns 0:2048 (key order
is a permutation, which softmax-attention is invariant to), computes
qkv projections, 4-head attention for its half of the queries, and the output
projection for its [256, 2048] output slice. No collectives.

v2 structure (cost-model aware: matmul cost = streamed rhs columns):
  - scores S^T = K^T Q in f32r, 256-col tiles (full-rate), keys-major PSUM
  - exp split between Act (native Exp, scale=16) and DVE (custom single-instr
    quartic: (q1(x)*q2(x))^16 ~ 24^16 * e^(16x); the 24^16 scale cancels in
    softmax since rowsums are computed from the same values). The engine
    assignment is per-(nt, column) so every softmax row is consistent.
  - attn@V in O-form: out[query, dh] with rhs=[V_h | ones] so rowsums ride
    along as a 65th column; 65-col bf16 matmuls (128-partition output).
  - normalization per 128-query chunk on DVE (reciprocal + stride-0-broadcast
    tensor_tensor), then PE transposes O back to channel-major for the
    output projection.
  - f32 -> f32r via bitcast (no conversion copies).
"""
import os
import sys

sys.path.insert(0, '/opt/trn_rl_repo')

import numpy as np
from contextlib import ExitStack

from concourse import bass, bacc, mybir
import concourse.tile as tile
from concourse import dve_ops
from concourse.dve_ops import DveOp, OPS, CUSTOM_DVE_SPECS, _SUB_OPCODE_FOR_NAME
from concourse.dve_spec import Spec, Src0, C0, C1, C2, C3, lower, sq, _spill_c3_to_src1
from concourse.dve_uop import DveOpSpec
from concourse.bass_utils import run_bass_kernel_spmd

F32 = mybir.dt.float32
F32R = mybir.dt.float32r
BF16 = mybir.dt.bfloat16
ActFn = mybir.ActivationFunctionType

B, C, H, W = 4, 256, 64, 64
HEADS, DH = 4, 64
N = H * W            # 4096 keys
NQ = N // 2          # 2048 queries per core
NT = 256             # phase-2 query tile
N_NT = NQ // NT      # 8
N_MC = N // 128      # 32 key chunks
VSTR = HEADS * (DH + 1)  # 260: per-mc vT stride ([V_h | ones] x 4 heads)

# exp(16t) * 24^16 ~ [(t^2 + c0 t + c1)(t^2 + c2 t + c3)]^16 for t in
# [-0.625, 0.625] (score x = 16t in [-10, 10]); max rel err ~9e-4. The
# 24^16 factor cancels in softmax normalization. Split into two DVE
# instructions: EXPQ2A computes P^2 (quartic + one square, 8 ALU ops),
# EXPQ2B cubes the squaring three more times ((P^2)^8 = P^16).
EQ = (0.5504330780327099, 6.148042182109957,
      3.5525352677618507, 3.903596315668177)

# Act exp column count (0..1024) per (nt, pair) slot; rest go to the DVE
# pipeline. Balanced per-mc: pair0 pure Act, pair1 split so both engines
# carry equal exp load concurrently (Act ~1.54us/mc == DVE ~1.54us/mc).
EXP_ACOLS = [int(v) for v in os.environ.get(
    "EXP_ACOLS", "740,740,740,740,740,740,740,740").split(",")]
assert len(EXP_ACOLS) == 8


def _ref_expq2a(in0, in1, c0, c1, c2):
    x = in0.astype(np.float32)
    c3 = in1.astype(np.float32) if isinstance(in1, np.ndarray) else np.float32(in1)
    p = (((x + np.float32(c0)) * x + np.float32(c1))
         * ((x + np.float32(c2)) * x + c3)).astype(np.float32)
    return (p * p).astype(np.float32)


def _ref_expq2b(in0, in1, c0, c1, c2):
    p = in0.astype(np.float32)
    for _ in range(3):
        p = (p * p).astype(np.float32)
    return p


def _register(name, spec, rd1_en):
    row = dve_ops._CUSTOM_DVE_ROW_BASE + len(OPS)
    assert row < 0x20
    _SUB_OPCODE_FOR_NAME[name] = row
    shas = {}
    for ver in ("v3", "v4"):
        uops = lower(spec, ver=ver)
        shas[ver] = DveOpSpec(name=name, opcode=row, uops=uops, rd1_en=rd1_en).sha(ver)
    op = DveOp(name, spec, subdim=False, uops_sha=shas)
    OPS.append(op)
    CUSTOM_DVE_SPECS[name] = spec
    return op


def register_expq_op():
    if "EXPQ2A_ANT" in _SUB_OPCODE_FOR_NAME:
        a = next(op for op in OPS if op.name == "EXPQ2A_ANT")
        b = next(op for op in OPS if op.name == "EXPQ2B_ANT")
        return a, b
    x = Src0
    body_a = _spill_c3_to_src1(
        sq(((x + C0) * x + C1) * ((x + C2) * x + C3)))
    op_a = _register("EXPQ2A_ANT", Spec(body=body_a, reference=_ref_expq2a), True)
    body_b = sq(sq(sq(x)))
    op_b = _register("EXPQ2B_ANT", Spec(body=body_b, reference=_ref_expq2b), False)
    return op_a, op_b


def _ap3(base_ap, dims):
    """Manual AP with the partition dim of base_ap plus custom free dims."""
    return bass.AP(tensor=base_ap.tensor, offset=base_ap.offset,
                   ap=[list(base_ap.ap[0])] + [list(d) for d in dims])


def build_program(expq_op):
    nc = bacc.Bacc(target_bir_lowering=False)

    x_d = nc.declare_dram_parameter("x", [C, N], F32R, isOutput=False)
    wq_d = nc.declare_dram_parameter("wq", [C, C], F32R, isOutput=False)
    wk_d = nc.declare_dram_parameter("wk", [C, C], F32R, isOutput=False)
    wv_d = nc.declare_dram_parameter("wv", [C, C], F32R, isOutput=False)
    wp_d = nc.declare_dram_parameter("wp", [C, C], F32R, isOutput=False)
    bias_d = nc.declare_dram_parameter("bias", [128, 2], F32, isOutput=False)
    id_d = nc.declare_dram_parameter("ident", [128, 128], F32R, isOutput=False)
    y_d = nc.declare_dram_parameter("y", [C, NQ], F32, isOutput=True)

    with tile.TileContext(nc) as tc, ExitStack() as ctx:
        sb = ctx.enter_context(tc.tile_pool(name="sb", bufs=1))
        pex = ctx.enter_context(tc.tile_pool(name="pex", bufs=3))
        pout = ctx.enter_context(tc.tile_pool(name="pout", bufs=2))
        ps = ctx.enter_context(tc.tile_pool(name="ps", bufs=1, space="PSUM"))

        # ---------------- loads (f32 tiles, bitcast to f32r at use) --------
        XC = 512  # x DMA chunk width so phase 1 can start early
        x_f = [sb.tile([128, N], F32R, tag=f"xf{i}", name=f"xf{i}") for i in range(2)]
        for ch in range(N // XC):
            for kc in range(2):
                nc.sync.dma_start(out=x_f[kc][:, ch * XC:(ch + 1) * XC],
                                  in_=x_d[kc * 128:(kc + 1) * 128, ch * XC:(ch + 1) * XC])
        w_sb = {}
        for name, dram in (("wq", wq_d), ("wk", wk_d), ("wv", wv_d), ("wp", wp_d)):
            tiles = []
            for kc in range(2):
                f = sb.tile([128, C], F32R, tag=f"{name}{kc}", name=f"{name}f{kc}")
                nc.sync.dma_start(out=f, in_=dram[kc * 128:(kc + 1) * 128, :])
                tiles.append(f)
            w_sb[name] = tiles
        bias_sb = sb.tile([128, 2], F32, tag="bias")
        nc.sync.dma_start(out=bias_sb, in_=bias_d[:, :])
        id_sb = sb.tile([128, 128], F32R, tag="id")
        nc.sync.dma_start(out=id_sb, in_=id_d[:, :])

        c3_t = sb.tile([128, 1], F32, tag="c3")
        nc.vector.memset(c3_t, float(EQ[3]))

        def xr(kc, sl):
            return x_f[kc][:, sl]

        def wr(name, kc, oc):
            return w_sb[name][kc][:, oc * 128:(oc + 1) * 128]

        # ---------------- phase 1: qkv projections ----------------
        # per-head tiles, always at partition offset 0 (mixed-partition-offset
        # matmul operands crash the walrus/HW path)
        q_sb = [sb.tile([64, NQ], F32R, tag=f"q{h}", name=f"q_sb{h}") for h in range(4)]
        k_sb = [sb.tile([64, N], F32R, tag=f"k{h}", name=f"k_sb{h}") for h in range(4)]
        vT_sb = sb.tile([128, N_MC * VSTR], BF16, tag="vT")

        # ones columns of vT (col 64 + 65*h + 260*mc), written once on Pool
        ones_ap = _ap3(vT_sb[:, DH:DH + 1], [[VSTR, N_MC], [DH + 1, HEADS]])
        nc.gpsimd.memset(ones_ap, 1.0)

        evac_i = [0]

        def evac_copy(out_ap, in_ap):
            # alternate PSUM evacuations between Act and DVE
            eng = nc.scalar.copy if evac_i[0] % 2 == 0 else nc.vector.tensor_copy
            evac_i[0] += 1
            return eng(out_ap, in_ap)

        for oc in range(2):
            for t4 in range(4):
                pq = ps.tile([128, 512], F32, tag="st", bufs=3, name=f"pq{oc}_{t4}")
                sl = slice(t4 * 512, (t4 + 1) * 512)
                nc.tensor.matmul(out=pq[:, :], lhsT=wr("wq", 0, oc), rhs=xr(0, sl),
                                 start=True, stop=False)
                nc.tensor.matmul(out=pq[:, :], lhsT=wr("wq", 1, oc), rhs=xr(1, sl),
                                 start=False, stop=True)
                evac_copy(q_sb[2 * oc][:, sl], pq[0:64, :])
                evac_copy(q_sb[2 * oc + 1][:, sl], pq[64:128, :])
        for oc in range(2):
            for t8 in range(8):
                pk = ps.tile([128, 512], F32, tag="st", bufs=3, name=f"pk{oc}_{t8}")
                sl = slice(t8 * 512, (t8 + 1) * 512)
                nc.tensor.matmul(out=pk[:, :], lhsT=wr("wk", 0, oc), rhs=xr(0, sl),
                                 start=True, stop=False)
                nc.tensor.matmul(out=pk[:, :], lhsT=wr("wk", 1, oc), rhs=xr(1, sl),
                                 start=False, stop=True)
                evac_copy(k_sb[2 * oc][:, sl], pk[0:64, :])
                evac_copy(k_sb[2 * oc + 1][:, sl], pk[64:128, :])
        for mc in range(N_MC):
            pv = ps.tile([128, 256], F32, tag="st", bufs=3, name=f"pv{mc}")
            msl = slice(mc * 128, (mc + 1) * 128)
            nc.tensor.matmul(out=pv[:, :], lhsT=xr(0, msl), rhs=w_sb["wv"][0][:, :],
                             start=True, stop=False)
            nc.tensor.matmul(out=pv[:, :], lhsT=xr(1, msl), rhs=w_sb["wv"][1][:, :],
                             start=False, stop=True)
            # strided copy into the [V_h | ones] layout: col 65*h + d
            vout = _ap3(vT_sb[:, mc * VSTR:mc * VSTR + 1], [[DH + 1, HEADS], [1, DH]])
            vin = _ap3(pv[:, 0:1], [[DH, HEADS], [1, DH]])
            evac_copy(vout, vin)

        # ---------------- phase 2: attention ----------------
        o_n = sb.tile([128, 16 * 256], F32R, tag="on")   # normalized O, [q, c]
        out_sp = [sb.tile([128, NQ], F32R, tag=f"osp{oc}", name=f"osp{oc}") for oc in range(2)]

        op_a, op_b = expq_op
        for nt in range(N_NT):               # 256-query tiles
            qsl = slice(nt * NT, (nt + 1) * NT)
            O_ps = [ps.tile([128, 512], F32, tag="o", bufs=2, name=f"O{nt}_{qs}")
                    for qs in range(2)]
            for mc in range(N_MC):
                msl = slice(mc * 128, (mc + 1) * 128)
                # all 4 heads' scores for this (nt, mc) in one 2-bank tile;
                # triple-buffered so the exp WAR chain stays off the
                # critical path.
                pst = ps.tile([128, 1024], F32, tag="st", bufs=3,
                              name=f"pst{nt}_{mc}")
                for h in range(4):
                    # per-head operands at partition offset 0; two heads per
                    # 2KB PSUM bank: first starts the group (lazy-zeroing the
                    # bank), second stops it.
                    nc.tensor.matmul(out=pst[:, h * 256:(h + 1) * 256],
                                     lhsT=k_sb[h][:, msl],
                                     rhs=q_sb[h][:, qsl],
                                     start=(h % 2 == 0), stop=(h % 2 == 1))
                et = pex.tile([128, 1024], BF16, tag="et", name=f"et{nt}_{mc}")
                acols = EXP_ACOLS[nt]
                if acols > 0:
                    nc.scalar.activation(et[:, 0:acols], pst[:, 0:acols],
                                         ActFn.Exp, scale=16.0)
                if acols < 1024:
                    y1 = pex.tile([128, 1024], F32, tag="y1", name=f"y1{nt}_{mc}")
                    nc.vector._custom_dve(op_a, out=y1[:, acols:1024],
                                          in0=pst[:, acols:1024],
                                          in1=c3_t[:, :], s0=float(EQ[0]),
                                          s1=float(EQ[1]), imm2=float(EQ[2]))
                    # deprioritize the second stage so the scheduler slots the
                    # next tile's EXPQ2A into the A->B ack gap instead of
                    # idling the DVE on the y1 write-ack.
                    with tc.high_priority(-24):
                        nc.vector._custom_dve(op_b, out=et[:, acols:1024],
                                              in0=y1[:, acols:1024])
                first, last = mc == 0, mc == N_MC - 1
                for h in range(4):
                    for qs in range(2):
                        # one accumulation group per O bank: start only on the
                        # very first write (the zero-region covers all 4 heads'
                        # columns), stop only on the very last.
                        nc.tensor.matmul(
                            out=O_ps[qs][:, h * 128:h * 128 + DH + 1],
                            lhsT=et[:, h * 256 + qs * 128:h * 256 + qs * 128 + 128],
                            rhs=vT_sb[:, mc * VSTR + h * (DH + 1):mc * VSTR + (h + 1) * (DH + 1)],
                            start=(first and h == 0), stop=(last and h == 3))
            for qs in range(2):
                rcp = sb.tile([128, 4], F32, tag="rcp", bufs=2, name=f"rcp{nt}_{qs}")
                rs_ap = _ap3(O_ps[qs][:, DH:DH + 1], [[128, 4], [1, 1]])
                nc.vector.reciprocal_approx_fast(out=rcp[:, :], in_=rs_ap)
                qc = nt * 2 + qs
                o_out = _ap3(o_n[:, qc * 256:qc * 256 + 1], [[64, 4], [1, 64]])
                o_in = _ap3(O_ps[qs][:, 0:1], [[128, 4], [1, 64]])
                r_in = _ap3(rcp[:, 0:1], [[1, 4], [0, 64]])
                nc.vector.tensor_tensor(out=o_out, in0=o_in, in1=r_in,
                                        op=mybir.AluOpType.mult)
            if nt % 2 == 1:
                # transpose the last 4 qchunks back to channel-major and
                # project, pipelined with the next nt's attention.
                g = nt // 2
                sl = slice(g * 512, (g + 1) * 512)
                for cc in range(2):
                    psT = ps.tile([128, 512], F32R, tag="o", bufs=2,
                                  name=f"psT{g}_{cc}")
                    for j in range(4):
                        qc = g * 4 + j
                        nc.tensor.matmul(
                            out=psT[:, j * 128:(j + 1) * 128],
                            lhsT=o_n[:, qc * 256 + cc * 128:qc * 256 + cc * 128 + 128],
                            rhs=id_sb[:, :],
                            is_transpose=True, start=(j == 0), stop=(j == 3))
                    evac_copy(out_sp[cc][:, sl], psT[:, :])
                for oc in range(2):
                    py = ps.tile([128, 512], F32, tag="o", bufs=2, name=f"py{oc}_{g}")
                    nc.tensor.matmul(out=py[:, :], lhsT=wr("wp", 0, oc),
                                     rhs=out_sp[0][:, sl],
                                     start=True, stop=False)
                    nc.tensor.matmul(out=py[:, :], lhsT=wr("wp", 1, oc),
                                     rhs=out_sp[1][:, sl],
                                     start=False, stop=True)
                    y_sb = pout.tile([128, 512], F32, tag="y", name=f"y_sb{oc}_{g}")
                    nc.vector.tensor_scalar_add(y_sb[:, :], py[:, :],
                                                bias_sb[:, oc:oc + 1])
                    nc.sync.dma_start(out=y_d[oc * 128:(oc + 1) * 128, sl],
                                      in_=y_sb[:, :])

    nc.compile()
    return nc


_CACHE = {}


def _get_program():
    if "nc" not in _CACHE:
        op = register_expq_op()
        _CACHE["nc"] = build_program(op)
    return _CACHE["nc"]


_IDENT = np.eye(128, dtype=np.float32)


def make_in_maps(x, w_qkv, w_proj, b_proj):
    x2 = x.reshape(B, C, N)
    wq_t = np.ascontiguousarray((w_qkv[0:C] / 128.0).T)
    wk_t = np.ascontiguousarray(w_qkv[C:2 * C].T)
    wv_t = np.ascontiguousarray(w_qkv[2 * C:3 * C].T)
    wp_t = np.ascontiguousarray(w_proj.T)
    bias2 = np.ascontiguousarray(b_proj.reshape(2, 128).T)
    in_maps = []
    for core in range(8):
        b, half = divmod(core, 2)
        n0 = half * NQ
        x_rot = np.concatenate([x2[b][:, n0:], x2[b][:, :n0]], axis=1)
        in_maps.append({
            "x": np.ascontiguousarray(x_rot),
            "wq": wq_t, "wk": wk_t, "wv": wv_t, "wp": wp_t,
            "bias": bias2, "ident": _IDENT,
        })
    return in_maps


def kernel(x, w_qkv, w_proj, b_proj):
    x = np.asarray(x, np.float32)
    w_qkv = np.asarray(w_qkv, np.float32)
    w_proj = np.asarray(w_proj, np.float32)
    b_proj = np.asarray(b_proj, np.float32)

    nc = _get_program()
    in_maps = make_in_maps(x, w_qkv, w_proj, b_proj)
    res = run_bass_kernel_spmd(nc, in_maps, list(range(8)))

    y = np.empty((B, C, N), np.float32)
    for core in range(8):
        b, half = divmod(core, 2)
        n0 = half * NQ
        y[b][:, n0:n0 + NQ] = res.results[core]["y"]
    return y.reshape(B, C, H, W)


# revision 24
# speedup vs baseline: 2.0211x; 1.0273x over previous
"""AttentionBlock (1x1-conv QKV + 4-head softmax attention + 1x1-conv proj)
on 8 Trainium2 NeuronCores.

Sharding: data-parallel over (batch b, query-half h) -> 8 shards. Each core
gets x rotated so its 2048 query columns are always columns 0:2048 (key order
is a permutation, which softmax-attention is invariant to), computes
qkv projections, 4-head attention for its half of the queries, and the output
projection for its [256, 2048] output slice. No collectives.

v2 structure (cost-model aware: matmul cost = streamed rhs columns):
  - scores S^T = K^T Q in f32r, 256-col tiles (full-rate), keys-major PSUM
  - exp split between Act (native Exp, scale=16) and DVE (custom single-instr
    quartic: (q1(x)*q2(x))^16 ~ 24^16 * e^(16x); the 24^16 scale cancels in
    softmax since rowsums are computed from the same values). The engine
    assignment is per-(nt, column) so every softmax row is consistent.
  - attn@V in O-form: out[query, dh] with rhs=[V_h | ones] so rowsums ride
    along as a 65th column; 65-col bf16 matmuls (128-partition output).
  - normalization per 128-query chunk on DVE (reciprocal + stride-0-broadcast
    tensor_tensor), then PE transposes O back to channel-major for the
    output projection.
  - f32 -> f32r via bitcast (no conversion copies).
"""
import os
import sys

sys.path.insert(0, '/opt/trn_rl_repo')

import numpy as np
from contextlib import ExitStack

from concourse import bass, bacc, mybir
import concourse.tile as tile
from concourse import dve_ops
from concourse.dve_ops import DveOp, OPS, CUSTOM_DVE_SPECS, _SUB_OPCODE_FOR_NAME
from concourse.dve_spec import Spec, Src0, C0, C1, C2, C3, lower, sq, _spill_c3_to_src1
from concourse.dve_uop import DveOpSpec
from concourse.bass_utils import run_bass_kernel_spmd

F32 = mybir.dt.float32
F32R = mybir.dt.float32r
BF16 = mybir.dt.bfloat16
ActFn = mybir.ActivationFunctionType

B, C, H, W = 4, 256, 64, 64
HEADS, DH = 4, 64
N = H * W            # 4096 keys
NQ = N // 2          # 2048 queries per core
NT = 256             # phase-2 query tile
N_NT = NQ // NT      # 8
N_MC = N // 128      # 32 key chunks
VSTR = HEADS * (DH + 1)  # 260: per-mc vT stride ([V_h | ones] x 4 heads)

# exp(16t) * 24^16 ~ [(t^2 + c0 t + c1)(t^2 + c2 t + c3)]^16 for t in
# [-0.625, 0.625] (score x = 16t in [-10, 10]); max rel err ~9e-4. The
# 24^16 factor cancels in softmax normalization. Split into two DVE
# instructions: EXPQ2A computes P^2 (quartic + one square, 8 ALU ops),
# EXPQ2B cubes the squaring three more times ((P^2)^8 = P^16).
EQ = (0.5504330780327099, 6.148042182109957,
      3.5525352677618507, 3.903596315668177)

# Act exp column count (0..1024) per (nt, pair) slot; rest go to the DVE
# pipeline. Balanced per-mc: pair0 pure Act, pair1 split so both engines
# carry equal exp load concurrently (Act ~1.54us/mc == DVE ~1.54us/mc).
EXP_ACOLS = [int(v) for v in os.environ.get(
    "EXP_ACOLS", "740,740,740,740,740,740,740,740").split(",")]
assert len(EXP_ACOLS) == 8


def _ref_expq2a(in0, in1, c0, c1, c2):
    x = in0.astype(np.float32)
    c3 = in1.astype(np.float32) if isinstance(in1, np.ndarray) else np.float32(in1)
    p = (((x + np.float32(c0)) * x + np.float32(c1))
         * ((x + np.float32(c2)) * x + c3)).astype(np.float32)
    return (p * p).astype(np.float32)


def _ref_expq2b(in0, in1, c0, c1, c2):
    p = in0.astype(np.float32)
    for _ in range(3):
        p = (p * p).astype(np.float32)
    return p


def _register(name, spec, rd1_en):
    row = dve_ops._CUSTOM_DVE_ROW_BASE + len(OPS)
    assert row < 0x20
    _SUB_OPCODE_FOR_NAME[name] = row
    shas = {}
    for ver in ("v3", "v4"):
        uops = lower(spec, ver=ver)
        shas[ver] = DveOpSpec(name=name, opcode=row, uops=uops, rd1_en=rd1_en).sha(ver)
    op = DveOp(name, spec, subdim=False, uops_sha=shas)
    OPS.append(op)
    CUSTOM_DVE_SPECS[name] = spec
    return op


def register_expq_op():
    if "EXPQ2A_ANT" in _SUB_OPCODE_FOR_NAME:
        a = next(op for op in OPS if op.name == "EXPQ2A_ANT")
        b = next(op for op in OPS if op.name == "EXPQ2B_ANT")
        return a, b
    x = Src0
    body_a = _spill_c3_to_src1(
        sq(((x + C0) * x + C1) * ((x + C2) * x + C3)))
    op_a = _register("EXPQ2A_ANT", Spec(body=body_a, reference=_ref_expq2a), True)
    body_b = sq(sq(sq(x)))
    op_b = _register("EXPQ2B_ANT", Spec(body=body_b, reference=_ref_expq2b), False)
    return op_a, op_b


def _ap3(base_ap, dims):
    """Manual AP with the partition dim of base_ap plus custom free dims."""
    return bass.AP(tensor=base_ap.tensor, offset=base_ap.offset,
                   ap=[list(base_ap.ap[0])] + [list(d) for d in dims])


def build_program(expq_op):
    nc = bacc.Bacc(target_bir_lowering=False)

    x_d = nc.declare_dram_parameter("x", [C, N], F32R, isOutput=False)
    wq_d = nc.declare_dram_parameter("wq", [C, C], F32R, isOutput=False)
    wk_d = nc.declare_dram_parameter("wk", [C, C], F32R, isOutput=False)
    wv_d = nc.declare_dram_parameter("wv", [C, C], F32R, isOutput=False)
    wp_d = nc.declare_dram_parameter("wp", [C, C], F32R, isOutput=False)
    bias_d = nc.declare_dram_parameter("bias", [128, 2], F32, isOutput=False)
    id_d = nc.declare_dram_parameter("ident", [128, 128], F32R, isOutput=False)
    y_d = nc.declare_dram_parameter("y", [C, NQ], F32, isOutput=True)

    with tile.TileContext(nc) as tc, ExitStack() as ctx:
        sb = ctx.enter_context(tc.tile_pool(name="sb", bufs=1))
        pex = ctx.enter_context(tc.tile_pool(name="pex", bufs=3))
        pout = ctx.enter_context(tc.tile_pool(name="pout", bufs=2))
        ps = ctx.enter_context(tc.tile_pool(name="ps", bufs=1, space="PSUM"))

        # ---------------- loads (weights first so QKV can start early) -----
        XC = 512  # x DMA chunk width so phase 1 can start early
        w_sb = {}
        for name, dram in (("wq", wq_d), ("wk", wk_d), ("wv", wv_d), ("wp", wp_d)):
            tiles = []
            for kc in range(2):
                f = sb.tile([128, C], F32R, tag=f"{name}{kc}", name=f"{name}f{kc}")
                nc.sync.dma_start(out=f, in_=dram[kc * 128:(kc + 1) * 128, :])
                tiles.append(f)
            w_sb[name] = tiles
        x_f = [sb.tile([128, N], F32R, tag=f"xf{i}", name=f"xf{i}") for i in range(2)]
        for ch in range(N // XC):
            for kc in range(2):
                nc.sync.dma_start(out=x_f[kc][:, ch * XC:(ch + 1) * XC],
                                  in_=x_d[kc * 128:(kc + 1) * 128, ch * XC:(ch + 1) * XC])
        bias_sb = sb.tile([128, 2], F32, tag="bias")
        nc.sync.dma_start(out=bias_sb, in_=bias_d[:, :])
        id_sb = sb.tile([128, 128], F32R, tag="id")
        nc.sync.dma_start(out=id_sb, in_=id_d[:, :])

        c3_t = sb.tile([128, 1], F32, tag="c3")
        nc.vector.memset(c3_t, float(EQ[3]))

        def xr(kc, sl):
            return x_f[kc][:, sl]

        def wr(name, kc, oc):
            return w_sb[name][kc][:, oc * 128:(oc + 1) * 128]

        # ---------------- phase 1: qkv projections ----------------
        # per-head tiles, always at partition offset 0 (mixed-partition-offset
        # matmul operands crash the walrus/HW path)
        q_sb = [sb.tile([64, NQ], F32R, tag=f"q{h}", name=f"q_sb{h}") for h in range(4)]
        k_sb = [sb.tile([64, N], F32R, tag=f"k{h}", name=f"k_sb{h}") for h in range(4)]
        vT_sb = sb.tile([128, N_MC * VSTR], BF16, tag="vT")

        # ones columns of vT (col 64 + 65*h + 260*mc), written once on Pool
        ones_ap = _ap3(vT_sb[:, DH:DH + 1], [[VSTR, N_MC], [DH + 1, HEADS]])
        nc.gpsimd.memset(ones_ap, 1.0)

        evac_i = [0]

        def evac_copy(out_ap, in_ap):
            # alternate PSUM evacuations between Act and DVE
            eng = nc.scalar.copy if evac_i[0] % 2 == 0 else nc.vector.tensor_copy
            evac_i[0] += 1
            return eng(out_ap, in_ap)

        for oc in range(2):
            for t4 in range(4):
                pq = ps.tile([128, 512], F32, tag="st", bufs=3, name=f"pq{oc}_{t4}")
                sl = slice(t4 * 512, (t4 + 1) * 512)
                nc.tensor.matmul(out=pq[:, :], lhsT=wr("wq", 0, oc), rhs=xr(0, sl),
                                 start=True, stop=False)
                nc.tensor.matmul(out=pq[:, :], lhsT=wr("wq", 1, oc), rhs=xr(1, sl),
                                 start=False, stop=True)
                evac_copy(q_sb[2 * oc][:, sl], pq[0:64, :])
                evac_copy(q_sb[2 * oc + 1][:, sl], pq[64:128, :])
        for oc in range(2):
            for t8 in range(8):
                pk = ps.tile([128, 512], F32, tag="st", bufs=3, name=f"pk{oc}_{t8}")
                sl = slice(t8 * 512, (t8 + 1) * 512)
                nc.tensor.matmul(out=pk[:, :], lhsT=wr("wk", 0, oc), rhs=xr(0, sl),
                                 start=True, stop=False)
                nc.tensor.matmul(out=pk[:, :], lhsT=wr("wk", 1, oc), rhs=xr(1, sl),
                                 start=False, stop=True)
                evac_copy(k_sb[2 * oc][:, sl], pk[0:64, :])
                evac_copy(k_sb[2 * oc + 1][:, sl], pk[64:128, :])
        for mc in range(N_MC):
            pv = ps.tile([128, 256], F32, tag="st", bufs=3, name=f"pv{mc}")
            msl = slice(mc * 128, (mc + 1) * 128)
            nc.tensor.matmul(out=pv[:, :], lhsT=xr(0, msl), rhs=w_sb["wv"][0][:, :],
                             start=True, stop=False)
            nc.tensor.matmul(out=pv[:, :], lhsT=xr(1, msl), rhs=w_sb["wv"][1][:, :],
                             start=False, stop=True)
            # strided copy into the [V_h | ones] layout: col 65*h + d
            vout = _ap3(vT_sb[:, mc * VSTR:mc * VSTR + 1], [[DH + 1, HEADS], [1, DH]])
            vin = _ap3(pv[:, 0:1], [[DH, HEADS], [1, DH]])
            evac_copy(vout, vin)

        # ---------------- phase 2: attention ----------------
        o_n = sb.tile([128, 16 * 256], F32R, tag="on")   # normalized O, [q, c]
        out_sp = [sb.tile([128, NQ], F32R, tag=f"osp{oc}", name=f"osp{oc}") for oc in range(2)]

        op_a, op_b = expq_op
        for nt in range(N_NT):               # 256-query tiles
            qsl = slice(nt * NT, (nt + 1) * NT)
            O_ps = [ps.tile([128, 512], F32, tag="o", bufs=2, name=f"O{nt}_{qs}")
                    for qs in range(2)]
            for mc in range(N_MC):
                msl = slice(mc * 128, (mc + 1) * 128)
                # all 4 heads' scores for this (nt, mc) in one 2-bank tile;
                # triple-buffered so the exp WAR chain stays off the
                # critical path.
                pst = ps.tile([128, 1024], F32, tag="st", bufs=3,
                              name=f"pst{nt}_{mc}")
                for h in range(4):
                    # per-head operands at partition offset 0; two heads per
                    # 2KB PSUM bank: first starts the group (lazy-zeroing the
                    # bank), second stops it.
                    nc.tensor.matmul(out=pst[:, h * 256:(h + 1) * 256],
                                     lhsT=k_sb[h][:, msl],
                                     rhs=q_sb[h][:, qsl],
                                     start=(h % 2 == 0), stop=(h % 2 == 1))
                et = pex.tile([128, 1024], BF16, tag="et", name=f"et{nt}_{mc}")
                acols = EXP_ACOLS[nt]
                if acols > 0:
                    nc.scalar.activation(et[:, 0:acols], pst[:, 0:acols],
                                         ActFn.Exp, scale=16.0)
                if acols < 1024:
                    y1 = pex.tile([128, 1024], F32, tag="y1", name=f"y1{nt}_{mc}")
                    nc.vector._custom_dve(op_a, out=y1[:, acols:1024],
                                          in0=pst[:, acols:1024],
                                          in1=c3_t[:, :], s0=float(EQ[0]),
                                          s1=float(EQ[1]), imm2=float(EQ[2]))
                    # deprioritize the second stage so the scheduler slots the
                    # next tile's EXPQ2A into the A->B ack gap instead of
                    # idling the DVE on the y1 write-ack.
                    with tc.high_priority(-24):
                        nc.vector._custom_dve(op_b, out=et[:, acols:1024],
                                              in0=y1[:, acols:1024])
                first, last = mc == 0, mc == N_MC - 1
                for h in range(4):
                    for qs in range(2):
                        # one accumulation group per O bank: start only on the
                        # very first write (the zero-region covers all 4 heads'
                        # columns), stop only on the very last.
                        nc.tensor.matmul(
                            out=O_ps[qs][:, h * 128:h * 128 + DH + 1],
                            lhsT=et[:, h * 256 + qs * 128:h * 256 + qs * 128 + 128],
                            rhs=vT_sb[:, mc * VSTR + h * (DH + 1):mc * VSTR + (h + 1) * (DH + 1)],
                            start=(first and h == 0), stop=(last and h == 3))
            for qs in range(2):
                rcp = sb.tile([128, 4], F32, tag="rcp", bufs=2, name=f"rcp{nt}_{qs}")
                rs_ap = _ap3(O_ps[qs][:, DH:DH + 1], [[128, 4], [1, 1]])
                nc.vector.reciprocal_approx_fast(out=rcp[:, :], in_=rs_ap)
                qc = nt * 2 + qs
                o_out = _ap3(o_n[:, qc * 256:qc * 256 + 1], [[64, 4], [1, 64]])
                o_in = _ap3(O_ps[qs][:, 0:1], [[128, 4], [1, 64]])
                r_in = _ap3(rcp[:, 0:1], [[1, 4], [0, 64]])
                nc.vector.tensor_tensor(out=o_out, in0=o_in, in1=r_in,
                                        op=mybir.AluOpType.mult)
            if nt % 2 == 1:
                # transpose the last 4 qchunks back to channel-major and
                # project, pipelined with the next nt's attention.
                g = nt // 2
                sl = slice(g * 512, (g + 1) * 512)
                for cc in range(2):
                    psT = ps.tile([128, 512], F32R, tag="o", bufs=2,
                                  name=f"psT{g}_{cc}")
                    for j in range(4):
                        qc = g * 4 + j
                        nc.tensor.matmul(
                            out=psT[:, j * 128:(j + 1) * 128],
                            lhsT=o_n[:, qc * 256 + cc * 128:qc * 256 + cc * 128 + 128],
                            rhs=id_sb[:, :],
                            is_transpose=True, start=(j == 0), stop=(j == 3))
                    evac_copy(out_sp[cc][:, sl], psT[:, :])
                for oc in range(2):
                    py = ps.tile([128, 512], F32, tag="o", bufs=2, name=f"py{oc}_{g}")
                    nc.tensor.matmul(out=py[:, :], lhsT=wr("wp", 0, oc),
                                     rhs=out_sp[0][:, sl],
                                     start=True, stop=False)
                    nc.tensor.matmul(out=py[:, :], lhsT=wr("wp", 1, oc),
                                     rhs=out_sp[1][:, sl],
                                     start=False, stop=True)
                    y_sb = pout.tile([128, 512], F32, tag="y", name=f"y_sb{oc}_{g}")
                    nc.vector.tensor_scalar_add(y_sb[:, :], py[:, :],
                                                bias_sb[:, oc:oc + 1])
                    nc.sync.dma_start(out=y_d[oc * 128:(oc + 1) * 128, sl],
                                      in_=y_sb[:, :])

    nc.compile()
    return nc


_CACHE = {}


def _get_program():
    if "nc" not in _CACHE:
        op = register_expq_op()
        _CACHE["nc"] = build_program(op)
    return _CACHE["nc"]


_IDENT = np.eye(128, dtype=np.float32)


def make_in_maps(x, w_qkv, w_proj, b_proj):
    x2 = x.reshape(B, C, N)
    wq_t = np.ascontiguousarray((w_qkv[0:C] / 128.0).T)
    wk_t = np.ascontiguousarray(w_qkv[C:2 * C].T)
    wv_t = np.ascontiguousarray(w_qkv[2 * C:3 * C].T)
    wp_t = np.ascontiguousarray(w_proj.T)
    bias2 = np.ascontiguousarray(b_proj.reshape(2, 128).T)
    in_maps = []
    for core in range(8):
        b, half = divmod(core, 2)
        n0 = half * NQ
        x_rot = np.concatenate([x2[b][:, n0:], x2[b][:, :n0]], axis=1)
        in_maps.append({
            "x": np.ascontiguousarray(x_rot),
            "wq": wq_t, "wk": wk_t, "wv": wv_t, "wp": wp_t,
            "bias": bias2, "ident": _IDENT,
        })
    return in_maps


def kernel(x, w_qkv, w_proj, b_proj):
    x = np.asarray(x, np.float32)
    w_qkv = np.asarray(w_qkv, np.float32)
    w_proj = np.asarray(w_proj, np.float32)
    b_proj = np.asarray(b_proj, np.float32)

    nc = _get_program()
    in_maps = make_in_maps(x, w_qkv, w_proj, b_proj)
    res = run_bass_kernel_spmd(nc, in_maps, list(range(8)))

    y = np.empty((B, C, N), np.float32)
    for core in range(8):
        b, half = divmod(core, 2)
        n0 = half * NQ
        y[b][:, n0:n0 + NQ] = res.results[core]["y"]
    return y.reshape(B, C, H, W)


# revision 25
# speedup vs baseline: 2.0824x; 1.0304x over previous
"""AttentionBlock (1x1-conv QKV + 4-head softmax attention + 1x1-conv proj)
on 8 Trainium2 NeuronCores.

Sharding: data-parallel over (batch b, query-half h) -> 8 shards. Each core
gets x rotated so its 2048 query columns are always columns 0:2048 (key order
is a permutation, which softmax-attention is invariant to), computes
qkv projections, 4-head attention for its half of the queries, and the output
projection for its [256, 2048] output slice. No collectives.

v2 structure (cost-model aware: matmul cost = streamed rhs columns):
  - scores S^T = K^T Q in f32r, 256-col tiles (full-rate), keys-major PSUM
  - exp split between Act (native Exp, scale=16) and DVE (custom single-instr
    quartic: (q1(x)*q2(x))^16 ~ 24^16 * e^(16x); the 24^16 scale cancels in
    softmax since rowsums are computed from the same values). The engine
    assignment is per-(nt, column) so every softmax row is consistent.
  - attn@V in O-form: out[query, dh] with rhs=[V_h | ones] so rowsums ride
    along as a 65th column; 65-col bf16 matmuls (128-partition output).
  - normalization per 128-query chunk on DVE (reciprocal + stride-0-broadcast
    tensor_tensor), then PE transposes O back to channel-major for the
    output projection.
  - f32 -> f32r via bitcast (no conversion copies).
"""
import os
import sys

sys.path.insert(0, '/opt/trn_rl_repo')

import numpy as np
from contextlib import ExitStack

from concourse import bass, bacc, mybir
import concourse.tile as tile
from concourse import dve_ops
from concourse.dve_ops import DveOp, OPS, CUSTOM_DVE_SPECS, _SUB_OPCODE_FOR_NAME
from concourse.dve_spec import Spec, Src0, C0, C1, C2, C3, lower, sq, _spill_c3_to_src1
from concourse.dve_uop import DveOpSpec
from concourse.bass_utils import run_bass_kernel_spmd

F32 = mybir.dt.float32
F32R = mybir.dt.float32r
BF16 = mybir.dt.bfloat16
ActFn = mybir.ActivationFunctionType

B, C, H, W = 4, 256, 64, 64
HEADS, DH = 4, 64
N = H * W            # 4096 keys
NQ = N // 2          # 2048 queries per core
NT = 256             # phase-2 query tile
N_NT = NQ // NT      # 8
N_MC = N // 128      # 32 key chunks
VSTR = HEADS * (DH + 1)  # 260: per-mc vT stride ([V_h | ones] x 4 heads)

# exp(16t) * 24^16 ~ [(t^2 + c0 t + c1)(t^2 + c2 t + c3)]^16 for t in
# [-0.625, 0.625] (score x = 16t in [-10, 10]); max rel err ~9e-4. The
# 24^16 factor cancels in softmax normalization. Split into two DVE
# instructions: EXPQ2A computes P^2 (quartic + one square, 8 ALU ops),
# EXPQ2B cubes the squaring three more times ((P^2)^8 = P^16).
EQ = (0.5504330780327099, 6.148042182109957,
      3.5525352677618507, 3.903596315668177)

# Act exp column count (0..1024) per (nt, pair) slot; rest go to the DVE
# pipeline. Balanced per-mc: pair0 pure Act, pair1 split so both engines
# carry equal exp load concurrently (Act ~1.54us/mc == DVE ~1.54us/mc).
EXP_ACOLS = [int(v) for v in os.environ.get(
    "EXP_ACOLS", "750,750,750,750,750,750,750,750").split(",")]
assert len(EXP_ACOLS) == 8


def _ref_expq2a(in0, in1, c0, c1, c2):
    x = in0.astype(np.float32)
    c3 = in1.astype(np.float32) if isinstance(in1, np.ndarray) else np.float32(in1)
    p = (((x + np.float32(c0)) * x + np.float32(c1))
         * ((x + np.float32(c2)) * x + c3)).astype(np.float32)
    return (p * p).astype(np.float32)


def _ref_expq2b(in0, in1, c0, c1, c2):
    p = in0.astype(np.float32)
    for _ in range(3):
        p = (p * p).astype(np.float32)
    return p


def _register(name, spec, rd1_en):
    row = dve_ops._CUSTOM_DVE_ROW_BASE + len(OPS)
    assert row < 0x20
    _SUB_OPCODE_FOR_NAME[name] = row
    shas = {}
    for ver in ("v3", "v4"):
        uops = lower(spec, ver=ver)
        shas[ver] = DveOpSpec(name=name, opcode=row, uops=uops, rd1_en=rd1_en).sha(ver)
    op = DveOp(name, spec, subdim=False, uops_sha=shas)
    OPS.append(op)
    CUSTOM_DVE_SPECS[name] = spec
    return op


def register_expq_op():
    if "EXPQ2A_ANT" in _SUB_OPCODE_FOR_NAME:
        a = next(op for op in OPS if op.name == "EXPQ2A_ANT")
        b = next(op for op in OPS if op.name == "EXPQ2B_ANT")
        return a, b
    x = Src0
    body_a = _spill_c3_to_src1(
        sq(((x + C0) * x + C1) * ((x + C2) * x + C3)))
    op_a = _register("EXPQ2A_ANT", Spec(body=body_a, reference=_ref_expq2a), True)
    body_b = sq(sq(sq(x)))
    op_b = _register("EXPQ2B_ANT", Spec(body=body_b, reference=_ref_expq2b), False)
    return op_a, op_b


def _ap3(base_ap, dims):
    """Manual AP with the partition dim of base_ap plus custom free dims."""
    return bass.AP(tensor=base_ap.tensor, offset=base_ap.offset,
                   ap=[list(base_ap.ap[0])] + [list(d) for d in dims])


def build_program(expq_op):
    nc = bacc.Bacc(target_bir_lowering=False)

    x_d = nc.declare_dram_parameter("x", [C, N], F32R, isOutput=False)
    wq_d = nc.declare_dram_parameter("wq", [C, C], F32R, isOutput=False)
    wk_d = nc.declare_dram_parameter("wk", [C, C], F32R, isOutput=False)
    wv_d = nc.declare_dram_parameter("wv", [C, C], F32R, isOutput=False)
    wp_d = nc.declare_dram_parameter("wp", [C, C], F32R, isOutput=False)
    bias_d = nc.declare_dram_parameter("bias", [128, 2], F32, isOutput=False)
    id_d = nc.declare_dram_parameter("ident", [128, 128], F32R, isOutput=False)
    y_d = nc.declare_dram_parameter("y", [C, NQ], F32, isOutput=True)

    with tile.TileContext(nc) as tc, ExitStack() as ctx:
        sb = ctx.enter_context(tc.tile_pool(name="sb", bufs=1))
        pex = ctx.enter_context(tc.tile_pool(name="pex", bufs=3))
        pout = ctx.enter_context(tc.tile_pool(name="pout", bufs=2))
        ps = ctx.enter_context(tc.tile_pool(name="ps", bufs=1, space="PSUM"))

        # ---------------- loads (weights first so QKV can start early) -----
        XC = 512  # x DMA chunk width so phase 1 can start early
        w_sb = {}
        for name, dram in (("wq", wq_d), ("wk", wk_d), ("wv", wv_d), ("wp", wp_d)):
            tiles = []
            for kc in range(2):
                f = sb.tile([128, C], F32R, tag=f"{name}{kc}", name=f"{name}f{kc}")
                nc.sync.dma_start(out=f, in_=dram[kc * 128:(kc + 1) * 128, :])
                tiles.append(f)
            w_sb[name] = tiles
        x_f = [sb.tile([128, N], F32R, tag=f"xf{i}", name=f"xf{i}") for i in range(2)]
        for ch in range(N // XC):
            for kc in range(2):
                nc.sync.dma_start(out=x_f[kc][:, ch * XC:(ch + 1) * XC],
                                  in_=x_d[kc * 128:(kc + 1) * 128, ch * XC:(ch + 1) * XC])
        bias_sb = sb.tile([128, 2], F32, tag="bias")
        nc.sync.dma_start(out=bias_sb, in_=bias_d[:, :])
        id_sb = sb.tile([128, 128], F32R, tag="id")
        nc.sync.dma_start(out=id_sb, in_=id_d[:, :])

        c3_t = sb.tile([128, 1], F32, tag="c3")
        nc.vector.memset(c3_t, float(EQ[3]))

        def xr(kc, sl):
            return x_f[kc][:, sl]

        def wr(name, kc, oc):
            return w_sb[name][kc][:, oc * 128:(oc + 1) * 128]

        # ---------------- phase 1: qkv projections ----------------
        # per-head tiles, always at partition offset 0 (mixed-partition-offset
        # matmul operands crash the walrus/HW path)
        q_sb = [sb.tile([64, NQ], F32R, tag=f"q{h}", name=f"q_sb{h}") for h in range(4)]
        k_sb = [sb.tile([64, N], F32R, tag=f"k{h}", name=f"k_sb{h}") for h in range(4)]
        vT_sb = sb.tile([128, N_MC * VSTR], BF16, tag="vT")

        # ones columns of vT (col 64 + 65*h + 260*mc), written once on Pool
        ones_ap = _ap3(vT_sb[:, DH:DH + 1], [[VSTR, N_MC], [DH + 1, HEADS]])
        nc.gpsimd.memset(ones_ap, 1.0)

        evac_i = [0]

        def evac_copy(out_ap, in_ap):
            # alternate PSUM evacuations between Act and DVE
            eng = nc.scalar.copy if evac_i[0] % 2 == 0 else nc.vector.tensor_copy
            evac_i[0] += 1
            return eng(out_ap, in_ap)

        for oc in range(2):
            for t4 in range(4):
                pq = ps.tile([128, 512], F32, tag="st", bufs=3, name=f"pq{oc}_{t4}")
                sl = slice(t4 * 512, (t4 + 1) * 512)
                nc.tensor.matmul(out=pq[:, :], lhsT=wr("wq", 0, oc), rhs=xr(0, sl),
                                 start=True, stop=False)
                nc.tensor.matmul(out=pq[:, :], lhsT=wr("wq", 1, oc), rhs=xr(1, sl),
                                 start=False, stop=True)
                evac_copy(q_sb[2 * oc][:, sl], pq[0:64, :])
                evac_copy(q_sb[2 * oc + 1][:, sl], pq[64:128, :])
        for oc in range(2):
            for t8 in range(8):
                pk = ps.tile([128, 512], F32, tag="st", bufs=3, name=f"pk{oc}_{t8}")
                sl = slice(t8 * 512, (t8 + 1) * 512)
                nc.tensor.matmul(out=pk[:, :], lhsT=wr("wk", 0, oc), rhs=xr(0, sl),
                                 start=True, stop=False)
                nc.tensor.matmul(out=pk[:, :], lhsT=wr("wk", 1, oc), rhs=xr(1, sl),
                                 start=False, stop=True)
                evac_copy(k_sb[2 * oc][:, sl], pk[0:64, :])
                evac_copy(k_sb[2 * oc + 1][:, sl], pk[64:128, :])
        for mc in range(N_MC):
            pv = ps.tile([128, 256], F32, tag="st", bufs=3, name=f"pv{mc}")
            msl = slice(mc * 128, (mc + 1) * 128)
            nc.tensor.matmul(out=pv[:, :], lhsT=xr(0, msl), rhs=w_sb["wv"][0][:, :],
                             start=True, stop=False)
            nc.tensor.matmul(out=pv[:, :], lhsT=xr(1, msl), rhs=w_sb["wv"][1][:, :],
                             start=False, stop=True)
            # strided copy into the [V_h | ones] layout: col 65*h + d
            vout = _ap3(vT_sb[:, mc * VSTR:mc * VSTR + 1], [[DH + 1, HEADS], [1, DH]])
            vin = _ap3(pv[:, 0:1], [[DH, HEADS], [1, DH]])
            evac_copy(vout, vin)

        # ---------------- phase 2: attention ----------------
        o_n = sb.tile([128, 16 * 256], F32R, tag="on")   # normalized O, [q, c]
        out_sp = [sb.tile([128, NQ], F32R, tag=f"osp{oc}", name=f"osp{oc}") for oc in range(2)]

        op_a, op_b = expq_op
        for nt in range(N_NT):               # 256-query tiles
            qsl = slice(nt * NT, (nt + 1) * NT)
            O_ps = [ps.tile([128, 512], F32, tag="o", bufs=2, name=f"O{nt}_{qs}")
                    for qs in range(2)]
            for mc in range(N_MC):
                msl = slice(mc * 128, (mc + 1) * 128)
                # all 4 heads' scores for this (nt, mc) in one 2-bank tile;
                # triple-buffered so the exp WAR chain stays off the
                # critical path.
                pst = ps.tile([128, 1024], F32, tag="st", bufs=3,
                              name=f"pst{nt}_{mc}")
                for h in range(4):
                    # per-head operands at partition offset 0; two heads per
                    # 2KB PSUM bank: first starts the group (lazy-zeroing the
                    # bank), second stops it.
                    nc.tensor.matmul(out=pst[:, h * 256:(h + 1) * 256],
                                     lhsT=k_sb[h][:, msl],
                                     rhs=q_sb[h][:, qsl],
                                     start=(h % 2 == 0), stop=(h % 2 == 1))
                et = pex.tile([128, 1024], BF16, tag="et", name=f"et{nt}_{mc}")
                acols = EXP_ACOLS[nt]
                if acols > 0:
                    nc.scalar.activation(et[:, 0:acols], pst[:, 0:acols],
                                         ActFn.Exp, scale=16.0)
                if acols < 1024:
                    y1 = pex.tile([128, 1024], F32, tag="y1", name=f"y1{nt}_{mc}")
                    nc.vector._custom_dve(op_a, out=y1[:, acols:1024],
                                          in0=pst[:, acols:1024],
                                          in1=c3_t[:, :], s0=float(EQ[0]),
                                          s1=float(EQ[1]), imm2=float(EQ[2]))
                    # deprioritize the second stage so the scheduler slots the
                    # next tile's EXPQ2A into the A->B ack gap instead of
                    # idling the DVE on the y1 write-ack.
                    with tc.high_priority(-24):
                        nc.vector._custom_dve(op_b, out=et[:, acols:1024],
                                              in0=y1[:, acols:1024])
                first, last = mc == 0, mc == N_MC - 1
                for h in range(4):
                    for qs in range(2):
                        # one accumulation group per O bank: start only on the
                        # very first write (the zero-region covers all 4 heads'
                        # columns), stop only on the very last.
                        nc.tensor.matmul(
                            out=O_ps[qs][:, h * 128:h * 128 + DH + 1],
                            lhsT=et[:, h * 256 + qs * 128:h * 256 + qs * 128 + 128],
                            rhs=vT_sb[:, mc * VSTR + h * (DH + 1):mc * VSTR + (h + 1) * (DH + 1)],
                            start=(first and h == 0), stop=(last and h == 3))
            for qs in range(2):
                rcp = sb.tile([128, 4], F32, tag="rcp", bufs=2, name=f"rcp{nt}_{qs}")
                rs_ap = _ap3(O_ps[qs][:, DH:DH + 1], [[128, 4], [1, 1]])
                nc.vector.reciprocal_approx_fast(out=rcp[:, :], in_=rs_ap)
                qc = nt * 2 + qs
                o_out = _ap3(o_n[:, qc * 256:qc * 256 + 1], [[64, 4], [1, 64]])
                o_in = _ap3(O_ps[qs][:, 0:1], [[128, 4], [1, 64]])
                r_in = _ap3(rcp[:, 0:1], [[1, 4], [0, 64]])
                nc.vector.tensor_tensor(out=o_out, in0=o_in, in1=r_in,
                                        op=mybir.AluOpType.mult)
            if nt % 2 == 1:
                # transpose the last 4 qchunks back to channel-major and
                # project, pipelined with the next nt's attention.
                g = nt // 2
                sl = slice(g * 512, (g + 1) * 512)
                for cc in range(2):
                    psT = ps.tile([128, 512], F32R, tag="o", bufs=2,
                                  name=f"psT{g}_{cc}")
                    for j in range(4):
                        qc = g * 4 + j
                        nc.tensor.matmul(
                            out=psT[:, j * 128:(j + 1) * 128],
                            lhsT=o_n[:, qc * 256 + cc * 128:qc * 256 + cc * 128 + 128],
                            rhs=id_sb[:, :],
                            is_transpose=True, start=(j == 0), stop=(j == 3))
                    nc.scalar.copy(out_sp[cc][:, sl], psT[:, :])
                for oc in range(2):
                    py = ps.tile([128, 512], F32, tag="o", bufs=2, name=f"py{oc}_{g}")
                    nc.tensor.matmul(out=py[:, :], lhsT=wr("wp", 0, oc),
                                     rhs=out_sp[0][:, sl],
                                     start=True, stop=False)
                    nc.tensor.matmul(out=py[:, :], lhsT=wr("wp", 1, oc),
                                     rhs=out_sp[1][:, sl],
                                     start=False, stop=True)
                    y_sb = pout.tile([128, 512], F32, tag="y", name=f"y_sb{oc}_{g}")
                    nc.vector.tensor_scalar_add(y_sb[:, :], py[:, :],
                                                bias_sb[:, oc:oc + 1])
                    nc.sync.dma_start(out=y_d[oc * 128:(oc + 1) * 128, sl],
                                      in_=y_sb[:, :])

    nc.compile()
    return nc


_CACHE = {}


def _get_program():
    if "nc" not in _CACHE:
        op = register_expq_op()
        _CACHE["nc"] = build_program(op)
    return _CACHE["nc"]


_IDENT = np.eye(128, dtype=np.float32)


def make_in_maps(x, w_qkv, w_proj, b_proj):
    x2 = x.reshape(B, C, N)
    wq_t = np.ascontiguousarray((w_qkv[0:C] / 128.0).T)
    wk_t = np.ascontiguousarray(w_qkv[C:2 * C].T)
    wv_t = np.ascontiguousarray(w_qkv[2 * C:3 * C].T)
    wp_t = np.ascontiguousarray(w_proj.T)
    bias2 = np.ascontiguousarray(b_proj.reshape(2, 128).T)
    in_maps = []
    for core in range(8):
        b, half = divmod(core, 2)
        n0 = half * NQ
        x_rot = np.concatenate([x2[b][:, n0:], x2[b][:, :n0]], axis=1)
        in_maps.append({
            "x": np.ascontiguousarray(x_rot),
            "wq": wq_t, "wk": wk_t, "wv": wv_t, "wp": wp_t,
            "bias": bias2, "ident": _IDENT,
        })
    return in_maps


def kernel(x, w_qkv, w_proj, b_proj):
    x = np.asarray(x, np.float32)
    w_qkv = np.asarray(w_qkv, np.float32)
    w_proj = np.asarray(w_proj, np.float32)
    b_proj = np.asarray(b_proj, np.float32)

    nc = _get_program()
    in_maps = make_in_maps(x, w_qkv, w_proj, b_proj)
    res = run_bass_kernel_spmd(nc, in_maps, list(range(8)))

    y = np.empty((B, C, N), np.float32)
    for core in range(8):
        b, half = divmod(core, 2)
        n0 = half * NQ
        y[b][:, n0:n0 + NQ] = res.results[core]["y"]
    return y.reshape(B, C, H, W)


# revision 26
# speedup vs baseline: 2.0861x; 1.0018x over previous
"""AttentionBlock (1x1-conv QKV + 4-head softmax attention + 1x1-conv proj)
on 8 Trainium2 NeuronCores.

Sharding: data-parallel over (batch b, query-half h) -> 8 shards. Each core
gets x rotated so its 2048 query columns are always columns 0:2048 (key order
is a permutation, which softmax-attention is invariant to), computes
qkv projections, 4-head attention for its half of the queries, and the output
projection for its [256, 2048] output slice. No collectives.

v2 structure (cost-model aware: matmul cost = streamed rhs columns):
  - scores S^T = K^T Q in f32r (1/16 pre-folded into w_q on the host so the
    DVE exp polynomial stays in range), 256-query tiles, keys-major PSUM.
    q/k live in per-head partition-0 tiles: mixed-partition-offset matmul
    operands crash the walrus/HW path.
  - exp split by column between Act (native Exp, scale=16) and a two-instr
    DVE pipeline (EXPQ2A: minimax-quartic^2 of exp(16t)*24, EXPQ2B: ^8),
    24^16 cancels in softmax since rowsums come from the same values. The
    per-nt column split keeps every softmax row on one implementation.
  - attn@V in O-form: out[query, dh] with rhs=[V_h | ones] so rowsums ride
    along as a 65th column; 65-col bf16 matmuls with 128-query-partition
    output (2x fewer streamed columns than the channel-major form). One
    PSUM accumulation group per 2KB bank (lazy zero-region semantics).
  - normalization per 128-query chunk on DVE (reciprocal + stride-0-broadcast
    tensor_tensor), then PE transposes O back to channel-major (identity
    rhs) for the output projection, pipelined per 512-query group.
  - f32r DRAM params + f32r SBUF tiles everywhere (no conversion copies);
    PSUM triple-buffered scores so the exp WAR chain stays off the critical
    path; EXPQ2B deprioritized so the next tile's EXPQ2A fills its ack gap.
"""
import os
import sys

sys.path.insert(0, '/opt/trn_rl_repo')

import numpy as np
from contextlib import ExitStack

from concourse import bass, bacc, mybir
import concourse.tile as tile
from concourse import dve_ops
from concourse.dve_ops import DveOp, OPS, CUSTOM_DVE_SPECS, _SUB_OPCODE_FOR_NAME
from concourse.dve_spec import Spec, Src0, C0, C1, C2, C3, lower, sq, _spill_c3_to_src1
from concourse.dve_uop import DveOpSpec
from concourse.bass_utils import run_bass_kernel_spmd

F32 = mybir.dt.float32
F32R = mybir.dt.float32r
BF16 = mybir.dt.bfloat16
ActFn = mybir.ActivationFunctionType

B, C, H, W = 4, 256, 64, 64
HEADS, DH = 4, 64
N = H * W            # 4096 keys
NQ = N // 2          # 2048 queries per core
NT = 256             # phase-2 query tile
N_NT = NQ // NT      # 8
N_MC = N // 128      # 32 key chunks
VSTR = HEADS * (DH + 1)  # 260: per-mc vT stride ([V_h | ones] x 4 heads)

# exp(16t) * 24^16 ~ [(t^2 + c0 t + c1)(t^2 + c2 t + c3)]^16 for t in
# [-0.625, 0.625] (score x = 16t in [-10, 10]); max rel err ~9e-4. The
# 24^16 factor cancels in softmax normalization. Split into two DVE
# instructions: EXPQ2A computes P^2 (quartic + one square, 8 ALU ops),
# EXPQ2B cubes the squaring three more times ((P^2)^8 = P^16).
EQ = (0.5504330780327099, 6.148042182109957,
      3.5525352677618507, 3.903596315668177)

# Act exp column count (0..1024) per (nt, pair) slot; rest go to the DVE
# pipeline. Balanced per-mc: pair0 pure Act, pair1 split so both engines
# carry equal exp load concurrently (Act ~1.54us/mc == DVE ~1.54us/mc).
EXP_ACOLS = [int(v) for v in os.environ.get(
    "EXP_ACOLS", "765,765,765,765,765,765,765,765").split(",")]
assert len(EXP_ACOLS) == 8


def _ref_expq2a(in0, in1, c0, c1, c2):
    x = in0.astype(np.float32)
    c3 = in1.astype(np.float32) if isinstance(in1, np.ndarray) else np.float32(in1)
    p = (((x + np.float32(c0)) * x + np.float32(c1))
         * ((x + np.float32(c2)) * x + c3)).astype(np.float32)
    return (p * p).astype(np.float32)


def _ref_expq2b(in0, in1, c0, c1, c2):
    p = in0.astype(np.float32)
    for _ in range(3):
        p = (p * p).astype(np.float32)
    return p


def _register(name, spec, rd1_en):
    row = dve_ops._CUSTOM_DVE_ROW_BASE + len(OPS)
    assert row < 0x20
    _SUB_OPCODE_FOR_NAME[name] = row
    shas = {}
    for ver in ("v3", "v4"):
        uops = lower(spec, ver=ver)
        shas[ver] = DveOpSpec(name=name, opcode=row, uops=uops, rd1_en=rd1_en).sha(ver)
    op = DveOp(name, spec, subdim=False, uops_sha=shas)
    OPS.append(op)
    CUSTOM_DVE_SPECS[name] = spec
    return op


def register_expq_op():
    if "EXPQ2A_ANT" in _SUB_OPCODE_FOR_NAME:
        a = next(op for op in OPS if op.name == "EXPQ2A_ANT")
        b = next(op for op in OPS if op.name == "EXPQ2B_ANT")
        return a, b
    x = Src0
    body_a = _spill_c3_to_src1(
        sq(((x + C0) * x + C1) * ((x + C2) * x + C3)))
    op_a = _register("EXPQ2A_ANT", Spec(body=body_a, reference=_ref_expq2a), True)
    body_b = sq(sq(sq(x)))
    op_b = _register("EXPQ2B_ANT", Spec(body=body_b, reference=_ref_expq2b), False)
    return op_a, op_b


def _ap3(base_ap, dims):
    """Manual AP with the partition dim of base_ap plus custom free dims."""
    return bass.AP(tensor=base_ap.tensor, offset=base_ap.offset,
                   ap=[list(base_ap.ap[0])] + [list(d) for d in dims])


def build_program(expq_op):
    nc = bacc.Bacc(target_bir_lowering=False)

    x_d = nc.declare_dram_parameter("x", [C, N], F32R, isOutput=False)
    wq_d = nc.declare_dram_parameter("wq", [C, C], F32R, isOutput=False)
    wk_d = nc.declare_dram_parameter("wk", [C, C], F32R, isOutput=False)
    wv_d = nc.declare_dram_parameter("wv", [C, C], F32R, isOutput=False)
    wp_d = nc.declare_dram_parameter("wp", [C, C], F32R, isOutput=False)
    bias_d = nc.declare_dram_parameter("bias", [128, 2], F32, isOutput=False)
    id_d = nc.declare_dram_parameter("ident", [128, 128], F32R, isOutput=False)
    y_d = nc.declare_dram_parameter("y", [C, NQ], F32, isOutput=True)

    with tile.TileContext(nc) as tc, ExitStack() as ctx:
        sb = ctx.enter_context(tc.tile_pool(name="sb", bufs=1))
        pex = ctx.enter_context(tc.tile_pool(name="pex", bufs=3))
        pout = ctx.enter_context(tc.tile_pool(name="pout", bufs=2))
        ps = ctx.enter_context(tc.tile_pool(name="ps", bufs=1, space="PSUM"))

        # ---------------- loads (weights first so QKV can start early) -----
        XC = 512  # x DMA chunk width so phase 1 can start early
        w_sb = {}
        for name, dram in (("wq", wq_d), ("wk", wk_d), ("wv", wv_d), ("wp", wp_d)):
            tiles = []
            for kc in range(2):
                f = sb.tile([128, C], F32R, tag=f"{name}{kc}", name=f"{name}f{kc}")
                nc.sync.dma_start(out=f, in_=dram[kc * 128:(kc + 1) * 128, :])
                tiles.append(f)
            w_sb[name] = tiles
        x_f = [sb.tile([128, N], F32R, tag=f"xf{i}", name=f"xf{i}") for i in range(2)]
        for ch in range(N // XC):
            for kc in range(2):
                nc.sync.dma_start(out=x_f[kc][:, ch * XC:(ch + 1) * XC],
                                  in_=x_d[kc * 128:(kc + 1) * 128, ch * XC:(ch + 1) * XC])
        bias_sb = sb.tile([128, 2], F32, tag="bias")
        nc.sync.dma_start(out=bias_sb, in_=bias_d[:, :])
        id_sb = sb.tile([128, 128], F32R, tag="id")
        nc.sync.dma_start(out=id_sb, in_=id_d[:, :])

        c3_t = sb.tile([128, 1], F32, tag="c3")
        nc.vector.memset(c3_t, float(EQ[3]))

        def xr(kc, sl):
            return x_f[kc][:, sl]

        def wr(name, kc, oc):
            return w_sb[name][kc][:, oc * 128:(oc + 1) * 128]

        # ---------------- phase 1: qkv projections ----------------
        # per-head tiles, always at partition offset 0 (mixed-partition-offset
        # matmul operands crash the walrus/HW path)
        q_sb = [sb.tile([64, NQ], F32R, tag=f"q{h}", name=f"q_sb{h}") for h in range(4)]
        k_sb = [sb.tile([64, N], F32R, tag=f"k{h}", name=f"k_sb{h}") for h in range(4)]
        vT_sb = sb.tile([128, N_MC * VSTR], BF16, tag="vT")

        # ones columns of vT (col 64 + 65*h + 260*mc), written once on Pool
        ones_ap = _ap3(vT_sb[:, DH:DH + 1], [[VSTR, N_MC], [DH + 1, HEADS]])
        nc.gpsimd.memset(ones_ap, 1.0)

        evac_i = [0]

        def evac_copy(out_ap, in_ap):
            # alternate PSUM evacuations between Act and DVE
            eng = nc.scalar.copy if evac_i[0] % 2 == 0 else nc.vector.tensor_copy
            evac_i[0] += 1
            return eng(out_ap, in_ap)

        for oc in range(2):
            for t4 in range(4):
                pq = ps.tile([128, 512], F32, tag="st", bufs=3, name=f"pq{oc}_{t4}")
                sl = slice(t4 * 512, (t4 + 1) * 512)
                nc.tensor.matmul(out=pq[:, :], lhsT=wr("wq", 0, oc), rhs=xr(0, sl),
                                 start=True, stop=False)
                nc.tensor.matmul(out=pq[:, :], lhsT=wr("wq", 1, oc), rhs=xr(1, sl),
                                 start=False, stop=True)
                evac_copy(q_sb[2 * oc][:, sl], pq[0:64, :])
                evac_copy(q_sb[2 * oc + 1][:, sl], pq[64:128, :])
        for oc in range(2):
            for t8 in range(8):
                pk = ps.tile([128, 512], F32, tag="st", bufs=3, name=f"pk{oc}_{t8}")
                sl = slice(t8 * 512, (t8 + 1) * 512)
                nc.tensor.matmul(out=pk[:, :], lhsT=wr("wk", 0, oc), rhs=xr(0, sl),
                                 start=True, stop=False)
                nc.tensor.matmul(out=pk[:, :], lhsT=wr("wk", 1, oc), rhs=xr(1, sl),
                                 start=False, stop=True)
                evac_copy(k_sb[2 * oc][:, sl], pk[0:64, :])
                evac_copy(k_sb[2 * oc + 1][:, sl], pk[64:128, :])
        for mc in range(N_MC):
            pv = ps.tile([128, 256], F32, tag="st", bufs=3, name=f"pv{mc}")
            msl = slice(mc * 128, (mc + 1) * 128)
            nc.tensor.matmul(out=pv[:, :], lhsT=xr(0, msl), rhs=w_sb["wv"][0][:, :],
                             start=True, stop=False)
            nc.tensor.matmul(out=pv[:, :], lhsT=xr(1, msl), rhs=w_sb["wv"][1][:, :],
                             start=False, stop=True)
            # strided copy into the [V_h | ones] layout: col 65*h + d
            vout = _ap3(vT_sb[:, mc * VSTR:mc * VSTR + 1], [[DH + 1, HEADS], [1, DH]])
            vin = _ap3(pv[:, 0:1], [[DH, HEADS], [1, DH]])
            evac_copy(vout, vin)

        # ---------------- phase 2: attention ----------------
        o_n = sb.tile([128, 16 * 256], F32R, tag="on")   # normalized O, [q, c]
        out_sp = [sb.tile([128, NQ], F32R, tag=f"osp{oc}", name=f"osp{oc}") for oc in range(2)]

        op_a, op_b = expq_op
        for nt in range(N_NT):               # 256-query tiles
            qsl = slice(nt * NT, (nt + 1) * NT)
            O_ps = [ps.tile([128, 512], F32, tag="o", bufs=2, name=f"O{nt}_{qs}")
                    for qs in range(2)]
            for mc in range(N_MC):
                msl = slice(mc * 128, (mc + 1) * 128)
                # all 4 heads' scores for this (nt, mc) in one 2-bank tile;
                # triple-buffered so the exp WAR chain stays off the
                # critical path.
                pst = ps.tile([128, 1024], F32, tag="st", bufs=3,
                              name=f"pst{nt}_{mc}")
                for h in range(4):
                    # per-head operands at partition offset 0; two heads per
                    # 2KB PSUM bank: first starts the group (lazy-zeroing the
                    # bank), second stops it.
                    nc.tensor.matmul(out=pst[:, h * 256:(h + 1) * 256],
                                     lhsT=k_sb[h][:, msl],
                                     rhs=q_sb[h][:, qsl],
                                     start=(h % 2 == 0), stop=(h % 2 == 1))
                et = pex.tile([128, 1024], BF16, tag="et", name=f"et{nt}_{mc}")
                acols = EXP_ACOLS[nt]
                if acols > 0:
                    nc.scalar.activation(et[:, 0:acols], pst[:, 0:acols],
                                         ActFn.Exp, scale=16.0)
                if acols < 1024:
                    y1 = pex.tile([128, 1024], F32, tag="y1", name=f"y1{nt}_{mc}")
                    nc.vector._custom_dve(op_a, out=y1[:, acols:1024],
                                          in0=pst[:, acols:1024],
                                          in1=c3_t[:, :], s0=float(EQ[0]),
                                          s1=float(EQ[1]), imm2=float(EQ[2]))
                    # deprioritize the second stage so the scheduler slots the
                    # next tile's EXPQ2A into the A->B ack gap instead of
                    # idling the DVE on the y1 write-ack.
                    with tc.high_priority(-24):
                        nc.vector._custom_dve(op_b, out=et[:, acols:1024],
                                              in0=y1[:, acols:1024])
                first, last = mc == 0, mc == N_MC - 1
                for h in range(4):
                    for qs in range(2):
                        # one accumulation group per O bank: start only on the
                        # very first write (the zero-region covers all 4 heads'
                        # columns), stop only on the very last.
                        nc.tensor.matmul(
                            out=O_ps[qs][:, h * 128:h * 128 + DH + 1],
                            lhsT=et[:, h * 256 + qs * 128:h * 256 + qs * 128 + 128],
                            rhs=vT_sb[:, mc * VSTR + h * (DH + 1):mc * VSTR + (h + 1) * (DH + 1)],
                            start=(first and h == 0), stop=(last and h == 3))
            for qs in range(2):
                rcp = sb.tile([128, 4], F32, tag="rcp", bufs=2, name=f"rcp{nt}_{qs}")
                rs_ap = _ap3(O_ps[qs][:, DH:DH + 1], [[128, 4], [1, 1]])
                nc.vector.reciprocal_approx_fast(out=rcp[:, :], in_=rs_ap)
                qc = nt * 2 + qs
                o_out = _ap3(o_n[:, qc * 256:qc * 256 + 1], [[64, 4], [1, 64]])
                o_in = _ap3(O_ps[qs][:, 0:1], [[128, 4], [1, 64]])
                r_in = _ap3(rcp[:, 0:1], [[1, 4], [0, 64]])
                nc.vector.tensor_tensor(out=o_out, in0=o_in, in1=r_in,
                                        op=mybir.AluOpType.mult)
            if nt % 2 == 1:
                # transpose the last 4 qchunks back to channel-major and
                # project, pipelined with the next nt's attention.
                g = nt // 2
                sl = slice(g * 512, (g + 1) * 512)
                for cc in range(2):
                    psT = ps.tile([128, 512], F32R, tag="o", bufs=2,
                                  name=f"psT{g}_{cc}")
                    for j in range(4):
                        qc = g * 4 + j
                        nc.tensor.matmul(
                            out=psT[:, j * 128:(j + 1) * 128],
                            lhsT=o_n[:, qc * 256 + cc * 128:qc * 256 + cc * 128 + 128],
                            rhs=id_sb[:, :],
                            is_transpose=True, start=(j == 0), stop=(j == 3))
                    nc.scalar.copy(out_sp[cc][:, sl], psT[:, :])
                for oc in range(2):
                    py = ps.tile([128, 512], F32, tag="o", bufs=2, name=f"py{oc}_{g}")
                    nc.tensor.matmul(out=py[:, :], lhsT=wr("wp", 0, oc),
                                     rhs=out_sp[0][:, sl],
                                     start=True, stop=False)
                    nc.tensor.matmul(out=py[:, :], lhsT=wr("wp", 1, oc),
                                     rhs=out_sp[1][:, sl],
                                     start=False, stop=True)
                    y_sb = pout.tile([128, 512], F32, tag="y", name=f"y_sb{oc}_{g}")
                    nc.vector.tensor_scalar_add(y_sb[:, :], py[:, :],
                                                bias_sb[:, oc:oc + 1])
                    nc.sync.dma_start(out=y_d[oc * 128:(oc + 1) * 128, sl],
                                      in_=y_sb[:, :])

    nc.compile()
    return nc


_CACHE = {}


def _get_program():
    if "nc" not in _CACHE:
        op = register_expq_op()
        _CACHE["nc"] = build_program(op)
    return _CACHE["nc"]


_IDENT = np.eye(128, dtype=np.float32)


def make_in_maps(x, w_qkv, w_proj, b_proj):
    x2 = x.reshape(B, C, N)
    wq_t = np.ascontiguousarray((w_qkv[0:C] / 128.0).T)
    wk_t = np.ascontiguousarray(w_qkv[C:2 * C].T)
    wv_t = np.ascontiguousarray(w_qkv[2 * C:3 * C].T)
    wp_t = np.ascontiguousarray(w_proj.T)
    bias2 = np.ascontiguousarray(b_proj.reshape(2, 128).T)
    in_maps = []
    for core in range(8):
        b, half = divmod(core, 2)
        n0 = half * NQ
        x_rot = np.concatenate([x2[b][:, n0:], x2[b][:, :n0]], axis=1)
        in_maps.append({
            "x": np.ascontiguousarray(x_rot),
            "wq": wq_t, "wk": wk_t, "wv": wv_t, "wp": wp_t,
            "bias": bias2, "ident": _IDENT,
        })
    return in_maps


def kernel(x, w_qkv, w_proj, b_proj):
    x = np.asarray(x, np.float32)
    w_qkv = np.asarray(w_qkv, np.float32)
    w_proj = np.asarray(w_proj, np.float32)
    b_proj = np.asarray(b_proj, np.float32)

    nc = _get_program()
    in_maps = make_in_maps(x, w_qkv, w_proj, b_proj)
    res = run_bass_kernel_spmd(nc, in_maps, list(range(8)))

    y = np.empty((B, C, N), np.float32)
    for core in range(8):
        b, half = divmod(core, 2)
        n0 = half * NQ
        y[b][:, n0:n0 + NQ] = res.results[core]["y"]
    return y.reshape(B, C, H, W)
